# revision 1
# baseline (speedup 1.0000x reference)
"""Trainium2 Bass kernel for nn_BG_ALRT_62921270886438 (moe_routing).

Sharding: cores 0-3 replicate batch 0, cores 4-7 replicate batch 1 (the step
loop runs per-batch on every core with zero collectives); the lm_head matmul
is vocab-sharded 4 ways within each batch group. Exploits w_eff sparsity:
only layers with exp(-|depth - t|) > 0.15 are computed each step.

Self-contained: only numpy + the concourse toolchain on sys.path.
"""
import os

import numpy as np

import concourse.bacc as bacc
import concourse.tile as tile
from concourse import mybir
from concourse.alu_op_type import AluOpType
from concourse.bass_utils import run_bass_kernel_spmd

AF = mybir.ActivationFunctionType
F32 = mybir.dt.float32
F32R = mybir.dt.float32r

B, T, E, G, GD, L, N, V = 2, 256, 512, 8, 64, 8, 64, 50257
HD = GD // 2          # 32, rope half
NC = 8                # cores
VSH = 4               # vocab shards per batch group
VW = (V + VSH - 1) // VSH          # 12565 raw shard width
VQ = ((VW + 511) // 512) * 512     # 12800 padded shard width
EPS = float(np.finfo(np.float32).eps)
KT = E // 128         # 4 contraction tiles over E
PAIRS = 4             # node pairs per layer (8 nodes)

_PROGRAM_CACHE = {}


def _trunc(a):
    """Truncate fp32 mantissa to fp32r (low 12 bits zeroed), matching HW."""
    a = np.ascontiguousarray(a, dtype=np.float32)
    b = np.frombuffer(a.tobytes(), dtype=np.uint32) & np.uint32(0xFFFFF000)
    return np.frombuffer(b.tobytes(), dtype=np.float32).reshape(a.shape).copy()


def _build_program(active_sets):
    """active_sets: tuple of tuples — active layer list per step."""
    nc = bacc.Bacc("TRN2", target_bir_lowering=False, debug=False, num_devices=NC)
    n_ls = max(sum(len(a) for a in active_sets), 1)

    d_x0t = nc.dram_tensor("x0t", [E, T], F32, kind="ExternalInput")
    d_adw = nc.dram_tensor("adw", [L, 128, KT * 512], F32, kind="ExternalInput")
    d_qkw = nc.dram_tensor("qkw", [L, 128, 512], F32, kind="ExternalInput")
    d_vw = nc.dram_tensor("vw", [L, 128, 512], F32, kind="ExternalInput")
    d_fcw = nc.dram_tensor("fcw", [L, 128, 1024], F32, kind="ExternalInput")
    d_cr = nc.dram_tensor("cstr", [128, 640], F32, kind="ExternalInput")
    d_cf = nc.dram_tensor("cstf", [128, 648], F32, kind="ExternalInput")
    d_wap = nc.dram_tensor("wapP", [128, L * PAIRS], F32, kind="ExternalInput")
    d_waw = nc.dram_tensor("wawP", [128, n_ls * PAIRS], F32, kind="ExternalInput")
    d_wmw = nc.dram_tensor("wmwP", [128, n_ls * PAIRS], F32, kind="ExternalInput")
    d_rw = nc.dram_tensor("rwP", [128, KT], F32, kind="ExternalInput")
    d_rb = nc.dram_tensor("rbias", [1, 1], F32, kind="ExternalInput")
    d_lm = nc.dram_tensor("lmt", [E, VQ], F32, kind="ExternalInput")
    d_out = nc.dram_tensor("out_lg", [T, VQ], F32, kind="ExternalOutput")

    NVT = VQ // 512   # 25 vocab tiles of 512
    NTT = T // 128    # 2 token tiles

    with tile.TileContext(nc) as tc:
        with tc.tile_pool(name="cst", bufs=1) as cst, \
             tc.tile_pool(name="st", bufs=1) as st, \
             tc.tile_pool(name="wk", bufs=2) as wk, \
             tc.tile_pool(name="wk4", bufs=4) as wk4, \
             tc.tile_pool(name="adp", bufs=2) as adp, \
             tc.tile_pool(name="lmp", bufs=2) as lmp, \
             tc.tile_pool(name="ps6", bufs=6, space="PSUM") as ps6, \
             tc.tile_pool(name="ps1", bufs=1, space="PSUM") as ps1:

            # ---------------- constants / weights ----------------
            c_r = cst.tile([128, 640], F32R, tag="c_r", name="c_r")
            nc.sync.dma_start(c_r[:], d_cr.ap().bitcast(F32R))
            perm = c_r[:, 0:128]            # rope swap permutation
            oblk = c_r[:, 128:256]          # 1/64 block-diagonal(64) lhsT
            ocol = c_r[:, 256:320]          # (128,64) all ones
            orow128 = c_r[0:1, 256:384]     # (1,128) ones
            orow64 = c_r[0:1, 256:320]      # (1,64) ones
            oc1 = c_r[:, 384:385]           # (128,1) ones
            sel2 = c_r[0:2, 385:513]        # row0 -> rows 0:64, row1 -> rows 64:128
            oblk2 = c_r[:, 513:515]         # col0: 1/64 on rows 0:64; col1: rows 64:128

            c_f = cst.tile([128, 648], F32, tag="c_f", name="c_f")
            nc.sync.dma_start(c_f[:], d_cf.ap())
            C128 = c_f[:, 0:256]
            S128 = c_f[:, 256:512]
            tri = c_f[:, 512:640]
            one_f = c_f[0:1, 0:1]           # cos(0)=1.0, identity for transpose
            eps128 = c_f[:, 640:641]
            eps1 = c_f[0:1, 640:641]

            wap = cst.tile([128, L * PAIRS], F32, tag="wap", name="wap")
            nc.sync.dma_start(wap[:], d_wap.ap())
            waw = cst.tile([128, n_ls * PAIRS], F32, tag="waw", name="waw")
            nc.sync.dma_start(waw[:], d_waw.ap())
            wmw = cst.tile([128, n_ls * PAIRS], F32, tag="wmw", name="wmw")
            nc.sync.dma_start(wmw[:], d_wmw.ap())
            rw = cst.tile([128, KT], F32R, tag="rw", name="rw")
            nc.sync.dma_start(rw[:], d_rw.ap().bitcast(F32R))
            rbias = cst.tile([1, 1], F32, tag="rbias", name="rbias")
            nc.sync.dma_start(rbias[:], d_rb.ap())

            qkw, vw, fcw = [], [], []
            for l in range(L):
                q_t = cst.tile([128, 512], F32R, tag=f"qkw{l}", name=f"qkw{l}")
                nc.sync.dma_start(q_t[:], d_qkw.ap().bitcast(F32R)[l])
                qkw.append(q_t)
                v_t = cst.tile([128, 512], F32R, tag=f"vw{l}", name=f"vw{l}")
                nc.sync.dma_start(v_t[:], d_vw.ap().bitcast(F32R)[l])
                vw.append(v_t)
                f_t = cst.tile([128, 1024], F32R, tag=f"fcw{l}", name=f"fcw{l}")
                nc.sync.dma_start(f_t[:], d_fcw.ap().bitcast(F32R)[l])
                fcw.append(f_t)

            # ---------------- state ----------------
            xT = [st.tile([128, T], F32, tag=f"xT{k}", name=f"xT{k}") for k in range(KT)]
            xr = [st.tile([128, T], F32R, tag=f"xr{k}", name=f"xr{k}") for k in range(KT)]
            acc = [st.tile([128, T], F32, tag=f"acc{k}", name=f"acc{k}") for k in range(KT)]
            pcont = st.tile([1, T], F32, tag="pcont", name="pcont")
            pcr = st.tile([1, T], F32R, tag="pcr", name="pcr")
            nc.vector.memset(pcont[:], 1.0)
            nc.vector.memset(pcr[:].bitcast(F32), 1.0)
            for k in range(KT):
                nc.gpsimd.memset(acc[k][:], 0.0)

            # ---------------- initial x = rms(wte[idx]) ----------------
            x0 = []
            for k in range(KT):
                x0k = wk.tile([128, T], F32, tag=f"x0_{k}", name=f"x0_{k}")
                nc.sync.dma_start(x0k[:], d_x0t.ap()[k * 128:(k + 1) * 128, :])
                x0.append(x0k)
            p_ms = ps6.tile([1, T], F32, tag="ps", name="ps")
            for k in range(KT):
                sq = wk.tile([128, T], F32R, tag="sq0", name="sq0")
                nc.scalar.activation(sq[:], x0[k][:], AF.Square)
                nc.tensor.matmul(p_ms[:], oc1, sq[:], start=(k == 0), stop=(k == KT - 1))
            rrow = wk.tile([1, T], F32, tag="rrow", name="rrow")
            nc.scalar.activation(rrow[:], p_ms[:], AF.Sqrt, bias=eps1, scale=1.0 / E)
            rrec = wk.tile([1, T], F32R, tag="rrec", name="rrec")
            with nc.allow_low_precision(reason="fp32r broadcast operand"):
                nc.vector.reciprocal(rrec[:], rrow[:])   # rsqrt(mean+eps)
            p_rb0 = ps6.tile([128, T], F32, tag="ps", name="ps")
            nc.tensor.matmul(p_rb0[:], orow128, rrec[:], start=True, stop=True)
            for k in range(KT):
                nc.vector.tensor_tensor(xT[k][:], x0[k][:], p_rb0[:], AluOpType.mult)
                nc.vector.tensor_copy(xr[k][:], xT[k][:])

            # ---------------- step loop ----------------
            ls_idx = 0
            for t, layers in enumerate(active_sets):
                for l in layers:
                    adl = adp.tile([128, KT * 512], F32R, tag="adl", name="adl")
                    nc.sync.dma_start(adl[:], d_adw.ap().bitcast(F32R)[l])
                    for p in range(PAIRS):
                        rows_e, rows_o = slice(0, 64), slice(64, 128)
                        node_rc = ((rows_e, (0, 0)), (rows_o, (64, 0)))

                        # xi = adapters @ x (+ x)
                        p_xi = ps6.tile([128, T], F32, tag="ps", name="ps")
                        for k in range(KT):
                            nc.tensor.matmul(
                                p_xi[:], adl[:, k * 512 + p * 128: k * 512 + (p + 1) * 128],
                                xr[k][:], start=(k == 0), stop=(k == KT - 1))
                        xi = wk.tile([128, T], F32R, tag="xi", name="xi")
                        nc.vector.tensor_tensor(xi[:], p_xi[:], xT[p][:], AluOpType.add)

                        # qk per node -> [q;k] psum
                        p_qk = []
                        for rows, tp in node_rc:
                            pq = ps6.tile([128, T], F32, tag="ps", name="ps")
                            nc.tensor.matmul(pq[:], qkw[l][rows, p * 128:(p + 1) * 128],
                                             xi[rows, :], start=True, stop=True,
                                             tile_position=tp)
                            p_qk.append(pq)

                        # rope + rms -> qt/kt pair tiles
                        qt = wk.tile([128, T], F32R, tag="qt", name="qt")
                        kt = wk.tile([128, T], F32R, tag="kt", name="kt")
                        for o in range(2):
                            qs = wk.tile([128, T], F32R, tag="qs", name="qs")
                            nc.scalar.copy(qs[:], p_qk[o][:])
                            p_sw = ps6.tile([128, T], F32, tag="ps", name="ps")
                            nc.tensor.matmul(p_sw[:], perm, qs[:], start=True, stop=True)
                            t1 = wk.tile([128, T], F32, tag="t1", name="t1")
                            nc.gpsimd.tensor_tensor(t1[:], qs[:].bitcast(F32), C128,
                                                    AluOpType.mult)
                            rop = wk.tile([128, T], F32, tag="rop", name="rop")
                            t2 = wk.tile([128, T], F32, tag="t2", name="t2")
                            nc.vector.tensor_tensor(t2[:], p_sw[:], S128, AluOpType.mult)
                            nc.vector.tensor_tensor(rop[:], t1[:], t2[:], AluOpType.add)
                            sqr = wk.tile([128, T], F32R, tag="sqr", name="sqr")
                            nc.scalar.activation(sqr[:], rop[:], AF.Square)
                            p_m = ps6.tile([128, T], F32, tag="ps", name="ps")
                            nc.tensor.matmul(p_m[:], oblk, sqr[:], start=True, stop=True)
                            srt = wk.tile([128, T], F32, tag="srt", name="srt")
                            nc.scalar.activation(srt[:], p_m[:], AF.Sqrt, bias=eps128)
                            rsq = wk.tile([128, T], F32, tag="rsq", name="rsq")
                            nc.vector.reciprocal(rsq[:], srt[:])
                            orows = rows_e if o == 0 else rows_o
                            nc.vector.tensor_tensor(qt[orows, :], rop[0:64, :],
                                                    rsq[0:64, :], AluOpType.mult)
                            nc.vector.tensor_tensor(kt[orows, :], rop[64:128, :],
                                                    rsq[64:128, :], AluOpType.mult)

                        # scores -> exp/mask -> em tiles
                        em0, em1 = [None, None], [None, None]
                        for o, (rows, tp) in enumerate(node_rc):
                            p_s0 = ps6.tile([128, T], F32, tag="ps", name="ps")
                            nc.tensor.matmul(p_s0[:], kt[rows, 0:128], qt[rows, :],
                                             start=True, stop=True, tile_position=tp)
                            p_s1 = ps6.tile([128, 128], F32, tag="ps", name="ps")
                            nc.tensor.matmul(p_s1[:], kt[rows, 128:256], qt[rows, 128:256],
                                             start=True, stop=True, tile_position=tp)
                            e0 = wk4.tile([128, T], F32R, tag="em0", name="em0")
                            tmp = wk.tile([128, 128], F32, tag="etmp", name="etmp")
                            nc.scalar.activation(tmp[:], p_s0[:, 0:128], AF.Exp, scale=0.125)
                            nc.gpsimd.tensor_tensor(e0[:, 0:128], tmp[:], tri, AluOpType.mult)
                            nc.scalar.activation(e0[:, 128:256], p_s0[:, 128:256],
                                                 AF.Exp, scale=0.125)
                            e1 = wk4.tile([128, 128], F32R, tag="em1", name="em1")
                            tmp2 = wk.tile([128, 128], F32, tag="etmp2", name="etmp2")
                            nc.scalar.activation(tmp2[:], p_s1[:], AF.Exp, scale=0.125)
                            nc.gpsimd.tensor_tensor(e1[:], tmp2[:], tri, AluOpType.mult)
                            em0[o], em1[o] = e0, e1

                        # v per node per s-tile
                        v_sb = [[None, None], [None, None]]
                        for o, (rows, tp) in enumerate(node_rc):
                            for s in range(2):
                                p_v = ps6.tile([128, 64], F32, tag="ps", name="ps")
                                nc.tensor.matmul(
                                    p_v[:], xi[rows, s * 128:(s + 1) * 128],
                                    vw[l][rows, p * 128 + o * 64: p * 128 + (o + 1) * 64],
                                    start=True, stop=True, tile_position=tp)
                                vt = wk4.tile([128, 64], F32R, tag="vt", name="vt")
                                nc.scalar.copy(vt[:], p_v[:])
                                v_sb[o][s] = vt

                        # att + colsum
                        p_atts = []
                        p_cs0 = ps1.tile([1, T], F32, tag="pcs0", name="pcs0")
                        p_cs1 = ps1.tile([1, T], F32, tag="pcs1", name="pcs1")
                        for o in range(2):
                            p_att = ps6.tile([64, T], F32, tag="ps", name="ps")
                            p_atts.append(p_att)
                            p_cs = p_cs0 if o == 0 else p_cs1
                            nc.tensor.matmul(p_att[:, 0:128], v_sb[o][0][:],
                                             em0[o][:, 0:128], start=True, stop=True)
                            nc.tensor.matmul(p_att[:, 128:256], v_sb[o][0][:],
                                             em0[o][:, 128:256], start=True, stop=False)
                            nc.tensor.matmul(p_att[:, 128:256], v_sb[o][1][:],
                                             em1[o][:], start=False, stop=True)
                            nc.tensor.matmul(p_cs[0:1, 0:128], oc1, em0[o][:, 0:128],
                                             start=True, stop=True)
                            nc.tensor.matmul(p_cs[0:1, 128:256], oc1, em0[o][:, 128:256],
                                             start=True, stop=False)
                            nc.tensor.matmul(p_cs[0:1, 128:256], oc1, em1[o][:],
                                             start=False, stop=True)

                        rc0 = wk.tile([1, T], F32R, tag="rc0", name="rc0")
                        rc1 = wk.tile([1, T], F32R, tag="rc1", name="rc1")
                        with nc.allow_low_precision(reason="fp32r broadcast operand"):
                            nc.vector.reciprocal(rc0[:], p_cs0[0:1, :])
                            nc.vector.reciprocal(rc1[:], p_cs1[0:1, :])
                        p_rbe = ps6.tile([64, T], F32, tag="ps", name="ps")
                        nc.tensor.matmul(p_rbe[:], orow64, rc0[:], start=True, stop=True)
                        p_rbo = ps6.tile([64, T], F32, tag="ps", name="ps")
                        nc.tensor.matmul(p_rbo[:], orow64, rc1[:], start=True, stop=True)
                        att_sb = wk.tile([128, T], F32, tag="att", name="att")
                        nc.scalar.copy(att_sb[0:64, :], p_atts[0][:])
                        nc.scalar.copy(att_sb[64:128, :], p_atts[1][:])
                        tt = wk.tile([128, T], F32, tag="tt", name="tt")
                        nc.vector.tensor_tensor(tt[0:64, :], att_sb[0:64, :], p_rbe[:],
                                                AluOpType.mult)
                        nc.vector.tensor_tensor(tt[64:128, :], att_sb[64:128, :], p_rbo[:],
                                                AluOpType.mult)

                        xim = wk.tile([128, T], F32R, tag="xim", name="xim")
                        nc.vector.scalar_tensor_tensor(
                            xim[:], tt[:], wap[:, l * PAIRS + p: l * PAIRS + p + 1],
                            xi[:], AluOpType.mult, AluOpType.add)
                        nc.vector.scalar_tensor_tensor(
                            acc[p][:], tt[:],
                            waw[:, ls_idx * PAIRS + p: ls_idx * PAIRS + p + 1],
                            acc[p][:], AluOpType.mult, AluOpType.add)

                        # mlp
                        p_srs = []
                        for o, (rows, tp) in enumerate(node_rc):
                            p_sr = ps6.tile([64, T], F32, tag="ps", name="ps")
                            p_srs.append(p_sr)
                            for h in range(2):
                                p_fc = ps6.tile([128, T], F32, tag="ps", name="ps")
                                nc.tensor.matmul(
                                    p_fc[:],
                                    fcw[l][rows, p * 256 + h * 128: p * 256 + (h + 1) * 128],
                                    xim[rows, :], start=True, stop=True, tile_position=tp)
                                frel = wk.tile([128, T], F32R, tag="frel", name="frel")
                                nc.scalar.activation(frel[:], p_fc[:], AF.Relu)
                                rsq2 = wk.tile([128, T], F32R, tag="rsq2", name="rsq2")
                                nc.scalar.activation(rsq2[:], frel[:], AF.Square)
                                nc.tensor.matmul(p_sr[:], ocol, rsq2[:],
                                                 start=(h == 0), stop=(h == 1))
                        sqm = wk.tile([128, T], F32R, tag="sqm", name="sqm")
                        nc.scalar.activation(sqm[:], xim[:], AF.Square)
                        p_mq = ps6.tile([128, T], F32, tag="ps", name="ps")
                        nc.tensor.matmul(p_mq[:], oblk, sqm[:], start=True, stop=True)
                        pre = wk.tile([128, T], F32, tag="pre", name="pre")
                        nc.vector.tensor_scalar(pre[:], p_mq[:], 1.0, EPS,
                                                AluOpType.mult, AluOpType.add)
                        rec2 = wk.tile([128, T], F32, tag="rec2", name="rec2")
                        nc.vector.reciprocal(rec2[:], pre[:])
                        hm = wk.tile([128, T], F32, tag="hm", name="hm")
                        nc.vector.tensor_tensor(hm[0:64, :], p_srs[0][:], rec2[0:64, :],
                                                AluOpType.mult)
                        nc.vector.tensor_tensor(hm[64:128, :], p_srs[1][:], rec2[64:128, :],
                                                AluOpType.mult)
                        nc.vector.scalar_tensor_tensor(
                            acc[p][:], hm[:],
                            wmw[:, ls_idx * PAIRS + p: ls_idx * PAIRS + p + 1],
                            acc[p][:], AluOpType.mult, AluOpType.add)
                    ls_idx += 1

                # ---- x update + router ----
                p_pc = ps6.tile([128, T], F32, tag="ps", name="ps")
                nc.tensor.matmul(p_pc[:], orow128, pcr[:], start=True, stop=True)
                for k in range(KT):
                    upd = wk.tile([128, T], F32, tag="upd", name="upd")
                    nc.vector.tensor_tensor(upd[:], acc[k][:], p_pc[:], AluOpType.mult)
                    nc.vector.tensor_tensor(xT[k][:], upd[:], xT[k][:], AluOpType.add)
                    nc.vector.tensor_copy(xr[k][:], xT[k][:])
                    nc.gpsimd.memset(acc[k][:], 0.0)
                p_ph = ps6.tile([1, T], F32, tag="ps", name="ps")
                for k in range(KT):
                    nc.tensor.matmul(p_ph[:], rw[:, k:k + 1], xr[k][:],
                                     start=(k == 0), stop=(k == KT - 1))
                ph = wk.tile([1, T], F32, tag="ph", name="ph")
                nc.scalar.activation(ph[:], p_ph[:], AF.Sigmoid, bias=rbias[:])
                omp = wk.tile([1, T], F32, tag="omp", name="omp")
                nc.vector.tensor_scalar(omp[:], ph[:], -1.0, 1.0,
                                        AluOpType.mult, AluOpType.add)
                nc.vector.tensor_tensor(pcont[:], pcont[:], omp[:], AluOpType.mult)
                nc.vector.tensor_copy(pcr[:], pcont[:])

            # ---------------- final rms + lm_head ----------------
            p_mr = ps6.tile([1, T], F32, tag="ps", name="ps")
            for k in range(KT):
                sqf = wk.tile([128, T], F32R, tag="sqf", name="sqf")
                nc.scalar.activation(sqf[:], xT[k][:], AF.Square)
                nc.tensor.matmul(p_mr[:], oc1, sqf[:], start=(k == 0), stop=(k == KT - 1))
            rr = wk.tile([1, T], F32, tag="rr", name="rr")
            nc.scalar.activation(rr[:], p_mr[:], AF.Sqrt, bias=eps1, scale=1.0 / E)
            rr2 = wk.tile([1, T], F32, tag="rr2", name="rr2")
            nc.vector.reciprocal(rr2[:], rr[:])
            rr15 = wk.tile([1, T], F32, tag="rr15", name="rr15")
            nc.vector.tensor_scalar(rr15[:], rr2[:], 1.0 / 15.0, 0.0,
                                    AluOpType.mult, AluOpType.add)
            rcol = []
            for i in range(NTT):
                p_tr = ps1.tile([128, 1], F32, tag="pcs0", name="ptr")
                nc.tensor.transpose(p_tr[:], rr15[:, i * 128:(i + 1) * 128], one_f)
                rc = st.tile([128, 1], F32, tag=f"rcol{i}", name=f"rcol{i}")
                nc.scalar.copy(rc[:], p_tr[:])
                rcol.append(rc)

            for i in range(NTT):
                for v in range(NVT):
                    lmt = lmp.tile([128, KT * 512], F32R, tag="lmt", name="lmt")
                    for k in range(KT):
                        nc.sync.dma_start(
                            lmt[:, k * 512:(k + 1) * 512],
                            d_lm.ap().bitcast(F32R)[k * 128:(k + 1) * 128, v * 512:(v + 1) * 512])
                    p_lg = ps6.tile([128, 512], F32, tag="ps", name="ps")
                    for k in range(KT):
                        nc.tensor.matmul(p_lg[:], xr[k][:, i * 128:(i + 1) * 128],
                                         lmt[:, k * 512:(k + 1) * 512],
                                         start=(k == 0), stop=(k == KT - 1))
                    lth = wk.tile([128, 512], F32, tag="lth", name="lth")
                    nc.scalar.activation(lth[:], p_lg[:], AF.Tanh, scale=rcol[i][:])
                    lt15 = wk.tile([128, 512], F32, tag="lt15", name="lt15")
                    nc.scalar.activation(lt15[:], lth[:], AF.Copy, scale=15.0)
                    nc.sync.dma_start(
                        d_out.ap()[i * 128:(i + 1) * 128, v * 512:(v + 1) * 512],
                        lt15[:])

    nc.compile()
    return nc


def _host_prep(idx, n_steps, wte, adapters, qkv_w, attn_proj, mlp_fc, mlp_proj,
               dep, router_w, router_b, lm_head_w):
    idx = np.asarray(idx)
    wte = np.asarray(wte, np.float32)
    adapters = np.asarray(adapters, np.float32)
    qkv_w = np.asarray(qkv_w, np.float32)
    attn_proj = np.asarray(attn_proj, np.float32)
    mlp_fc = np.asarray(mlp_fc, np.float32)
    mlp_proj = np.asarray(mlp_proj, np.float32)
    dep = np.asarray(dep, np.float32)
    router_w = np.asarray(router_w, np.float32).reshape(E, 1)
    router_b = np.asarray(router_b, np.float32).reshape(-1)
    lm_head_w = np.asarray(lm_head_w, np.float32)
    ns = int(n_steps)

    dp = np.maximum(dep, 0.0)
    depths = np.zeros((N,), np.float32)
    for _ in range(L):
        depths = (dp @ (depths + 1.0)).astype(np.float32)

    w_eff = np.zeros((ns, N), np.float32)
    active_sets = []
    for t in range(ns):
        td = t * (L / ns)
        w_all = np.exp(-np.abs(depths - np.float32(td))).astype(np.float32)
        w = np.where(w_all > 0.15, w_all, 0.0).astype(np.float32)
        w_eff[t] = w
        active_sets.append(tuple(sorted({n // G for n in range(N) if w[n] > 0})))
    active_sets = tuple(active_sets)
    n_ls = max(sum(len(a) for a in active_sets), 1)

    adw = np.zeros((L, 128, KT * 512), np.float32)
    qkw = np.zeros((L, 128, 512), np.float32)
    vw = np.zeros((L, 128, 512), np.float32)
    fcw = np.zeros((L, 128, 1024), np.float32)
    for l in range(L):
        for p in range(PAIRS):
            for o in range(2):
                n = l * G + 2 * p + o
                rows = slice(o * 64, (o + 1) * 64)
                for k in range(KT):
                    adw[l, :, k * 512 + p * 128 + o * 64: k * 512 + p * 128 + (o + 1) * 64] = \
                        adapters[n, :, k * 128:(k + 1) * 128].T
                qkw[l, rows, p * 128:(p + 1) * 128] = qkv_w[n, 0:128, :].T
                vw[l, rows, p * 128 + o * 64: p * 128 + (o + 1) * 64] = qkv_w[n, 128:192, :].T
                fcw[l, rows, p * 256:(p + 1) * 256] = mlp_fc[n].T
    adw, qkw, vw, fcw = _trunc(adw), _trunc(qkw), _trunc(vw), _trunc(fcw)

    cstr = np.zeros((128, 640), np.float32)
    permM = np.zeros((128, 128), np.float32)
    for m in range(128):
        kk = (m // 64) * 64 + ((m % 64) + HD) % 64
        permM[kk, m] = 1.0
    cstr[:, 0:128] = permM
    ob = np.zeros((128, 128), np.float32)
    ob[0:64, 0:64] = 1.0 / GD
    ob[64:128, 64:128] = 1.0 / GD
    cstr[:, 128:256] = ob
    cstr[:, 256:384] = 1.0
    cstr[:, 384:385] = 1.0
    cstr[0, 385:449] = 1.0
    cstr[1, 449:513] = 1.0
    cstr[0:64, 513] = 1.0 / GD
    cstr[64:128, 514] = 1.0 / GD
    cstr = _trunc(cstr)

    inv_freq = 1.0 / (10000.0 ** (np.arange(0, GD, 2, dtype=np.float64) / GD))
    freqs = np.outer(np.arange(T), inv_freq)
    cosT = np.cos(freqs).astype(np.float32).T
    sinT = np.sin(freqs).astype(np.float32).T
    cstf = np.zeros((128, 648), np.float32)
    cstf[:, 640] = EPS
    for blk in range(4):
        cstf[blk * 32:(blk + 1) * 32, 0:256] = cosT
        cstf[blk * 32:(blk + 1) * 32, 256:512] = sinT * (1.0 if blk % 2 == 0 else -1.0)
    s_i = np.arange(128)[:, None]
    t_i = np.arange(128)[None, :]
    cstf[:, 512:640] = (s_i <= t_i).astype(np.float32)

    w_ap = attn_proj.sum(axis=2)
    w_mp = mlp_proj.sum(axis=2)
    wapP = np.zeros((128, L * PAIRS), np.float32)
    wawP = np.zeros((128, n_ls * PAIRS), np.float32)
    wmwP = np.zeros((128, n_ls * PAIRS), np.float32)
    for l in range(L):
        for p in range(PAIRS):
            for o in range(2):
                n = l * G + 2 * p + o
                wapP[o * 64:(o + 1) * 64, l * PAIRS + p] = w_ap[n]
    ls = 0
    for t, layers in enumerate(active_sets):
        for l in layers:
            for p in range(PAIRS):
                for o in range(2):
                    n = l * G + 2 * p + o
                    wawP[o * 64:(o + 1) * 64, ls * PAIRS + p] = w_ap[n] * w_eff[t, n]
                    wmwP[o * 64:(o + 1) * 64, ls * PAIRS + p] = w_mp[n] * w_eff[t, n]
            ls += 1

    rwP = np.zeros((128, KT), np.float32)
    for k in range(KT):
        rwP[:, k] = router_w[k * 128:(k + 1) * 128, 0]
    rwP = _trunc(rwP)
    rbias = np.full((1, 1), np.float32(router_b[0]), np.float32)

    gathered = wte[idx]
    in_maps = []
    for c in range(NC):
        b, vs = c // VSH, c % VSH
        lo = vs * VW
        hi = min(lo + VW, V)
        lmt = np.zeros((E, VQ), np.float32)
        lmt[:, 0:hi - lo] = lm_head_w[lo:hi, :].T
        in_maps.append({
            "x0t": np.ascontiguousarray(gathered[b].T), "adw": adw, "qkw": qkw,
            "vw": vw, "fcw": fcw, "cstr": cstr, "cstf": cstf, "wapP": wapP,
            "wawP": wawP, "wmwP": wmwP, "rwP": rwP, "rbias": rbias,
            "lmt": _trunc(lmt),
        })
    return active_sets, in_maps


def kernel(idx, n_steps, wte, adapters, qkv_w, attn_proj, mlp_fc, mlp_proj,
           dep, router_w, router_b, lm_head_w):
    active_sets, in_maps = _host_prep(
        idx, n_steps, wte, adapters, qkv_w, attn_proj, mlp_fc, mlp_proj,
        dep, router_w, router_b, lm_head_w)

    if active_sets not in _PROGRAM_CACHE:
        _PROGRAM_CACHE[active_sets] = _build_program(active_sets)
    nc = _PROGRAM_CACHE[active_sets]

    trace = bool(int(os.environ.get("BASS_KERNEL_TRACE", "0")))
    res = run_bass_kernel_spmd(nc, in_maps, list(range(NC)), trace=trace)
    if trace and res.exec_time_ns is not None:
        print(f"HW exec time: {res.exec_time_ns} ns")

    out = np.zeros((B, T, V), np.float32)
    for c in range(NC):
        b, vs = c // VSH, c % VSH
        lo = vs * VW
        hi = min(lo + VW, V)
        out[b, :, lo:hi] = res.results[c]["out_lg"][:, 0:hi - lo]
    return out



# revision 19
# speedup vs baseline: 2.6444x; 2.6444x over previous
"""Trainium2 Bass kernel for nn_BG_ALRT_62921270886438 (moe_routing).

Sharding v2: core c -> (batch b = c // 4, pair p = c % 4).  Each core computes
only its pair's two nodes per active layer; the group-wise scatter-add target
of pair p is exactly E-rows [128p, 128p+128), so the per-step x update needs
only an AllGather (groups {0-3}, {4-7}) of each core's [128, T] acc slice.
lm_head is vocab-sharded 4 ways within each batch group (same output contract
as v1).  Matmuls run in fp16 (1 cycle/row vs 4 for fp32), x state stays fp32.

Self-contained: only numpy + the concourse toolchain on sys.path.
"""
import os

import numpy as np

import concourse.bacc as bacc
import concourse.tile as tile
from concourse import mybir
from concourse.alu_op_type import AluOpType
from concourse.bass_utils import run_bass_kernel_spmd

AF = mybir.ActivationFunctionType
F32 = mybir.dt.float32
F16 = mybir.dt.float16

B, T, E, G, GD, L, N, V = 2, 256, 512, 8, 64, 8, 64, 50257
HD = GD // 2          # 32, rope half
NC = 8                # cores
VSH = 4               # vocab shards per batch group
VW = (V + VSH - 1) // VSH          # 12565 raw shard width
VQ = ((VW + 511) // 512) * 512     # 12800 padded shard width
EPS = float(np.finfo(np.float32).eps)
KT = E // 128         # 4 contraction tiles over E
NVT = VQ // 512       # 25 vocab tiles of 512
NTT = T // 128        # 2 token tiles

_PROGRAM_CACHE = {}


def _build_program(active_sets):
    """active_sets: tuple of tuples - active layer list per step."""
    nc = bacc.Bacc("TRN2", target_bir_lowering=False, debug=False, num_devices=NC)
    n_ls = max(sum(len(a) for a in active_sets), 1)
    groups = [[0, 1, 2, 3], [4, 5, 6, 7]]

    d_x0t = nc.dram_tensor("x0t", [E, T], F32, kind="ExternalInput")
    d_adw = nc.dram_tensor("adw", [L, 128, 512], F16, kind="ExternalInput")
    d_qkw = nc.dram_tensor("qkw", [L, 128, 256], F16, kind="ExternalInput")
    d_qpw = nc.dram_tensor("qpw", [L, 128, 256], F16, kind="ExternalInput")
    d_vww = nc.dram_tensor("vww", [L, 128, 128], F16, kind="ExternalInput")
    d_fcw = nc.dram_tensor("fcw", [L, 128, 512], F16, kind="ExternalInput")
    d_c16 = nc.dram_tensor("c16", [128, 705], F16, kind="ExternalInput")
    d_cf = nc.dram_tensor("cstf", [128, 1155], F32, kind="ExternalInput")
    d_wap = nc.dram_tensor("wapP", [128, L], F32, kind="ExternalInput")
    d_waw = nc.dram_tensor("wawP", [128, n_ls], F32, kind="ExternalInput")
    d_wmw = nc.dram_tensor("wmwP", [128, n_ls], F32, kind="ExternalInput")
    d_rw = nc.dram_tensor("rwP", [128, KT], F16, kind="ExternalInput")
    d_rb = nc.dram_tensor("rbias2", [1, 1], F32, kind="ExternalInput")
    d_lm = nc.dram_tensor("lmt", [E, VQ], F16, kind="ExternalInput")
    d_out = nc.dram_tensor("out_lg", [T, VQ], F16, kind="ExternalOutput")

    with tile.TileContext(nc) as tc:
        with tc.tile_pool(name="cst", bufs=1) as cst, \
             tc.tile_pool(name="st", bufs=1) as st, \
             tc.tile_pool(name="wk16", bufs=3) as wk16, \
             tc.tile_pool(name="wkf", bufs=2) as wkf, \
             tc.tile_pool(name="vsb", bufs=4) as vsb, \
             tc.tile_pool(name="ps", bufs=1, space="PSUM") as ps, \
             tc.tile_pool(name="dram", bufs=20, space="DRAM") as dram:

            # ---------------- constants ----------------
            c16 = cst.tile([128, 705], F16, tag="c16", name="c16")
            nc.sync.dma_start(c16[:], d_c16.ap())
            oblk = c16[:, 0:128]            # block-diag(64) of 1/64
            ocol = c16[:, 128:192]          # (128,64) ones
            oc1 = c16[:, 192:193]           # (128,1) ones
            sel2 = c16[0:2, 193:321]        # row0 -> parts 0:64, row1 -> 64:128
            onesrow = c16[0:1, 321:449]     # (1,128) ones
            tri2 = c16[:, 449:705]          # [tri | tri] fp16

            cf = cst.tile([128, 1155], F32, tag="cf", name="cf")
            nc.sync.dma_start(cf[:], d_cf.ap())
            CC2 = cf[:, 0:512]              # [C | C]
            SS2 = cf[:, 512:1024]           # [S | S]
            eps128 = cf[:, 1024:1025]
            eps1 = cf[0:1, 1024:1025]
            one_f = cf[0:1, 1025:1026]      # 1.0 (transpose identity)
            orowf = cf[0:1, 1027:1155]      # (1,128) ones f32

            wap = cst.tile([128, L], F32, tag="wap", name="wap")
            nc.sync.dma_start(wap[:], d_wap.ap())
            waw = cst.tile([128, n_ls], F32, tag="waw", name="waw")
            nc.sync.dma_start(waw[:], d_waw.ap())
            wmw = cst.tile([128, n_ls], F32, tag="wmw", name="wmw")
            nc.sync.dma_start(wmw[:], d_wmw.ap())
            rw = cst.tile([128, KT], F16, tag="rw", name="rw")
            nc.sync.dma_start(rw[:], d_rw.ap())
            rbias2 = cst.tile([1, 1], F32, tag="rbias2", name="rbias2")
            nc.sync.dma_start(rbias2[:], d_rb.ap())

            adw, qkw, qpw, vww, fcw = [], [], [], [], []
            for l in range(L):
                a_t = cst.tile([128, 512], F16, tag=f"adw{l}", name=f"adw{l}")
                nc.sync.dma_start(a_t[:], d_adw.ap()[l])
                adw.append(a_t)
                q_t = cst.tile([128, 256], F16, tag=f"qkw{l}", name=f"qkw{l}")
                nc.sync.dma_start(q_t[:], d_qkw.ap()[l])
                qkw.append(q_t)
                p_t = cst.tile([128, 256], F16, tag=f"qpw{l}", name=f"qpw{l}")
                nc.sync.dma_start(p_t[:], d_qpw.ap()[l])
                qpw.append(p_t)
                v_t = cst.tile([128, 128], F16, tag=f"vww{l}", name=f"vww{l}")
                nc.sync.dma_start(v_t[:], d_vww.ap()[l])
                vww.append(v_t)
                f_t = cst.tile([128, 512], F16, tag=f"fcw{l}", name=f"fcw{l}")
                nc.sync.dma_start(f_t[:], d_fcw.ap()[l])
                fcw.append(f_t)

            # lm_head weights: full shard resident in SBUF, chunked DMA so the
            # prefetch never head-of-line blocks the per-step bounce DMAs.
            lmsb = []
            LCH = 1600
            for k in range(KT):
                t_ = cst.tile([128, VQ], F16, tag=f"lm{k}", name=f"lm{k}")
                lmsb.append(t_)
                for c0 in range(0, VQ, LCH):
                    nc.sync.dma_start(
                        t_[:, c0:c0 + LCH],
                        d_lm.ap()[k * 128:(k + 1) * 128, c0:c0 + LCH])

            # ---------------- state ----------------
            xT = [st.tile([128, T], F32, tag=f"xT{k}", name=f"xT{k}") for k in range(KT)]
            xr = [st.tile([128, T], F16, tag=f"xr{k}", name=f"xr{k}") for k in range(KT)]
            acc = st.tile([128, T], F32, tag="acc", name="acc")
            xg = st.tile([128, KT * T], F32, tag="xg", name="xg")
            pcont = st.tile([1, T], F32, tag="pcont", name="pcont")
            nc.vector.memset(pcont[:], 1.0)
            nc.gpsimd.memset(acc[:], 0.0)

            # initial x (rms applied host-side)
            for k in range(KT):
                nc.sync.dma_start(xT[k][:], d_x0t.ap()[k * 128:(k + 1) * 128, :])
                with nc.allow_low_precision(reason="fp16 compute"):
                    nc.vector.tensor_copy(xr[k][:], xT[k][:])

            # CC warm-up: dummy AllGather so the first real one is cheap
            NO_CC = bool(int(os.environ.get("BASS_V2_NO_CC", "0")))
            db_in = dram.tile([128, 8], F32, tag="dbi", name="dbi")
            db_out = dram.tile([512, 8], F32, tag="dbo", name="dbo")
            nc.sync.dma_start(db_in[:], cf[:, 0:8])
            if not NO_CC:
                nc.gpsimd.collective_compute(
                    "AllGather", mybir.AluOpType.bypass, replica_groups=groups,
                    ins=[db_in[:].opt()], outs=[db_out[:].opt()])

            ls_idx = 0
            with nc.allow_low_precision(reason="fp16 compute"):
                for t, layers in enumerate(active_sets):
                    for l in layers:
                        # ---- xi = adapters' @ x (identity folded in) ----
                        H1 = ps.tile([128, 2 * T], F32, tag="H1", bufs=1, name="ps")
                        p_xi = H1[:, 0:T]
                        p_s1 = H1[:, T:2 * T]
                        for k in range(KT):
                            nc.tensor.matmul(
                                p_xi[:], adw[l][:, k * 128:(k + 1) * 128],
                                xr[k][:], start=(k == 0), stop=(k == KT - 1))
                        xi = wk16.tile([128, T], F16, tag="xi", name="xi")
                        nc.scalar.copy(xi[:], p_xi[:])

                        # ---- v per s-block (both nodes at once; vww is
                        # host-zero-padded so full-128 contraction is exact)
                        S1 = ps.tile([128, 2 * T], F32, tag="S1", bufs=1, name="ps")
                        p_v = S1[:, 0:T]
                        p_sr0 = S1[0:64, T:2 * T]
                        v_sb = [None, None]
                        for s in range(2):
                            nc.tensor.matmul(
                                p_v[:, s * 128:(s + 1) * 128],
                                xi[:, s * 128:(s + 1) * 128],
                                vww[l][:], start=True, stop=True)
                            vt = vsb.tile([128, 130], F16, tag="vt", name="vt")
                            if s == 0:
                                nc.scalar.copy(vt[:, 0:64], p_v[:, 0:64])
                                nc.scalar.copy(vt[:, 65:129], p_v[:, 64:128])
                            else:
                                nc.vector.tensor_copy(vt[:, 0:64], p_v[:, 128:192])
                                nc.vector.tensor_copy(vt[:, 65:129], p_v[:, 192:256])
                            nc.gpsimd.memset(vt[:, 64:65], 1.0)
                            nc.gpsimd.memset(vt[:, 129:130], 1.0)
                            v_sb[s] = vt

                        # ---- q/k (raw + pre-permuted), both nodes packed ----
                        p_qk = ps.tile([128, 2 * T], F32, tag="A", bufs=2, name="ps")
                        p_qp = ps.tile([128, 2 * T], F32, tag="B", bufs=1, name="ps")
                        for o in range(2):
                            nc.tensor.matmul(p_qk[:, o * T:(o + 1) * T],
                                             qkw[l][:, o * 128:(o + 1) * 128],
                                             xi[:], start=True, stop=True)
                            nc.tensor.matmul(p_qp[:, o * T:(o + 1) * T],
                                             qpw[l][:, o * 128:(o + 1) * 128],
                                             xi[:], start=True, stop=True)

                        # rms scale from pre-rope q/k (rope is norm-preserving)
                        sq = wk16.tile([128, 2 * T], F16, tag="sq", name="sq")
                        nc.scalar.activation(sq[:], p_qk[:], AF.Square)
                        p_ms = ps.tile([128, 2 * T], F32, tag="C", bufs=1, name="ps")
                        nc.tensor.matmul(p_ms[:], oblk, sq[:], start=True, stop=True)
                        srt = wkf.tile([128, 2 * T], F32, tag="srt", name="srt")
                        nc.scalar.activation(srt[:], p_ms[:], AF.Sqrt, bias=eps128)
                        rsq = wkf.tile([128, 2 * T], F32, tag="rsq", name="rsq")
                        nc.vector.reciprocal(rsq[:], srt[:])

                        # rope: rot = qk*C + qp*S, then normalize + split q/k
                        t1 = wk16.tile([128, 2 * T], F16, bufs=2, tag="t1", name="t1")
                        nc.vector.tensor_tensor(t1[:], p_qk[:], CC2, AluOpType.mult)
                        t2 = wk16.tile([128, 2 * T], F16, bufs=2, tag="t2", name="t2")
                        nc.vector.tensor_tensor(t2[:], p_qp[:], SS2, AluOpType.mult)
                        rop = wk16.tile([128, 2 * T], F16, bufs=2, tag="rop", name="rop")
                        nc.vector.tensor_tensor(rop[:], t1[:], t2[:], AluOpType.add)
                        qt = wk16.tile([128, T], F16, tag="qt", name="qt")
                        kt = wk16.tile([128, 2 * T], F16, tag="kt", name="kt")
                        nc.gpsimd.memset(kt[64:128, 0:T], 0.0)
                        nc.gpsimd.memset(kt[0:64, T:2 * T], 0.0)
                        for o in range(2):
                            orows = slice(64 * o, 64 * o + 64)
                            nc.vector.tensor_tensor(
                                qt[orows, :], rop[0:64, o * T:(o + 1) * T],
                                rsq[0:64, o * T:(o + 1) * T], AluOpType.mult)
                            nc.vector.tensor_tensor(
                                kt[orows, o * T:(o + 1) * T],
                                rop[64:128, o * T:(o + 1) * T],
                                rsq[64:128, o * T:(o + 1) * T], AluOpType.mult)

                        # ---- scores -> masked exp ----
                        p_s0 = ps.tile([128, 2 * T], F32, tag="C", bufs=1, name="ps")
                        for o in range(2):
                            nc.tensor.matmul(p_s0[:, o * T:(o + 1) * T],
                                             kt[:, o * T:o * T + 128], qt[:],
                                             start=True, stop=True)
                            nc.tensor.matmul(p_s1[:, o * 128:(o + 1) * 128],
                                             kt[:, o * T + 128:(o + 1) * T],
                                             qt[:, 128:256],
                                             start=True, stop=True)
                        em0 = wk16.tile([128, 2 * T], F16, tag="em0", name="em0")
                        nc.scalar.activation(em0[:], p_s0[:], AF.Exp, scale=0.125)
                        em1 = wk16.tile([128, T], F16, tag="em1", name="em1")
                        nc.scalar.activation(em1[:], p_s1[:], AF.Exp, scale=0.125)
                        # masked diagonal blocks (separate tiles, no in-place)
                        m0 = wk16.tile([128, T], F16, tag="m0", name="m0")
                        nc.gpsimd.tensor_tensor(m0[:, 0:128], em0[:, 0:128],
                                                tri2[:, 0:128], AluOpType.mult)
                        nc.gpsimd.tensor_tensor(m0[:, 128:256], em0[:, T:T + 128],
                                                tri2[:, 0:128], AluOpType.mult)
                        m1 = wk16.tile([128, T], F16, tag="m1", name="m1")
                        nc.gpsimd.tensor_tensor(m1[:], em1[:], tri2, AluOpType.mult)

                        # ---- att (+colsum via ones col) ----
                        S2 = ps.tile([128, 2 * T], F32, tag="S2", bufs=1, name="ps")
                        p_att = [S2[0:65, 0:T], S2[0:65, T:2 * T]]
                        for o in range(2):
                            pa = p_att[o]
                            nc.tensor.matmul(pa[:, 0:128],
                                             v_sb[0][:, o * 65:(o + 1) * 65],
                                             m0[:, o * 128:(o + 1) * 128],
                                             start=True, stop=True)
                            nc.tensor.matmul(pa[:, 128:256],
                                             v_sb[0][:, o * 65:(o + 1) * 65],
                                             em0[:, o * T + 128:(o + 1) * T],
                                             start=True, stop=False)
                            nc.tensor.matmul(pa[:, 128:256],
                                             v_sb[1][:, o * 65:(o + 1) * 65],
                                             m1[:, o * 128:(o + 1) * 128],
                                             start=False, stop=True)
                        rc2 = wkf.tile([1, 2 * T], F32, bufs=1, tag="rc2", name="rc2")
                        nc.vector.reciprocal(rc2[:], S2[64:65, 0:2 * T])
                        H2 = ps.tile([128, 2 * T], F32, tag="H2", bufs=1, name="ps")
                        nc.tensor.matmul(H2[:], orowf, rc2[:], start=True, stop=True)
                        att_sb = wk16.tile([128, T], F16, tag="att", name="att")
                        nc.scalar.copy(att_sb[0:64, :], p_att[0][0:64, :])
                        nc.scalar.copy(att_sb[64:128, :], p_att[1][0:64, :])
                        tt = wk16.tile([128, T], F16, tag="tt", name="tt")
                        nc.vector.tensor_tensor(tt[0:64, :], att_sb[0:64, :],
                                                H2[0:64, 0:T], AluOpType.mult)
                        nc.vector.tensor_tensor(tt[64:128, :], att_sb[64:128, :],
                                                H2[64:128, T:2 * T], AluOpType.mult)

                        xim = wk16.tile([128, T], F16, tag="xim", name="xim")
                        nc.vector.scalar_tensor_tensor(
                            xim[:], tt[:], wap[:, l:l + 1], xi[:],
                            AluOpType.mult, AluOpType.add)
                        nc.vector.scalar_tensor_tensor(
                            acc[:], tt[:], waw[:, ls_idx:ls_idx + 1], acc[:],
                            AluOpType.mult, AluOpType.add)

                        # ---- mlp (rms folded into 1/(mean+eps) post-scale) ----
                        sqm = wk16.tile([128, T], F16, tag="sqm", name="sqm")
                        nc.gpsimd.tensor_tensor(sqm[:], xim[:], xim[:],
                                                AluOpType.mult)
                        p_mq = ps.tile([128, T], F32, tag="H1", bufs=1, name="ps")
                        nc.tensor.matmul(p_mq[:], oblk, sqm[:], start=True, stop=True)
                        pre = wkf.tile([128, T], F32, tag="pre", name="pre")
                        nc.scalar.activation(pre[:], p_mq[:], AF.Identity,
                                             bias=eps128)
                        rec2 = wkf.tile([128, T], F32, tag="rec2", name="rec2")
                        nc.vector.reciprocal(rec2[:], pre[:])

                        p_srs = []
                        for o in range(2):
                            p_fc = ps.tile([128, 2 * T], F32, tag="A" if o == 0 else "B",
                                            bufs=2 if o == 0 else 1, name="ps")
                            for h in range(2):
                                nc.tensor.matmul(
                                    p_fc[:, h * T:(h + 1) * T],
                                    fcw[l][:, o * 256 + h * 128:o * 256 + (h + 1) * 128],
                                    xim[:], start=True, stop=True)
                            frel = wk16.tile([128, 2 * T], F16, tag="frel", name="frel")
                            nc.scalar.activation(frel[:], p_fc[:], AF.Relu)
                            rsq2 = wk16.tile([128, 2 * T], F16, tag="rsq2", name="rsq2")
                            nc.gpsimd.tensor_tensor(rsq2[:], frel[:], frel[:],
                                                    AluOpType.mult)
                            if o == 0:
                                p_sr = p_sr0
                            else:
                                p_sr = ps.tile([64, T], F32, tag="H2", bufs=1, name="ps")
                            p_srs.append(p_sr)
                            nc.tensor.matmul(p_sr[:], ocol, rsq2[:, 0:T],
                                             start=True, stop=False)
                            nc.tensor.matmul(p_sr[:], ocol, rsq2[:, T:2 * T],
                                             start=False, stop=True)
                        hm = wk16.tile([128, T], F16, tag="hm", name="hm")
                        nc.vector.tensor_tensor(hm[0:64, :], p_srs[0][:],
                                                rec2[0:64, :], AluOpType.mult)
                        nc.vector.tensor_tensor(hm[64:128, :], p_srs[1][:],
                                                rec2[64:128, :], AluOpType.mult)
                        nc.vector.scalar_tensor_tensor(
                            acc[:], hm[:], wmw[:, ls_idx:ls_idx + 1], acc[:],
                            AluOpType.mult, AluOpType.add)
                        ls_idx += 1

                    # ---- step sync: scale acc by pcont, AllGather, update x ----
                    p_pc = ps.tile([128, T], F32, tag="H1", bufs=1, name="ps")
                    nc.tensor.matmul(p_pc[:], orowf, pcont[:], start=True, stop=True)
                    acc2 = wkf.tile([128, T], F32, bufs=1, tag="acc2", name="acc2")
                    nc.vector.tensor_tensor(acc2[:], acc[:], p_pc[:], AluOpType.mult)
                    nc.gpsimd.memset(acc[:], 0.0)
                    b_in = dram.tile([128, T], F32, tag="bin", name=f"bin{t}")
                    b_out = dram.tile([KT * 128, T], F32, tag="bout", name=f"bout{t}")
                    nc.sync.dma_start(b_in[:], acc2[:])
                    if not NO_CC:
                        nc.gpsimd.collective_compute(
                            "AllGather", mybir.AluOpType.bypass, replica_groups=groups,
                            ins=[b_in[:].opt()], outs=[b_out[:].opt()])
                        for k in range(KT):
                            nc.sync.dma_start(xg[:, k * T:(k + 1) * T],
                                              b_out[k * 128:(k + 1) * 128, :])
                    else:
                        for k in range(KT):
                            nc.sync.dma_start(xg[:, k * T:(k + 1) * T], b_in[:])
                    for k in range(KT):
                        nc.vector.tensor_tensor(xT[k][:], xT[k][:],
                                                xg[:, k * T:(k + 1) * T],
                                                AluOpType.add)
                        nc.vector.tensor_copy(xr[k][:], xT[k][:])

                    # ---- router: pcont *= 1 - sigmoid(x@rw + rb) ----
                    p_ph = ps.tile([1, T], F32, tag="S2", bufs=1, name="ps")
                    for k in range(KT):
                        nc.tensor.matmul(p_ph[:], rw[:, k:k + 1], xr[k][:],
                                         start=(k == 0), stop=(k == KT - 1))
                    th = wkf.tile([1, T], F32, bufs=1, tag="th", name="th")
                    nc.scalar.activation(th[:], p_ph[:], AF.Tanh,
                                         scale=0.5, bias=rbias2[:])
                    omp = wkf.tile([1, T], F32, bufs=1, tag="omp", name="omp")
                    nc.vector.tensor_scalar(omp[:], th[:], -0.5, 0.5,
                                            AluOpType.mult, AluOpType.add)
                    nc.vector.tensor_tensor(pcont[:], pcont[:], omp[:],
                                            AluOpType.mult)

                # ---------------- final rms + lm_head ----------------
                p_mr = ps.tile([1, T], F32, tag="S1", bufs=1, name="ps")
                for k in range(KT):
                    sqf = wk16.tile([128, T], F16, tag="sqf", name="sqf")
                    nc.scalar.activation(sqf[:], xr[k][:], AF.Square)
                    nc.tensor.matmul(p_mr[:], oc1, sqf[:],
                                     start=(k == 0), stop=(k == KT - 1))
                rr = wkf.tile([1, T], F32, bufs=1, tag="rr", name="rr")
                nc.scalar.activation(rr[:], p_mr[:], AF.Sqrt, bias=eps1,
                                     scale=1.0 / E)
                rr2 = wkf.tile([1, T], F32, bufs=1, tag="rr2", name="rr2")
                nc.vector.reciprocal(rr2[:], rr[:])
                rr15 = wkf.tile([1, T], F32, bufs=1, tag="rr15", name="rr15")
                nc.vector.tensor_scalar(rr15[:], rr2[:], 1.0 / 15.0, 0.0,
                                        AluOpType.mult, AluOpType.add)
                rcol = []
                for i in range(NTT):
                    p_tr = ps.tile([128, 1], F32, tag="S2", bufs=1, name="ptr")
                    nc.tensor.transpose(p_tr[:], rr15[:, i * 128:(i + 1) * 128],
                                        one_f)
                    rc = st.tile([128, 1], F32, tag=f"rcol{i}", name=f"rcol{i}")
                    nc.scalar.copy(rc[:], p_tr[:])
                    rcol.append(rc)

                for i in range(NTT):
                    for v in range(NVT):
                        p_lg = ps.tile([128, 512], F32, tag="A", bufs=2, name="ps")
                        for k in range(KT):
                            nc.tensor.matmul(
                                p_lg[:], xr[k][:, i * 128:(i + 1) * 128],
                                lmsb[k][:, v * 512:(v + 1) * 512],
                                start=(k == 0), stop=(k == KT - 1))
                        lth = wk16.tile([128, 512], F16, tag="lth", name="lth")
                        nc.scalar.activation(lth[:], p_lg[:], AF.Tanh,
                                             scale=rcol[i][:])
                        nc.sync.dma_start(
                            d_out.ap()[i * 128:(i + 1) * 128,
                                       v * 512:(v + 1) * 512],
                            lth[:])

    nc.compile()
    return nc


def _rms_np(x):
    return x * (1.0 / np.sqrt(np.mean(x * x, axis=-1, keepdims=True) + EPS))


def _host_prep(idx, n_steps, wte, adapters, qkv_w, attn_proj, mlp_fc, mlp_proj,
               dep, router_w, router_b, lm_head_w):
    idx = np.asarray(idx)
    wte = np.asarray(wte, np.float32)
    adapters = np.asarray(adapters, np.float32)
    qkv_w = np.asarray(qkv_w, np.float32)
    attn_proj = np.asarray(attn_proj, np.float32)
    mlp_fc = np.asarray(mlp_fc, np.float32)
    mlp_proj = np.asarray(mlp_proj, np.float32)
    dep = np.asarray(dep, np.float32)
    router_w = np.asarray(router_w, np.float32).reshape(E, 1)
    router_b = np.asarray(router_b, np.float32).reshape(-1)
    lm_head_w = np.asarray(lm_head_w, np.float32)
    ns = int(n_steps)

    dp = np.maximum(dep, 0.0)
    depths = np.zeros((N,), np.float32)
    for _ in range(L):
        depths = (dp @ (depths + 1.0)).astype(np.float32)

    w_eff = np.zeros((ns, N), np.float32)
    active_sets = []
    for t in range(ns):
        td = t * (L / ns)
        w_all = np.exp(-np.abs(depths - np.float32(td))).astype(np.float32)
        w = np.where(w_all > 0.15, w_all, 0.0).astype(np.float32)
        w_eff[t] = w
        active_sets.append(tuple(sorted({n // G for n in range(N) if w[n] > 0})))
    active_sets = tuple(active_sets)
    n_ls = max(sum(len(a) for a in active_sets), 1)

    # fold the group-slice identity into the adapters
    adapters_f = adapters.copy()
    for n in range(N):
        g = n % G
        adapters_f[n, :, g * GD:(g + 1) * GD] += np.eye(GD, dtype=np.float32)

    # rope permutation of the q/k OUTPUT index: out j <- out (j+32)%64 within
    # each 64-block (q block and k block separately)
    perm64 = (np.arange(GD) + HD) % GD
    perm128 = np.concatenate([perm64, GD + perm64])

    w_ap = attn_proj.sum(axis=2)
    w_mp = mlp_proj.sum(axis=2)

    # per-pair weight payloads
    payload = []
    for p in range(VSH):
        adw = np.zeros((L, 128, 512), np.float16)
        qkwA = np.zeros((L, 128, 256), np.float16)
        qpwA = np.zeros((L, 128, 256), np.float16)
        vwwA = np.zeros((L, 128, 128), np.float16)
        fcwA = np.zeros((L, 128, 512), np.float16)
        wapP = np.zeros((128, L), np.float32)
        wawP = np.zeros((128, n_ls), np.float32)
        wmwP = np.zeros((128, n_ls), np.float32)
        for l in range(L):
            for o in range(2):
                n = l * G + 2 * p + o
                rows = slice(o * 64, (o + 1) * 64)
                for k in range(KT):
                    adw[l, :, k * 128 + o * 64: k * 128 + (o + 1) * 64] = \
                        adapters_f[n, :, k * 128:(k + 1) * 128].T
                # zero-padded full-128-contraction stationaries (node o's
                # weights live on its own 64 rows; the rest stay zero)
                qkwA[l, rows, o * 128:(o + 1) * 128] = qkv_w[n, 0:128, :].T
                qpwA[l, rows, o * 128:(o + 1) * 128] = qkv_w[n, 0:128, :].T[:, perm128]
                vwwA[l, rows, o * 64:(o + 1) * 64] = qkv_w[n, 128:192, :].T
                fcwA[l, rows, o * 256:(o + 1) * 256] = mlp_fc[n].T
                wapP[o * 64:(o + 1) * 64, l] = w_ap[n]
        ls = 0
        for tt, layers in enumerate(active_sets):
            for l in layers:
                for o in range(2):
                    n = l * G + 2 * p + o
                    wawP[o * 64:(o + 1) * 64, ls] = w_ap[n] * w_eff[tt, n]
                    wmwP[o * 64:(o + 1) * 64, ls] = w_mp[n] * w_eff[tt, n]
                ls += 1
        payload.append((adw, qkwA, qpwA, vwwA, fcwA, wapP, wawP, wmwP))

    # constants
    c16 = np.zeros((128, 705), np.float16)
    ob = np.zeros((128, 128), np.float32)
    ob[0:64, 0:64] = 1.0 / GD
    ob[64:128, 64:128] = 1.0 / GD
    c16[:, 0:128] = ob.astype(np.float16)
    c16[:, 128:192] = 1.0
    c16[:, 192:193] = 1.0
    c16[0, 193:257] = 1.0
    c16[1, 257:321] = 1.0
    c16[0, 321:449] = 1.0
    s_i = np.arange(128)[:, None]
    t_i = np.arange(128)[None, :]
    tri = (s_i <= t_i).astype(np.float16)
    c16[:, 449:577] = tri
    c16[:, 577:705] = tri

    inv_freq = 1.0 / (10000.0 ** (np.arange(0, GD, 2, dtype=np.float64) / GD))
    freqs = np.outer(np.arange(T), inv_freq)
    cosT = np.cos(freqs).astype(np.float32).T
    sinT = np.sin(freqs).astype(np.float32).T
    cstf = np.zeros((128, 1155), np.float32)
    for blk in range(4):
        cstf[blk * 32:(blk + 1) * 32, 0:256] = cosT
        cstf[blk * 32:(blk + 1) * 32, 256:512] = cosT
        cstf[blk * 32:(blk + 1) * 32, 512:768] = sinT * (1.0 if blk % 2 == 0 else -1.0)
        cstf[blk * 32:(blk + 1) * 32, 768:1024] = sinT * (1.0 if blk % 2 == 0 else -1.0)
    cstf[:, 1024] = EPS
    cstf[0, 1025] = 1.0
    cstf[0, 1027:1155] = 1.0

    rwP = np.zeros((128, KT), np.float16)
    for k in range(KT):
        rwP[:, k] = router_w[k * 128:(k + 1) * 128, 0].astype(np.float16)
    rbias2 = np.full((1, 1), np.float32(router_b[0]) * 0.5, np.float32)

    x0 = _rms_np(wte[idx])  # (B, T, E) f32

    in_maps = []
    for c in range(NC):
        b, p = c // VSH, c % VSH
        lo = p * VW
        hi = min(lo + VW, V)
        lmt = np.zeros((E, VQ), np.float16)
        lmt[:, 0:hi - lo] = lm_head_w[lo:hi, :].T.astype(np.float16)
        adw, qkwA, qpwA, vwwA, fcwA, wapP, wawP, wmwP = payload[p]
        in_maps.append({
            "x0t": np.ascontiguousarray(x0[b].T), "adw": adw, "qkw": qkwA,
            "qpw": qpwA, "vww": vwwA, "fcw": fcwA, "c16": c16, "cstf": cstf,
            "wapP": wapP, "wawP": wawP, "wmwP": wmwP, "rwP": rwP,
            "rbias2": rbias2, "lmt": lmt,
        })
    return active_sets, in_maps


def kernel(idx, n_steps, wte, adapters, qkv_w, attn_proj, mlp_fc, mlp_proj,
           dep, router_w, router_b, lm_head_w):
    active_sets, in_maps = _host_prep(
        idx, n_steps, wte, adapters, qkv_w, attn_proj, mlp_fc, mlp_proj,
        dep, router_w, router_b, lm_head_w)

    if active_sets not in _PROGRAM_CACHE:
        _PROGRAM_CACHE[active_sets] = _build_program(active_sets)
    nc = _PROGRAM_CACHE[active_sets]

    trace = bool(int(os.environ.get("BASS_KERNEL_TRACE", "0")))
    res = run_bass_kernel_spmd(nc, in_maps, list(range(NC)), trace=trace)
    if trace and res.exec_time_ns is not None:
        print(f"HW exec time: {res.exec_time_ns} ns")

    out = np.zeros((B, T, V), np.float32)
    for c in range(NC):
        b, p = c // VSH, c % VSH
        lo = p * VW
        hi = min(lo + VW, V)
        out[b, :, lo:hi] = 15.0 * res.results[c]["out_lg"][:, 0:hi - lo].astype(np.float32)
    return out


# revision 20
# speedup vs baseline: 3.0581x; 1.1564x over previous
"""Trainium2 Bass kernel for nn_BG_ALRT_62921270886438 (moe_routing).

Sharding v2: core c -> (batch b = c // 4, pair p = c % 4).  Each core computes
only its pair's two nodes per active layer; the group-wise scatter-add target
of pair p is exactly E-rows [128p, 128p+128), so the per-step x update needs
only an AllGather (groups {0-3}, {4-7}) of each core's [128, T] acc slice.
lm_head is vocab-sharded 4 ways within each batch group (same output contract
as v1).  Matmuls run in fp16 (1 cycle/row vs 4 for fp32), x state stays fp32.

Self-contained: only numpy + the concourse toolchain on sys.path.
"""
import os

import numpy as np

import concourse.bacc as bacc
import concourse.tile as tile
from concourse import mybir
from concourse.alu_op_type import AluOpType
from concourse.bass_utils import run_bass_kernel_spmd

AF = mybir.ActivationFunctionType
F32 = mybir.dt.float32
F16 = mybir.dt.float16

B, T, E, G, GD, L, N, V = 2, 256, 512, 8, 64, 8, 64, 50257
HD = GD // 2          # 32, rope half
NC = 8                # cores
VSH = 4               # vocab shards per batch group
VW = (V + VSH - 1) // VSH          # 12565 raw shard width
VQ = ((VW + 511) // 512) * 512     # 12800 padded shard width
EPS = float(np.finfo(np.float32).eps)
KT = E // 128         # 4 contraction tiles over E
NVT = VQ // 512       # 25 vocab tiles of 512
NTT = T // 128        # 2 token tiles

_PROGRAM_CACHE = {}


def _build_program(active_sets):
    """active_sets: tuple of tuples - active layer list per step."""
    nc = bacc.Bacc("TRN2", target_bir_lowering=False, debug=False, num_devices=NC)
    n_ls = max(sum(len(a) for a in active_sets), 1)
    groups = [[0, 1, 2, 3], [4, 5, 6, 7]]

    d_x0t = nc.dram_tensor("x0t", [E, T], F32, kind="ExternalInput")
    d_adw = nc.dram_tensor("adw", [L, 128, 512], F16, kind="ExternalInput")
    d_qkw = nc.dram_tensor("qkw", [L, 128, 256], F16, kind="ExternalInput")
    d_qpw = nc.dram_tensor("qpw", [L, 128, 256], F16, kind="ExternalInput")
    d_vww = nc.dram_tensor("vww", [L, 128, 128], F16, kind="ExternalInput")
    d_fcw = nc.dram_tensor("fcw", [L, 128, 512], F16, kind="ExternalInput")
    d_c16 = nc.dram_tensor("c16", [128, 705], F16, kind="ExternalInput")
    d_cf = nc.dram_tensor("cstf", [128, 1155], F32, kind="ExternalInput")
    d_wap = nc.dram_tensor("wapP", [128, L], F32, kind="ExternalInput")
    d_waw = nc.dram_tensor("wawP", [128, n_ls], F32, kind="ExternalInput")
    d_wmw = nc.dram_tensor("wmwP", [128, n_ls], F32, kind="ExternalInput")
    d_rw = nc.dram_tensor("rwP", [128, KT], F16, kind="ExternalInput")
    d_rb = nc.dram_tensor("rbias2", [1, 1], F32, kind="ExternalInput")
    d_lm = nc.dram_tensor("lmt", [E, VQ], F16, kind="ExternalInput")
    d_out = nc.dram_tensor("out_lg", [T, VQ], F16, kind="ExternalOutput")

    with tile.TileContext(nc) as tc:
        with tc.tile_pool(name="cst", bufs=1) as cst, \
             tc.tile_pool(name="st", bufs=1) as st, \
             tc.tile_pool(name="wk16", bufs=3) as wk16, \
             tc.tile_pool(name="wkf", bufs=2) as wkf, \
             tc.tile_pool(name="vsb", bufs=4) as vsb, \
             tc.tile_pool(name="ps", bufs=1, space="PSUM") as ps, \
             tc.tile_pool(name="dram", bufs=20, space="DRAM") as dram:

            # ---------------- constants ----------------
            c16 = cst.tile([128, 705], F16, tag="c16", name="c16")
            nc.sync.dma_start(c16[:], d_c16.ap())
            oblk = c16[:, 0:128]            # block-diag(64) of 1/64
            ocol = c16[:, 128:192]          # (128,64) ones
            oc1 = c16[:, 192:193]           # (128,1) ones
            sel2 = c16[0:2, 193:321]        # row0 -> parts 0:64, row1 -> 64:128
            onesrow = c16[0:1, 321:449]     # (1,128) ones
            tri2 = c16[:, 449:705]          # [tri | tri] fp16

            cf = cst.tile([128, 1155], F32, tag="cf", name="cf")
            nc.sync.dma_start(cf[:], d_cf.ap())
            CC2 = cf[:, 0:512]              # [C | C]
            SS2 = cf[:, 512:1024]           # [S | S]
            eps128 = cf[:, 1024:1025]
            eps1 = cf[0:1, 1024:1025]
            one_f = cf[0:1, 1025:1026]      # 1.0 (transpose identity)
            mln15 = cf[0:1, 1026:1027]      # -ln(15)
            orowf = cf[0:1, 1027:1155]      # (1,128) ones f32

            wap = cst.tile([128, L], F32, tag="wap", name="wap")
            nc.sync.dma_start(wap[:], d_wap.ap())
            waw = cst.tile([128, n_ls], F32, tag="waw", name="waw")
            nc.sync.dma_start(waw[:], d_waw.ap())
            wmw = cst.tile([128, n_ls], F32, tag="wmw", name="wmw")
            nc.sync.dma_start(wmw[:], d_wmw.ap())
            rw = cst.tile([128, KT], F16, tag="rw", name="rw")
            nc.sync.dma_start(rw[:], d_rw.ap())
            rbias2 = cst.tile([1, 1], F32, tag="rbias2", name="rbias2")
            nc.sync.dma_start(rbias2[:], d_rb.ap())

            adw, qkw, qpw, vww, fcw = [], [], [], [], []
            for l in range(L):
                a_t = cst.tile([128, 512], F16, tag=f"adw{l}", name=f"adw{l}")
                nc.sync.dma_start(a_t[:], d_adw.ap()[l])
                adw.append(a_t)
                q_t = cst.tile([128, 256], F16, tag=f"qkw{l}", name=f"qkw{l}")
                nc.sync.dma_start(q_t[:], d_qkw.ap()[l])
                qkw.append(q_t)
                p_t = cst.tile([128, 256], F16, tag=f"qpw{l}", name=f"qpw{l}")
                nc.sync.dma_start(p_t[:], d_qpw.ap()[l])
                qpw.append(p_t)
                v_t = cst.tile([128, 128], F16, tag=f"vww{l}", name=f"vww{l}")
                nc.sync.dma_start(v_t[:], d_vww.ap()[l])
                vww.append(v_t)
                f_t = cst.tile([128, 512], F16, tag=f"fcw{l}", name=f"fcw{l}")
                nc.sync.dma_start(f_t[:], d_fcw.ap()[l])
                fcw.append(f_t)

            # lm_head weights: full shard resident in SBUF, chunked DMA so the
            # prefetch never head-of-line blocks the per-step bounce DMAs.
            lmsb = []
            LCH = 1600
            for k in range(KT):
                t_ = cst.tile([128, VQ], F16, tag=f"lm{k}", name=f"lm{k}")
                lmsb.append(t_)
                for c0 in range(0, VQ, LCH):
                    nc.sync.dma_start(
                        t_[:, c0:c0 + LCH],
                        d_lm.ap()[k * 128:(k + 1) * 128, c0:c0 + LCH])

            # ---------------- state ----------------
            xT = [st.tile([128, T], F32, tag=f"xT{k}", name=f"xT{k}") for k in range(KT)]
            xr = [st.tile([128, T], F16, tag=f"xr{k}", name=f"xr{k}") for k in range(KT)]
            acc = st.tile([128, T], F32, tag="acc", name="acc")
            xg = st.tile([128, KT * T], F32, tag="xg", name="xg")
            pcont = st.tile([1, T], F32, tag="pcont", name="pcont")
            nc.vector.memset(pcont[:], 1.0)
            nc.gpsimd.memset(acc[:], 0.0)

            # initial x (rms applied host-side)
            for k in range(KT):
                nc.sync.dma_start(xT[k][:], d_x0t.ap()[k * 128:(k + 1) * 128, :])
                with nc.allow_low_precision(reason="fp16 compute"):
                    nc.vector.tensor_copy(xr[k][:], xT[k][:])

            # CC warm-up: dummy AllGather so the first real one is cheap
            NO_CC = bool(int(os.environ.get("BASS_V2_NO_CC", "0")))
            db_in = dram.tile([128, 8], F32, tag="dbi", name="dbi")
            db_out = dram.tile([512, 8], F32, tag="dbo", name="dbo")
            nc.sync.dma_start(db_in[:], cf[:, 0:8])
            if not NO_CC:
                nc.gpsimd.collective_compute(
                    "AllGather", mybir.AluOpType.bypass, replica_groups=groups,
                    ins=[db_in[:].opt()], outs=[db_out[:].opt()])

            ls_idx = 0
            with nc.allow_low_precision(reason="fp16 compute"):
                for t, layers in enumerate(active_sets):
                    for l in layers:
                        # ---- xi = adapters' @ x (identity folded in) ----
                        H1 = ps.tile([128, 2 * T], F32, tag="H1", bufs=1, name="ps")
                        p_xi = H1[:, 0:T]
                        p_s1 = H1[:, T:2 * T]
                        for k in range(KT):
                            nc.tensor.matmul(
                                p_xi[:], adw[l][:, k * 128:(k + 1) * 128],
                                xr[k][:], start=(k == 0), stop=(k == KT - 1))
                        xi = wk16.tile([128, T], F16, tag="xi", name="xi")
                        nc.vector.tensor_copy(xi[:], p_xi[:])

                        # ---- v per s-block (both nodes at once; vww is
                        # host-zero-padded so full-128 contraction is exact)
                        S1 = ps.tile([128, 2 * T], F32, tag="S1", bufs=1, name="ps")
                        p_v = S1[:, 0:T]
                        p_sr0 = S1[0:64, T:2 * T]
                        v_sb = [None, None]
                        for s in range(2):
                            nc.tensor.matmul(
                                p_v[:, s * 128:(s + 1) * 128],
                                xi[:, s * 128:(s + 1) * 128],
                                vww[l][:], start=True, stop=True)
                            vt = vsb.tile([128, 130], F16, tag="vt", name="vt")
                            if s == 0:
                                nc.scalar.copy(vt[:, 0:64], p_v[:, 0:64])
                                nc.scalar.copy(vt[:, 65:129], p_v[:, 64:128])
                            else:
                                nc.vector.tensor_copy(vt[:, 0:64], p_v[:, 128:192])
                                nc.vector.tensor_copy(vt[:, 65:129], p_v[:, 192:256])
                            nc.gpsimd.memset(vt[:, 64:65], 1.0)
                            nc.gpsimd.memset(vt[:, 129:130], 1.0)
                            v_sb[s] = vt

                        # ---- q/k (raw + pre-permuted), both nodes packed ----
                        p_qk = ps.tile([128, 2 * T], F32, tag="A", bufs=2, name="ps")
                        p_qp = ps.tile([128, 2 * T], F32, tag="B", bufs=1, name="ps")
                        for o in range(2):
                            nc.tensor.matmul(p_qk[:, o * T:(o + 1) * T],
                                             qkw[l][:, o * 128:(o + 1) * 128],
                                             xi[:], start=True, stop=True)
                            nc.tensor.matmul(p_qp[:, o * T:(o + 1) * T],
                                             qpw[l][:, o * 128:(o + 1) * 128],
                                             xi[:], start=True, stop=True)

                        # rms scale from pre-rope q/k (rope is norm-preserving)
                        sq = wk16.tile([128, 2 * T], F16, tag="sq", name="sq")
                        nc.scalar.activation(sq[:], p_qk[:], AF.Square)
                        p_ms = ps.tile([128, 2 * T], F32, tag="C", bufs=1, name="ps")
                        nc.tensor.matmul(p_ms[:], oblk, sq[:], start=True, stop=True)
                        lnm = wkf.tile([128, 2 * T], F32, tag="srt", name="lnm")
                        nc.scalar.activation(lnm[:], p_ms[:], AF.Ln, bias=eps128)
                        rsq = wk16.tile([128, 2 * T], F16, tag="rsq", name="rsq")
                        nc.scalar.activation(rsq[:], lnm[:], AF.Exp, scale=-0.5)

                        # rope: rot = qk*C + qp*S, then normalize + split q/k
                        t1 = wk16.tile([128, 2 * T], F16, bufs=2, tag="t1", name="t1")
                        nc.vector.tensor_tensor(t1[:], p_qk[:], CC2, AluOpType.mult)
                        t2 = wk16.tile([128, 2 * T], F16, bufs=2, tag="t2", name="t2")
                        nc.vector.tensor_tensor(t2[:], p_qp[:], SS2, AluOpType.mult)
                        rop = wk16.tile([128, 2 * T], F16, bufs=2, tag="rop", name="rop")
                        nc.vector.tensor_tensor(rop[:], t1[:], t2[:], AluOpType.add)
                        qt = wk16.tile([128, T], F16, tag="qt", name="qt")
                        kt = wk16.tile([128, 2 * T], F16, tag="kt", name="kt")
                        nc.gpsimd.memset(kt[64:128, 0:T], 0.0)
                        nc.gpsimd.memset(kt[0:64, T:2 * T], 0.0)
                        for o in range(2):
                            orows = slice(64 * o, 64 * o + 64)
                            nc.vector.tensor_tensor(
                                qt[orows, :], rop[0:64, o * T:(o + 1) * T],
                                rsq[0:64, o * T:(o + 1) * T], AluOpType.mult)
                            nc.vector.tensor_tensor(
                                kt[orows, o * T:(o + 1) * T],
                                rop[64:128, o * T:(o + 1) * T],
                                rsq[64:128, o * T:(o + 1) * T], AluOpType.mult)

                        # ---- scores -> masked exp ----
                        p_s0 = ps.tile([128, 2 * T], F32, tag="C", bufs=1, name="ps")
                        for o in range(2):
                            nc.tensor.matmul(p_s0[:, o * T:(o + 1) * T],
                                             kt[:, o * T:o * T + 128], qt[:],
                                             start=True, stop=True)
                            nc.tensor.matmul(p_s1[:, o * 128:(o + 1) * 128],
                                             kt[:, o * T + 128:(o + 1) * T],
                                             qt[:, 128:256],
                                             start=True, stop=True)
                        em0 = wk16.tile([128, 2 * T], F16, tag="em0", name="em0")
                        nc.scalar.activation(em0[:], p_s0[:], AF.Exp, scale=0.125)
                        em1 = wk16.tile([128, T], F16, tag="em1", name="em1")
                        nc.scalar.activation(em1[:], p_s1[:], AF.Exp, scale=0.125)
                        # masked diagonal blocks (separate tiles, no in-place)
                        m0 = wk16.tile([128, T], F16, tag="m0", name="m0")
                        nc.gpsimd.tensor_tensor(m0[:, 0:128], em0[:, 0:128],
                                                tri2[:, 0:128], AluOpType.mult)
                        nc.gpsimd.tensor_tensor(m0[:, 128:256], em0[:, T:T + 128],
                                                tri2[:, 0:128], AluOpType.mult)
                        m1 = wk16.tile([128, T], F16, tag="m1", name="m1")
                        nc.gpsimd.tensor_tensor(m1[:], em1[:], tri2, AluOpType.mult)

                        # ---- att (+colsum via ones col) ----
                        S2 = ps.tile([128, 2 * T], F32, tag="S2", bufs=1, name="ps")
                        p_att = [S2[0:65, 0:T], S2[0:65, T:2 * T]]
                        for o in range(2):
                            pa = p_att[o]
                            nc.tensor.matmul(pa[:, 0:128],
                                             v_sb[0][:, o * 65:(o + 1) * 65],
                                             m0[:, o * 128:(o + 1) * 128],
                                             start=True, stop=True)
                            nc.tensor.matmul(pa[:, 128:256],
                                             v_sb[0][:, o * 65:(o + 1) * 65],
                                             em0[:, o * T + 128:(o + 1) * T],
                                             start=True, stop=False)
                            nc.tensor.matmul(pa[:, 128:256],
                                             v_sb[1][:, o * 65:(o + 1) * 65],
                                             m1[:, o * 128:(o + 1) * 128],
                                             start=False, stop=True)
                        rcl = wkf.tile([1, 2 * T], F32, bufs=1, tag="rcl", name="rcl")
                        nc.scalar.activation(rcl[:], S2[64:65, 0:2 * T], AF.Ln)
                        rc2 = wkf.tile([1, 2 * T], F32, bufs=1, tag="rc2", name="rc2")
                        nc.scalar.activation(rc2[:], rcl[:], AF.Exp, scale=-1.0)
                        H2 = ps.tile([128, 2 * T], F32, tag="H2", bufs=1, name="ps")
                        nc.tensor.matmul(H2[:], orowf, rc2[:], start=True, stop=True)
                        att_sb = wk16.tile([128, T], F16, tag="att", name="att")
                        nc.scalar.copy(att_sb[0:64, :], p_att[0][0:64, :])
                        nc.scalar.copy(att_sb[64:128, :], p_att[1][0:64, :])
                        tt = wk16.tile([128, T], F16, tag="tt", name="tt")
                        nc.vector.tensor_tensor(tt[0:64, :], att_sb[0:64, :],
                                                H2[0:64, 0:T], AluOpType.mult)
                        nc.vector.tensor_tensor(tt[64:128, :], att_sb[64:128, :],
                                                H2[64:128, T:2 * T], AluOpType.mult)

                        xim = wk16.tile([128, T], F16, tag="xim", name="xim")
                        nc.vector.scalar_tensor_tensor(
                            xim[:], tt[:], wap[:, l:l + 1], xi[:],
                            AluOpType.mult, AluOpType.add)
                        nc.vector.scalar_tensor_tensor(
                            acc[:], tt[:], waw[:, ls_idx:ls_idx + 1], acc[:],
                            AluOpType.mult, AluOpType.add)

                        # ---- mlp (rms folded into 1/(mean+eps) post-scale) ----
                        sqm = wk16.tile([128, T], F16, tag="sqm", name="sqm")
                        nc.gpsimd.tensor_tensor(sqm[:], xim[:], xim[:],
                                                AluOpType.mult)
                        p_mq = ps.tile([128, T], F32, tag="H1", bufs=1, name="ps")
                        nc.tensor.matmul(p_mq[:], oblk, sqm[:], start=True, stop=True)
                        lnm2 = wkf.tile([128, T], F32, bufs=1, tag="pre", name="lnm2")
                        nc.scalar.activation(lnm2[:], p_mq[:], AF.Ln, bias=eps128)
                        rec2 = wk16.tile([128, T], F16, tag="rec2", name="rec2")
                        nc.scalar.activation(rec2[:], lnm2[:], AF.Exp, scale=-1.0)

                        p_srs = []
                        for o in range(2):
                            p_fc = ps.tile([128, 2 * T], F32, tag="A" if o == 0 else "B",
                                            bufs=2 if o == 0 else 1, name="ps")
                            for h in range(2):
                                nc.tensor.matmul(
                                    p_fc[:, h * T:(h + 1) * T],
                                    fcw[l][:, o * 256 + h * 128:o * 256 + (h + 1) * 128],
                                    xim[:], start=True, stop=True)
                            frel = wk16.tile([128, 2 * T], F16, tag="frel", name="frel")
                            nc.scalar.activation(frel[:], p_fc[:], AF.Relu)
                            rsq2 = wk16.tile([128, 2 * T], F16, tag="rsq2", name="rsq2")
                            nc.gpsimd.tensor_tensor(rsq2[:], frel[:], frel[:],
                                                    AluOpType.mult)
                            if o == 0:
                                p_sr = p_sr0
                            else:
                                p_sr = ps.tile([64, T], F32, tag="H2", bufs=1, name="ps")
                            p_srs.append(p_sr)
                            nc.tensor.matmul(p_sr[:], ocol, rsq2[:, 0:T],
                                             start=True, stop=False)
                            nc.tensor.matmul(p_sr[:], ocol, rsq2[:, T:2 * T],
                                             start=False, stop=True)
                        hm = wk16.tile([128, T], F16, tag="hm", name="hm")
                        nc.vector.tensor_tensor(hm[0:64, :], p_srs[0][:],
                                                rec2[0:64, :], AluOpType.mult)
                        nc.vector.tensor_tensor(hm[64:128, :], p_srs[1][:],
                                                rec2[64:128, :], AluOpType.mult)
                        nc.vector.scalar_tensor_tensor(
                            acc[:], hm[:], wmw[:, ls_idx:ls_idx + 1], acc[:],
                            AluOpType.mult, AluOpType.add)
                        ls_idx += 1

                    # ---- step sync: scale acc by pcont, AllGather, update x ----
                    p_pc = ps.tile([128, T], F32, tag="H1", bufs=1, name="ps")
                    nc.tensor.matmul(p_pc[:], orowf, pcont[:], start=True, stop=True)
                    acc2 = wkf.tile([128, T], F32, bufs=1, tag="acc2", name="acc2")
                    nc.vector.tensor_tensor(acc2[:], acc[:], p_pc[:], AluOpType.mult)
                    nc.gpsimd.memset(acc[:], 0.0)
                    b_in = dram.tile([128, T], F32, tag="bin", name=f"bin{t}")
                    b_out = dram.tile([KT * 128, T], F32, tag="bout", name=f"bout{t}")
                    nc.sync.dma_start(b_in[:], acc2[:])
                    if not NO_CC:
                        nc.gpsimd.collective_compute(
                            "AllGather", mybir.AluOpType.bypass, replica_groups=groups,
                            ins=[b_in[:].opt()], outs=[b_out[:].opt()])
                        for k in range(KT):
                            nc.sync.dma_start(xg[:, k * T:(k + 1) * T],
                                              b_out[k * 128:(k + 1) * 128, :])
                    else:
                        for k in range(KT):
                            nc.sync.dma_start(xg[:, k * T:(k + 1) * T], b_in[:])
                    for k in range(KT):
                        nc.vector.tensor_tensor(xT[k][:], xT[k][:],
                                                xg[:, k * T:(k + 1) * T],
                                                AluOpType.add)
                        nc.vector.tensor_copy(xr[k][:], xT[k][:])

                    # ---- router: pcont *= 1 - sigmoid(x@rw + rb) ----
                    p_ph = ps.tile([1, T], F32, tag="S2", bufs=1, name="ps")
                    for k in range(KT):
                        nc.tensor.matmul(p_ph[:], rw[:, k:k + 1], xr[k][:],
                                         start=(k == 0), stop=(k == KT - 1))
                    ez = wkf.tile([1, T], F32, bufs=1, tag="th", name="ez")
                    nc.scalar.activation(ez[:], p_ph[:], AF.Exp, bias=rbias2[:])
                    ez1 = wkf.tile([1, T], F32, bufs=1, tag="omp", name="ez1")
                    nc.vector.tensor_scalar(ez1[:], ez[:], 1.0, 1.0,
                                            AluOpType.mult, AluOpType.add)
                    lz = wkf.tile([1, T], F32, bufs=1, tag="lz", name="lz")
                    nc.scalar.activation(lz[:], ez1[:], AF.Ln)
                    omp = wkf.tile([1, T], F32, bufs=1, tag="omp2", name="omp")
                    nc.scalar.activation(omp[:], lz[:], AF.Exp, scale=-1.0)
                    nc.vector.tensor_tensor(pcont[:], pcont[:], omp[:],
                                            AluOpType.mult)

                # ---------------- final rms + lm_head ----------------
                p_mr = ps.tile([1, T], F32, tag="S1", bufs=1, name="ps")
                for k in range(KT):
                    sqf = wk16.tile([128, T], F16, tag="sqf", name="sqf")
                    nc.scalar.activation(sqf[:], xr[k][:], AF.Square)
                    nc.tensor.matmul(p_mr[:], oc1, sqf[:],
                                     start=(k == 0), stop=(k == KT - 1))
                lnf = wkf.tile([1, T], F32, bufs=1, tag="rr", name="lnf")
                nc.scalar.activation(lnf[:], p_mr[:], AF.Ln, bias=eps1,
                                     scale=1.0 / E)
                rr15 = wkf.tile([1, T], F32, bufs=1, tag="rr15", name="rr15")
                nc.scalar.activation(rr15[:], lnf[:], AF.Exp, scale=-0.5,
                                     bias=mln15)
                rcol = []
                for i in range(NTT):
                    p_tr = ps.tile([128, 1], F32, tag="S2", bufs=1, name="ptr")
                    nc.tensor.transpose(p_tr[:], rr15[:, i * 128:(i + 1) * 128],
                                        one_f)
                    rc = st.tile([128, 1], F32, tag=f"rcol{i}", name=f"rcol{i}")
                    nc.scalar.copy(rc[:], p_tr[:])
                    rcol.append(rc)

                for i in range(NTT):
                    for v in range(NVT):
                        p_lg = ps.tile([128, 512], F32, tag="A", bufs=2, name="ps")
                        for k in range(KT):
                            nc.tensor.matmul(
                                p_lg[:], xr[k][:, i * 128:(i + 1) * 128],
                                lmsb[k][:, v * 512:(v + 1) * 512],
                                start=(k == 0), stop=(k == KT - 1))
                        lth = wk16.tile([128, 512], F16, tag="lth", name="lth")
                        nc.scalar.activation(lth[:], p_lg[:], AF.Tanh,
                                             scale=rcol[i][:])
                        nc.sync.dma_start(
                            d_out.ap()[i * 128:(i + 1) * 128,
                                       v * 512:(v + 1) * 512],
                            lth[:])

    nc.compile()
    return nc


def _rms_np(x):
    return x * (1.0 / np.sqrt(np.mean(x * x, axis=-1, keepdims=True) + EPS))


def _host_prep(idx, n_steps, wte, adapters, qkv_w, attn_proj, mlp_fc, mlp_proj,
               dep, router_w, router_b, lm_head_w):
    idx = np.asarray(idx)
    wte = np.asarray(wte, np.float32)
    adapters = np.asarray(adapters, np.float32)
    qkv_w = np.asarray(qkv_w, np.float32)
    attn_proj = np.asarray(attn_proj, np.float32)
    mlp_fc = np.asarray(mlp_fc, np.float32)
    mlp_proj = np.asarray(mlp_proj, np.float32)
    dep = np.asarray(dep, np.float32)
    router_w = np.asarray(router_w, np.float32).reshape(E, 1)
    router_b = np.asarray(router_b, np.float32).reshape(-1)
    lm_head_w = np.asarray(lm_head_w, np.float32)
    ns = int(n_steps)

    dp = np.maximum(dep, 0.0)
    depths = np.zeros((N,), np.float32)
    for _ in range(L):
        depths = (dp @ (depths + 1.0)).astype(np.float32)

    w_eff = np.zeros((ns, N), np.float32)
    active_sets = []
    for t in range(ns):
        td = t * (L / ns)
        w_all = np.exp(-np.abs(depths - np.float32(td))).astype(np.float32)
        w = np.where(w_all > 0.15, w_all, 0.0).astype(np.float32)
        w_eff[t] = w
        active_sets.append(tuple(sorted({n // G for n in range(N) if w[n] > 0})))
    active_sets = tuple(active_sets)
    n_ls = max(sum(len(a) for a in active_sets), 1)

    # fold the group-slice identity into the adapters
    adapters_f = adapters.copy()
    for n in range(N):
        g = n % G
        adapters_f[n, :, g * GD:(g + 1) * GD] += np.eye(GD, dtype=np.float32)

    # rope permutation of the q/k OUTPUT index: out j <- out (j+32)%64 within
    # each 64-block (q block and k block separately)
    perm64 = (np.arange(GD) + HD) % GD
    perm128 = np.concatenate([perm64, GD + perm64])

    w_ap = attn_proj.sum(axis=2)
    w_mp = mlp_proj.sum(axis=2)

    # per-pair weight payloads
    payload = []
    for p in range(VSH):
        adw = np.zeros((L, 128, 512), np.float16)
        qkwA = np.zeros((L, 128, 256), np.float16)
        qpwA = np.zeros((L, 128, 256), np.float16)
        vwwA = np.zeros((L, 128, 128), np.float16)
        fcwA = np.zeros((L, 128, 512), np.float16)
        wapP = np.zeros((128, L), np.float32)
        wawP = np.zeros((128, n_ls), np.float32)
        wmwP = np.zeros((128, n_ls), np.float32)
        for l in range(L):
            for o in range(2):
                n = l * G + 2 * p + o
                rows = slice(o * 64, (o + 1) * 64)
                for k in range(KT):
                    adw[l, :, k * 128 + o * 64: k * 128 + (o + 1) * 64] = \
                        adapters_f[n, :, k * 128:(k + 1) * 128].T
                # zero-padded full-128-contraction stationaries (node o's
                # weights live on its own 64 rows; the rest stay zero)
                qkwA[l, rows, o * 128:(o + 1) * 128] = qkv_w[n, 0:128, :].T
                qpwA[l, rows, o * 128:(o + 1) * 128] = qkv_w[n, 0:128, :].T[:, perm128]
                vwwA[l, rows, o * 64:(o + 1) * 64] = qkv_w[n, 128:192, :].T
                fcwA[l, rows, o * 256:(o + 1) * 256] = mlp_fc[n].T
                wapP[o * 64:(o + 1) * 64, l] = w_ap[n]
        ls = 0
        for tt, layers in enumerate(active_sets):
            for l in layers:
                for o in range(2):
                    n = l * G + 2 * p + o
                    wawP[o * 64:(o + 1) * 64, ls] = w_ap[n] * w_eff[tt, n]
                    wmwP[o * 64:(o + 1) * 64, ls] = w_mp[n] * w_eff[tt, n]
                ls += 1
        payload.append((adw, qkwA, qpwA, vwwA, fcwA, wapP, wawP, wmwP))

    # constants
    c16 = np.zeros((128, 705), np.float16)
    ob = np.zeros((128, 128), np.float32)
    ob[0:64, 0:64] = 1.0 / GD
    ob[64:128, 64:128] = 1.0 / GD
    c16[:, 0:128] = ob.astype(np.float16)
    c16[:, 128:192] = 1.0
    c16[:, 192:193] = 1.0
    c16[0, 193:257] = 1.0
    c16[1, 257:321] = 1.0
    c16[0, 321:449] = 1.0
    s_i = np.arange(128)[:, None]
    t_i = np.arange(128)[None, :]
    tri = (s_i <= t_i).astype(np.float16)
    c16[:, 449:577] = tri
    c16[:, 577:705] = tri

    inv_freq = 1.0 / (10000.0 ** (np.arange(0, GD, 2, dtype=np.float64) / GD))
    freqs = np.outer(np.arange(T), inv_freq)
    cosT = np.cos(freqs).astype(np.float32).T
    sinT = np.sin(freqs).astype(np.float32).T
    cstf = np.zeros((128, 1155), np.float32)
    for blk in range(4):
        cstf[blk * 32:(blk + 1) * 32, 0:256] = cosT
        cstf[blk * 32:(blk + 1) * 32, 256:512] = cosT
        cstf[blk * 32:(blk + 1) * 32, 512:768] = sinT * (1.0 if blk % 2 == 0 else -1.0)
        cstf[blk * 32:(blk + 1) * 32, 768:1024] = sinT * (1.0 if blk % 2 == 0 else -1.0)
    cstf[:, 1024] = EPS
    cstf[0, 1025] = 1.0
    cstf[0, 1026] = -np.log(15.0)
    cstf[0, 1027:1155] = 1.0

    rwP = np.zeros((128, KT), np.float16)
    for k in range(KT):
        rwP[:, k] = router_w[k * 128:(k + 1) * 128, 0].astype(np.float16)
    rbias2 = np.full((1, 1), np.float32(router_b[0]), np.float32)

    x0 = _rms_np(wte[idx])  # (B, T, E) f32

    in_maps = []
    for c in range(NC):
        b, p = c // VSH, c % VSH
        lo = p * VW
        hi = min(lo + VW, V)
        lmt = np.zeros((E, VQ), np.float16)
        lmt[:, 0:hi - lo] = lm_head_w[lo:hi, :].T.astype(np.float16)
        adw, qkwA, qpwA, vwwA, fcwA, wapP, wawP, wmwP = payload[p]
        in_maps.append({
            "x0t": np.ascontiguousarray(x0[b].T), "adw": adw, "qkw": qkwA,
            "qpw": qpwA, "vww": vwwA, "fcw": fcwA, "c16": c16, "cstf": cstf,
            "wapP": wapP, "wawP": wawP, "wmwP": wmwP, "rwP": rwP,
            "rbias2": rbias2, "lmt": lmt,
        })
    return active_sets, in_maps


def kernel(idx, n_steps, wte, adapters, qkv_w, attn_proj, mlp_fc, mlp_proj,
           dep, router_w, router_b, lm_head_w):
    active_sets, in_maps = _host_prep(
        idx, n_steps, wte, adapters, qkv_w, attn_proj, mlp_fc, mlp_proj,
        dep, router_w, router_b, lm_head_w)

    if active_sets not in _PROGRAM_CACHE:
        _PROGRAM_CACHE[active_sets] = _build_program(active_sets)
    nc = _PROGRAM_CACHE[active_sets]

    trace = bool(int(os.environ.get("BASS_KERNEL_TRACE", "0")))
    res = run_bass_kernel_spmd(nc, in_maps, list(range(NC)), trace=trace)
    if trace and res.exec_time_ns is not None:
        print(f"HW exec time: {res.exec_time_ns} ns")

    out = np.zeros((B, T, V), np.float32)
    for c in range(NC):
        b, p = c // VSH, c % VSH
        lo = p * VW
        hi = min(lo + VW, V)
        out[b, :, lo:hi] = 15.0 * res.results[c]["out_lg"][:, 0:hi - lo].astype(np.float32)
    return out


# revision 21
# speedup vs baseline: 3.2426x; 1.0603x over previous
"""Trainium2 Bass kernel for nn_BG_ALRT_62921270886438 (moe_routing).

Sharding v2: core c -> (batch b = c // 4, pair p = c % 4).  Each core computes
only its pair's two nodes per active layer; the group-wise scatter-add target
of pair p is exactly E-rows [128p, 128p+128), so the per-step x update needs
only an AllGather (groups {0-3}, {4-7}) of each core's [128, T] acc slice.
lm_head is vocab-sharded 4 ways within each batch group (same output contract
as v1).  Matmuls run in fp16 (1 cycle/row vs 4 for fp32), x state stays fp32.

Self-contained: only numpy + the concourse toolchain on sys.path.
"""
import os

import numpy as np

import concourse.bacc as bacc
import concourse.tile as tile
from concourse import mybir
from concourse.alu_op_type import AluOpType
from concourse.bass_utils import run_bass_kernel_spmd

AF = mybir.ActivationFunctionType
F32 = mybir.dt.float32
F16 = mybir.dt.float16

B, T, E, G, GD, L, N, V = 2, 256, 512, 8, 64, 8, 64, 50257
HD = GD // 2          # 32, rope half
NC = 8                # cores
VSH = 4               # vocab shards per batch group
VW = (V + VSH - 1) // VSH          # 12565 raw shard width
VQ = ((VW + 511) // 512) * 512     # 12800 padded shard width
EPS = float(np.finfo(np.float32).eps)
KT = E // 128         # 4 contraction tiles over E
NVT = VQ // 512       # 25 vocab tiles of 512
NTT = T // 128        # 2 token tiles

_PROGRAM_CACHE = {}


def _tune_act_tables(arch):
    """Steer the act-table-load pass to one set for the whole step loop.

    All step-loop activations (square/ln/exp/relu/copy/identity) exist in
    `natural_log_exp_and_others`; tanh (lm head only) in `exp_and_others`.
    The pass picks the first listed set containing the function, which makes
    Ln and Exp resolve to different sets and thrash the single active table
    (~1.3us per reload).  get_activation_tables is functools.cache'd and
    returns the same dict object, so removing the overlapping functions from
    every other set (pure removals - set ids stay aligned with act_info.json)
    makes the combined set the unique choice.
    """
    from concourse.hw_specs import get_activation_tables
    tabs = get_activation_tables(arch)
    combined = tabs.get("natural_log_exp_and_others")
    if not combined:
        return
    for name, fns in tabs.items():
        if name != "natural_log_exp_and_others":
            fns.difference_update(combined)


def _build_program(active_sets):
    """active_sets: tuple of tuples - active layer list per step."""
    nc = bacc.Bacc("TRN2", target_bir_lowering=False, debug=False, num_devices=NC)
    _tune_act_tables(nc.m.arch)
    n_ls = max(sum(len(a) for a in active_sets), 1)
    groups = [[0, 1, 2, 3], [4, 5, 6, 7]]

    d_x0t = nc.dram_tensor("x0t", [E, T], F32, kind="ExternalInput")
    d_adw = nc.dram_tensor("adw", [L, 128, 512], F16, kind="ExternalInput")
    d_qkw = nc.dram_tensor("qkw", [L, 128, 256], F16, kind="ExternalInput")
    d_qpw = nc.dram_tensor("qpw", [L, 128, 256], F16, kind="ExternalInput")
    d_vww = nc.dram_tensor("vww", [L, 128, 128], F16, kind="ExternalInput")
    d_fcw = nc.dram_tensor("fcw", [L, 128, 512], F16, kind="ExternalInput")
    d_c16 = nc.dram_tensor("c16", [128, 705], F16, kind="ExternalInput")
    d_cf = nc.dram_tensor("cstf", [128, 1155], F32, kind="ExternalInput")
    d_wap = nc.dram_tensor("wapP", [128, L], F32, kind="ExternalInput")
    d_waw = nc.dram_tensor("wawP", [128, n_ls], F32, kind="ExternalInput")
    d_wmw = nc.dram_tensor("wmwP", [128, n_ls], F32, kind="ExternalInput")
    d_rw = nc.dram_tensor("rwP", [128, KT], F16, kind="ExternalInput")
    d_rb = nc.dram_tensor("rbias2", [1, 1], F32, kind="ExternalInput")
    d_lm = nc.dram_tensor("lmt", [E, VQ], F16, kind="ExternalInput")
    d_out = nc.dram_tensor("out_lg", [T, VQ], F16, kind="ExternalOutput")

    with tile.TileContext(nc) as tc:
        with tc.tile_pool(name="cst", bufs=1) as cst, \
             tc.tile_pool(name="st", bufs=1) as st, \
             tc.tile_pool(name="wk16", bufs=3) as wk16, \
             tc.tile_pool(name="wkf", bufs=2) as wkf, \
             tc.tile_pool(name="vsb", bufs=4) as vsb, \
             tc.tile_pool(name="ps", bufs=1, space="PSUM") as ps, \
             tc.tile_pool(name="dram", bufs=20, space="DRAM") as dram:

            # ---------------- constants ----------------
            c16 = cst.tile([128, 705], F16, tag="c16", name="c16")
            nc.sync.dma_start(c16[:], d_c16.ap())
            oblk = c16[:, 0:128]            # block-diag(64) of 1/64
            ocol = c16[:, 128:192]          # (128,64) ones
            oc1 = c16[:, 192:193]           # (128,1) ones
            sel2 = c16[0:2, 193:321]        # row0 -> parts 0:64, row1 -> 64:128
            onesrow = c16[0:1, 321:449]     # (1,128) ones
            tri2 = c16[:, 449:705]          # [tri | tri] fp16

            cf = cst.tile([128, 1155], F32, tag="cf", name="cf")
            nc.sync.dma_start(cf[:], d_cf.ap())
            CC2 = cf[:, 0:512]              # [C | C]
            SS2 = cf[:, 512:1024]           # [S | S]
            eps128 = cf[:, 1024:1025]
            eps1 = cf[0:1, 1024:1025]
            one_f = cf[0:1, 1025:1026]      # 1.0 (transpose identity)
            mln15 = cf[0:1, 1026:1027]      # -ln(15)
            orowf = cf[0:1, 1027:1155]      # (1,128) ones f32

            wap = cst.tile([128, L], F32, tag="wap", name="wap")
            nc.sync.dma_start(wap[:], d_wap.ap())
            waw = cst.tile([128, n_ls], F32, tag="waw", name="waw")
            nc.sync.dma_start(waw[:], d_waw.ap())
            wmw = cst.tile([128, n_ls], F32, tag="wmw", name="wmw")
            nc.sync.dma_start(wmw[:], d_wmw.ap())
            rw = cst.tile([128, KT], F16, tag="rw", name="rw")
            nc.sync.dma_start(rw[:], d_rw.ap())
            rbias2 = cst.tile([1, 1], F32, tag="rbias2", name="rbias2")
            nc.sync.dma_start(rbias2[:], d_rb.ap())

            adw, qkw, qpw, vww, fcw = [], [], [], [], []
            for l in range(L):
                a_t = cst.tile([128, 512], F16, tag=f"adw{l}", name=f"adw{l}")
                nc.sync.dma_start(a_t[:], d_adw.ap()[l])
                adw.append(a_t)
                q_t = cst.tile([128, 256], F16, tag=f"qkw{l}", name=f"qkw{l}")
                nc.sync.dma_start(q_t[:], d_qkw.ap()[l])
                qkw.append(q_t)
                p_t = cst.tile([128, 256], F16, tag=f"qpw{l}", name=f"qpw{l}")
                nc.sync.dma_start(p_t[:], d_qpw.ap()[l])
                qpw.append(p_t)
                v_t = cst.tile([128, 128], F16, tag=f"vww{l}", name=f"vww{l}")
                nc.sync.dma_start(v_t[:], d_vww.ap()[l])
                vww.append(v_t)
                f_t = cst.tile([128, 512], F16, tag=f"fcw{l}", name=f"fcw{l}")
                nc.sync.dma_start(f_t[:], d_fcw.ap()[l])
                fcw.append(f_t)

            # lm_head weights: full shard resident in SBUF, chunked DMA so the
            # prefetch never head-of-line blocks the per-step bounce DMAs.
            lmsb = []
            LCH = 1600
            for k in range(KT):
                t_ = cst.tile([128, VQ], F16, tag=f"lm{k}", name=f"lm{k}")
                lmsb.append(t_)
                for c0 in range(0, VQ, LCH):
                    nc.sync.dma_start(
                        t_[:, c0:c0 + LCH],
                        d_lm.ap()[k * 128:(k + 1) * 128, c0:c0 + LCH])

            # ---------------- state ----------------
            xT = [st.tile([128, T], F32, tag=f"xT{k}", name=f"xT{k}") for k in range(KT)]
            xr = [st.tile([128, T], F16, tag=f"xr{k}", name=f"xr{k}") for k in range(KT)]
            acc = st.tile([128, T], F32, tag="acc", name="acc")
            xg = st.tile([128, KT * T], F32, tag="xg", name="xg")
            pcont = st.tile([1, T], F32, tag="pcont", name="pcont")
            nc.vector.memset(pcont[:], 1.0)
            nc.gpsimd.memset(acc[:], 0.0)

            # initial x (rms applied host-side)
            for k in range(KT):
                nc.sync.dma_start(xT[k][:], d_x0t.ap()[k * 128:(k + 1) * 128, :])
                with nc.allow_low_precision(reason="fp16 compute"):
                    nc.vector.tensor_copy(xr[k][:], xT[k][:])

            # CC warm-up: dummy AllGather so the first real one is cheap
            NO_CC = bool(int(os.environ.get("BASS_V2_NO_CC", "0")))
            db_in = dram.tile([128, 8], F32, tag="dbi", name="dbi")
            db_out = dram.tile([512, 8], F32, tag="dbo", name="dbo")
            nc.sync.dma_start(db_in[:], cf[:, 0:8])
            if not NO_CC:
                nc.gpsimd.collective_compute(
                    "AllGather", mybir.AluOpType.bypass, replica_groups=groups,
                    ins=[db_in[:].opt()], outs=[db_out[:].opt()])

            ls_idx = 0
            with nc.allow_low_precision(reason="fp16 compute"):
                for t, layers in enumerate(active_sets):
                    for l in layers:
                        # ---- xi = adapters' @ x (identity folded in) ----
                        H1 = ps.tile([128, 2 * T], F32, tag="H1", bufs=1, name="ps")
                        p_xi = H1[:, 0:T]
                        p_s1 = H1[:, T:2 * T]
                        for k in range(KT):
                            nc.tensor.matmul(
                                p_xi[:], adw[l][:, k * 128:(k + 1) * 128],
                                xr[k][:], start=(k == 0), stop=(k == KT - 1))
                        xi = wk16.tile([128, T], F16, tag="xi", name="xi")
                        nc.vector.tensor_copy(xi[:], p_xi[:])

                        # ---- v per s-block (both nodes at once; vww is
                        # host-zero-padded so full-128 contraction is exact)
                        S1 = ps.tile([128, 2 * T], F32, tag="S1", bufs=1, name="ps")
                        p_v = S1[:, 0:T]
                        p_sr0 = S1[0:64, T:2 * T]
                        v_sb = [None, None]
                        for s in range(2):
                            nc.tensor.matmul(
                                p_v[:, s * 128:(s + 1) * 128],
                                xi[:, s * 128:(s + 1) * 128],
                                vww[l][:], start=True, stop=True)
                            vt = vsb.tile([128, 130], F16, tag="vt", name="vt")
                            if s == 0:
                                nc.scalar.copy(vt[:, 0:64], p_v[:, 0:64])
                                nc.scalar.copy(vt[:, 65:129], p_v[:, 64:128])
                            else:
                                nc.vector.tensor_copy(vt[:, 0:64], p_v[:, 128:192])
                                nc.vector.tensor_copy(vt[:, 65:129], p_v[:, 192:256])
                            nc.gpsimd.memset(vt[:, 64:65], 1.0)
                            nc.gpsimd.memset(vt[:, 129:130], 1.0)
                            v_sb[s] = vt

                        # ---- q/k (raw + pre-permuted), both nodes packed ----
                        p_qk = ps.tile([128, 2 * T], F32, tag="A", bufs=2, name="ps")
                        p_qp = ps.tile([128, 2 * T], F32, tag="B", bufs=1, name="ps")
                        for o in range(2):
                            nc.tensor.matmul(p_qk[:, o * T:(o + 1) * T],
                                             qkw[l][:, o * 128:(o + 1) * 128],
                                             xi[:], start=True, stop=True)
                            nc.tensor.matmul(p_qp[:, o * T:(o + 1) * T],
                                             qpw[l][:, o * 128:(o + 1) * 128],
                                             xi[:], start=True, stop=True)

                        # rms scale from pre-rope q/k (rope is norm-preserving)
                        sq = wk16.tile([128, 2 * T], F16, tag="sq", name="sq")
                        nc.scalar.activation(sq[:], p_qk[:], AF.Square)
                        p_ms = ps.tile([128, 2 * T], F32, tag="C", bufs=1, name="ps")
                        nc.tensor.matmul(p_ms[:], oblk, sq[:], start=True, stop=True)
                        lnm = wkf.tile([128, 2 * T], F32, tag="srt", name="lnm")
                        nc.scalar.activation(lnm[:], p_ms[:], AF.Ln, bias=eps128)
                        rsq = wk16.tile([128, 2 * T], F16, tag="rsq", name="rsq")
                        nc.scalar.activation(rsq[:], lnm[:], AF.Exp, scale=-0.5)

                        # rope: rot = qk*C + qp*S, then normalize + split q/k
                        t1 = wk16.tile([128, 2 * T], F16, bufs=2, tag="t1", name="t1")
                        nc.vector.tensor_tensor(t1[:], p_qk[:], CC2, AluOpType.mult)
                        t2 = wk16.tile([128, 2 * T], F16, bufs=2, tag="t2", name="t2")
                        nc.vector.tensor_tensor(t2[:], p_qp[:], SS2, AluOpType.mult)
                        rop = wk16.tile([128, 2 * T], F16, bufs=2, tag="rop", name="rop")
                        nc.vector.tensor_tensor(rop[:], t1[:], t2[:], AluOpType.add)
                        qt = wk16.tile([128, T], F16, tag="qt", name="qt")
                        kt = wk16.tile([128, 2 * T], F16, tag="kt", name="kt")
                        nc.gpsimd.memset(kt[64:128, 0:T], 0.0)
                        nc.gpsimd.memset(kt[0:64, T:2 * T], 0.0)
                        for o in range(2):
                            orows = slice(64 * o, 64 * o + 64)
                            nc.vector.tensor_tensor(
                                qt[orows, :], rop[0:64, o * T:(o + 1) * T],
                                rsq[0:64, o * T:(o + 1) * T], AluOpType.mult)
                            nc.vector.tensor_tensor(
                                kt[orows, o * T:(o + 1) * T],
                                rop[64:128, o * T:(o + 1) * T],
                                rsq[64:128, o * T:(o + 1) * T], AluOpType.mult)

                        # ---- scores -> masked exp ----
                        p_s0 = ps.tile([128, 2 * T], F32, tag="C", bufs=1, name="ps")
                        for o in range(2):
                            nc.tensor.matmul(p_s0[:, o * T:(o + 1) * T],
                                             kt[:, o * T:o * T + 128], qt[:],
                                             start=True, stop=True)
                            nc.tensor.matmul(p_s1[:, o * 128:(o + 1) * 128],
                                             kt[:, o * T + 128:(o + 1) * T],
                                             qt[:, 128:256],
                                             start=True, stop=True)
                        em0 = wk16.tile([128, 2 * T], F16, tag="em0", name="em0")
                        nc.scalar.activation(em0[:], p_s0[:], AF.Exp, scale=0.125)
                        em1 = wk16.tile([128, T], F16, tag="em1", name="em1")
                        nc.scalar.activation(em1[:], p_s1[:], AF.Exp, scale=0.125)
                        # masked diagonal blocks (separate tiles, no in-place)
                        m0 = wk16.tile([128, T], F16, tag="m0", name="m0")
                        nc.gpsimd.tensor_tensor(m0[:, 0:128], em0[:, 0:128],
                                                tri2[:, 0:128], AluOpType.mult)
                        nc.gpsimd.tensor_tensor(m0[:, 128:256], em0[:, T:T + 128],
                                                tri2[:, 0:128], AluOpType.mult)
                        m1 = wk16.tile([128, T], F16, tag="m1", name="m1")
                        nc.gpsimd.tensor_tensor(m1[:], em1[:], tri2, AluOpType.mult)

                        # ---- att (+colsum via ones col) ----
                        S2 = ps.tile([128, 2 * T], F32, tag="S2", bufs=1, name="ps")
                        p_att = [S2[0:65, 0:T], S2[0:65, T:2 * T]]
                        for o in range(2):
                            pa = p_att[o]
                            nc.tensor.matmul(pa[:, 0:128],
                                             v_sb[0][:, o * 65:(o + 1) * 65],
                                             m0[:, o * 128:(o + 1) * 128],
                                             start=True, stop=True)
                            nc.tensor.matmul(pa[:, 128:256],
                                             v_sb[0][:, o * 65:(o + 1) * 65],
                                             em0[:, o * T + 128:(o + 1) * T],
                                             start=True, stop=False)
                            nc.tensor.matmul(pa[:, 128:256],
                                             v_sb[1][:, o * 65:(o + 1) * 65],
                                             m1[:, o * 128:(o + 1) * 128],
                                             start=False, stop=True)
                        rcl = wkf.tile([1, 2 * T], F32, bufs=1, tag="rcl", name="rcl")
                        nc.scalar.activation(rcl[:], S2[64:65, 0:2 * T], AF.Ln)
                        rc2 = wkf.tile([1, 2 * T], F32, bufs=1, tag="rc2", name="rc2")
                        nc.scalar.activation(rc2[:], rcl[:], AF.Exp, scale=-1.0)
                        H2 = ps.tile([128, 2 * T], F32, tag="H2", bufs=1, name="ps")
                        nc.tensor.matmul(H2[:], orowf, rc2[:], start=True, stop=True)
                        att_sb = wk16.tile([128, T], F16, tag="att", name="att")
                        nc.scalar.copy(att_sb[0:64, :], p_att[0][0:64, :])
                        nc.scalar.copy(att_sb[64:128, :], p_att[1][0:64, :])
                        tt = wk16.tile([128, T], F16, tag="tt", name="tt")
                        nc.vector.tensor_tensor(tt[0:64, :], att_sb[0:64, :],
                                                H2[0:64, 0:T], AluOpType.mult)
                        nc.vector.tensor_tensor(tt[64:128, :], att_sb[64:128, :],
                                                H2[64:128, T:2 * T], AluOpType.mult)

                        xim = wk16.tile([128, T], F16, tag="xim", name="xim")
                        nc.vector.scalar_tensor_tensor(
                            xim[:], tt[:], wap[:, l:l + 1], xi[:],
                            AluOpType.mult, AluOpType.add)
                        nc.vector.scalar_tensor_tensor(
                            acc[:], tt[:], waw[:, ls_idx:ls_idx + 1], acc[:],
                            AluOpType.mult, AluOpType.add)

                        # ---- mlp (rms folded into 1/(mean+eps) post-scale) ----
                        sqm = wk16.tile([128, T], F16, tag="sqm", name="sqm")
                        nc.gpsimd.tensor_tensor(sqm[:], xim[:], xim[:],
                                                AluOpType.mult)
                        p_mq = ps.tile([128, T], F32, tag="H1", bufs=1, name="ps")
                        nc.tensor.matmul(p_mq[:], oblk, sqm[:], start=True, stop=True)
                        lnm2 = wkf.tile([128, T], F32, bufs=1, tag="pre", name="lnm2")
                        nc.scalar.activation(lnm2[:], p_mq[:], AF.Ln, bias=eps128)
                        rec2 = wk16.tile([128, T], F16, tag="rec2", name="rec2")
                        nc.scalar.activation(rec2[:], lnm2[:], AF.Exp, scale=-1.0)

                        p_srs = []
                        for o in range(2):
                            p_fc = ps.tile([128, 2 * T], F32, tag="A" if o == 0 else "B",
                                            bufs=2 if o == 0 else 1, name="ps")
                            for h in range(2):
                                nc.tensor.matmul(
                                    p_fc[:, h * T:(h + 1) * T],
                                    fcw[l][:, o * 256 + h * 128:o * 256 + (h + 1) * 128],
                                    xim[:], start=True, stop=True)
                            frel = wk16.tile([128, 2 * T], F16, tag="frel", name="frel")
                            nc.scalar.activation(frel[:], p_fc[:], AF.Relu)
                            rsq2 = wk16.tile([128, 2 * T], F16, tag="rsq2", name="rsq2")
                            nc.gpsimd.tensor_tensor(rsq2[:], frel[:], frel[:],
                                                    AluOpType.mult)
                            if o == 0:
                                p_sr = p_sr0
                            else:
                                p_sr = ps.tile([64, T], F32, tag="H2", bufs=1, name="ps")
                            p_srs.append(p_sr)
                            nc.tensor.matmul(p_sr[:], ocol, rsq2[:, 0:T],
                                             start=True, stop=False)
                            nc.tensor.matmul(p_sr[:], ocol, rsq2[:, T:2 * T],
                                             start=False, stop=True)
                        hm = wk16.tile([128, T], F16, tag="hm", name="hm")
                        nc.vector.tensor_tensor(hm[0:64, :], p_srs[0][:],
                                                rec2[0:64, :], AluOpType.mult)
                        nc.vector.tensor_tensor(hm[64:128, :], p_srs[1][:],
                                                rec2[64:128, :], AluOpType.mult)
                        nc.vector.scalar_tensor_tensor(
                            acc[:], hm[:], wmw[:, ls_idx:ls_idx + 1], acc[:],
                            AluOpType.mult, AluOpType.add)
                        ls_idx += 1

                    # ---- step sync: scale acc by pcont, AllGather, update x ----
                    p_pc = ps.tile([128, T], F32, tag="H1", bufs=1, name="ps")
                    nc.tensor.matmul(p_pc[:], orowf, pcont[:], start=True, stop=True)
                    acc2 = wkf.tile([128, T], F32, bufs=1, tag="acc2", name="acc2")
                    nc.vector.tensor_tensor(acc2[:], acc[:], p_pc[:], AluOpType.mult)
                    nc.gpsimd.memset(acc[:], 0.0)
                    b_in = dram.tile([128, T], F32, tag="bin", name=f"bin{t}")
                    b_out = dram.tile([KT * 128, T], F32, tag="bout", name=f"bout{t}")
                    nc.sync.dma_start(b_in[:], acc2[:])
                    if not NO_CC:
                        nc.gpsimd.collective_compute(
                            "AllGather", mybir.AluOpType.bypass, replica_groups=groups,
                            ins=[b_in[:].opt()], outs=[b_out[:].opt()])
                        for k in range(KT):
                            nc.sync.dma_start(xg[:, k * T:(k + 1) * T],
                                              b_out[k * 128:(k + 1) * 128, :])
                    else:
                        for k in range(KT):
                            nc.sync.dma_start(xg[:, k * T:(k + 1) * T], b_in[:])
                    for k in range(KT):
                        nc.vector.tensor_tensor(xT[k][:], xT[k][:],
                                                xg[:, k * T:(k + 1) * T],
                                                AluOpType.add)
                        nc.vector.tensor_copy(xr[k][:], xT[k][:])

                    # ---- router: pcont *= 1 - sigmoid(x@rw + rb) ----
                    p_ph = ps.tile([1, T], F32, tag="S2", bufs=1, name="ps")
                    for k in range(KT):
                        nc.tensor.matmul(p_ph[:], rw[:, k:k + 1], xr[k][:],
                                         start=(k == 0), stop=(k == KT - 1))
                    ez = wkf.tile([1, T], F32, bufs=1, tag="th", name="ez")
                    nc.scalar.activation(ez[:], p_ph[:], AF.Exp, bias=rbias2[:])
                    ez1 = wkf.tile([1, T], F32, bufs=1, tag="omp", name="ez1")
                    nc.vector.tensor_scalar(ez1[:], ez[:], 1.0, 1.0,
                                            AluOpType.mult, AluOpType.add)
                    lz = wkf.tile([1, T], F32, bufs=1, tag="lz", name="lz")
                    nc.scalar.activation(lz[:], ez1[:], AF.Ln)
                    omp = wkf.tile([1, T], F32, bufs=1, tag="omp2", name="omp")
                    nc.scalar.activation(omp[:], lz[:], AF.Exp, scale=-1.0)
                    nc.vector.tensor_tensor(pcont[:], pcont[:], omp[:],
                                            AluOpType.mult)

                # ---------------- final rms + lm_head ----------------
                p_mr = ps.tile([1, T], F32, tag="S1", bufs=1, name="ps")
                for k in range(KT):
                    sqf = wk16.tile([128, T], F16, tag="sqf", name="sqf")
                    nc.scalar.activation(sqf[:], xr[k][:], AF.Square)
                    nc.tensor.matmul(p_mr[:], oc1, sqf[:],
                                     start=(k == 0), stop=(k == KT - 1))
                lnf = wkf.tile([1, T], F32, bufs=1, tag="rr", name="lnf")
                nc.scalar.activation(lnf[:], p_mr[:], AF.Ln, bias=eps1,
                                     scale=1.0 / E)
                rr15 = wkf.tile([1, T], F32, bufs=1, tag="rr15", name="rr15")
                nc.scalar.activation(rr15[:], lnf[:], AF.Exp, scale=-0.5,
                                     bias=mln15)
                rcol = []
                for i in range(NTT):
                    p_tr = ps.tile([128, 1], F32, tag="S2", bufs=1, name="ptr")
                    nc.tensor.transpose(p_tr[:], rr15[:, i * 128:(i + 1) * 128],
                                        one_f)
                    rc = st.tile([128, 1], F32, tag=f"rcol{i}", name=f"rcol{i}")
                    nc.scalar.copy(rc[:], p_tr[:])
                    rcol.append(rc)

                for i in range(NTT):
                    for v in range(NVT):
                        p_lg = ps.tile([128, 512], F32, tag="A", bufs=2, name="ps")
                        for k in range(KT):
                            nc.tensor.matmul(
                                p_lg[:], xr[k][:, i * 128:(i + 1) * 128],
                                lmsb[k][:, v * 512:(v + 1) * 512],
                                start=(k == 0), stop=(k == KT - 1))
                        lth = wk16.tile([128, 512], F16, tag="lth", name="lth")
                        nc.scalar.activation(lth[:], p_lg[:], AF.Tanh,
                                             scale=rcol[i][:])
                        nc.sync.dma_start(
                            d_out.ap()[i * 128:(i + 1) * 128,
                                       v * 512:(v + 1) * 512],
                            lth[:])

    nc.compile()
    return nc


def _rms_np(x):
    return x * (1.0 / np.sqrt(np.mean(x * x, axis=-1, keepdims=True) + EPS))


def _host_prep(idx, n_steps, wte, adapters, qkv_w, attn_proj, mlp_fc, mlp_proj,
               dep, router_w, router_b, lm_head_w):
    idx = np.asarray(idx)
    wte = np.asarray(wte, np.float32)
    adapters = np.asarray(adapters, np.float32)
    qkv_w = np.asarray(qkv_w, np.float32)
    attn_proj = np.asarray(attn_proj, np.float32)
    mlp_fc = np.asarray(mlp_fc, np.float32)
    mlp_proj = np.asarray(mlp_proj, np.float32)
    dep = np.asarray(dep, np.float32)
    router_w = np.asarray(router_w, np.float32).reshape(E, 1)
    router_b = np.asarray(router_b, np.float32).reshape(-1)
    lm_head_w = np.asarray(lm_head_w, np.float32)
    ns = int(n_steps)

    dp = np.maximum(dep, 0.0)
    depths = np.zeros((N,), np.float32)
    for _ in range(L):
        depths = (dp @ (depths + 1.0)).astype(np.float32)

    w_eff = np.zeros((ns, N), np.float32)
    active_sets = []
    for t in range(ns):
        td = t * (L / ns)
        w_all = np.exp(-np.abs(depths - np.float32(td))).astype(np.float32)
        w = np.where(w_all > 0.15, w_all, 0.0).astype(np.float32)
        w_eff[t] = w
        active_sets.append(tuple(sorted({n // G for n in range(N) if w[n] > 0})))
    active_sets = tuple(active_sets)
    n_ls = max(sum(len(a) for a in active_sets), 1)

    # fold the group-slice identity into the adapters
    adapters_f = adapters.copy()
    for n in range(N):
        g = n % G
        adapters_f[n, :, g * GD:(g + 1) * GD] += np.eye(GD, dtype=np.float32)

    # rope permutation of the q/k OUTPUT index: out j <- out (j+32)%64 within
    # each 64-block (q block and k block separately)
    perm64 = (np.arange(GD) + HD) % GD
    perm128 = np.concatenate([perm64, GD + perm64])

    w_ap = attn_proj.sum(axis=2)
    w_mp = mlp_proj.sum(axis=2)

    # per-pair weight payloads
    payload = []
    for p in range(VSH):
        adw = np.zeros((L, 128, 512), np.float16)
        qkwA = np.zeros((L, 128, 256), np.float16)
        qpwA = np.zeros((L, 128, 256), np.float16)
        vwwA = np.zeros((L, 128, 128), np.float16)
        fcwA = np.zeros((L, 128, 512), np.float16)
        wapP = np.zeros((128, L), np.float32)
        wawP = np.zeros((128, n_ls), np.float32)
        wmwP = np.zeros((128, n_ls), np.float32)
        for l in range(L):
            for o in range(2):
                n = l * G + 2 * p + o
                rows = slice(o * 64, (o + 1) * 64)
                for k in range(KT):
                    adw[l, :, k * 128 + o * 64: k * 128 + (o + 1) * 64] = \
                        adapters_f[n, :, k * 128:(k + 1) * 128].T
                # zero-padded full-128-contraction stationaries (node o's
                # weights live on its own 64 rows; the rest stay zero)
                qkwA[l, rows, o * 128:(o + 1) * 128] = qkv_w[n, 0:128, :].T
                qpwA[l, rows, o * 128:(o + 1) * 128] = qkv_w[n, 0:128, :].T[:, perm128]
                vwwA[l, rows, o * 64:(o + 1) * 64] = qkv_w[n, 128:192, :].T
                fcwA[l, rows, o * 256:(o + 1) * 256] = mlp_fc[n].T
                wapP[o * 64:(o + 1) * 64, l] = w_ap[n]
        ls = 0
        for tt, layers in enumerate(active_sets):
            for l in layers:
                for o in range(2):
                    n = l * G + 2 * p + o
                    wawP[o * 64:(o + 1) * 64, ls] = w_ap[n] * w_eff[tt, n]
                    wmwP[o * 64:(o + 1) * 64, ls] = w_mp[n] * w_eff[tt, n]
                ls += 1
        payload.append((adw, qkwA, qpwA, vwwA, fcwA, wapP, wawP, wmwP))

    # constants
    c16 = np.zeros((128, 705), np.float16)
    ob = np.zeros((128, 128), np.float32)
    ob[0:64, 0:64] = 1.0 / GD
    ob[64:128, 64:128] = 1.0 / GD
    c16[:, 0:128] = ob.astype(np.float16)
    c16[:, 128:192] = 1.0
    c16[:, 192:193] = 1.0
    c16[0, 193:257] = 1.0
    c16[1, 257:321] = 1.0
    c16[0, 321:449] = 1.0
    s_i = np.arange(128)[:, None]
    t_i = np.arange(128)[None, :]
    tri = (s_i <= t_i).astype(np.float16)
    c16[:, 449:577] = tri
    c16[:, 577:705] = tri

    inv_freq = 1.0 / (10000.0 ** (np.arange(0, GD, 2, dtype=np.float64) / GD))
    freqs = np.outer(np.arange(T), inv_freq)
    cosT = np.cos(freqs).astype(np.float32).T
    sinT = np.sin(freqs).astype(np.float32).T
    cstf = np.zeros((128, 1155), np.float32)
    for blk in range(4):
        cstf[blk * 32:(blk + 1) * 32, 0:256] = cosT
        cstf[blk * 32:(blk + 1) * 32, 256:512] = cosT
        cstf[blk * 32:(blk + 1) * 32, 512:768] = sinT * (1.0 if blk % 2 == 0 else -1.0)
        cstf[blk * 32:(blk + 1) * 32, 768:1024] = sinT * (1.0 if blk % 2 == 0 else -1.0)
    cstf[:, 1024] = EPS
    cstf[0, 1025] = 1.0
    cstf[0, 1026] = -np.log(15.0)
    cstf[0, 1027:1155] = 1.0

    rwP = np.zeros((128, KT), np.float16)
    for k in range(KT):
        rwP[:, k] = router_w[k * 128:(k + 1) * 128, 0].astype(np.float16)
    rbias2 = np.full((1, 1), np.float32(router_b[0]), np.float32)

    x0 = _rms_np(wte[idx])  # (B, T, E) f32

    in_maps = []
    for c in range(NC):
        b, p = c // VSH, c % VSH
        lo = p * VW
        hi = min(lo + VW, V)
        lmt = np.zeros((E, VQ), np.float16)
        lmt[:, 0:hi - lo] = lm_head_w[lo:hi, :].T.astype(np.float16)
        adw, qkwA, qpwA, vwwA, fcwA, wapP, wawP, wmwP = payload[p]
        in_maps.append({
            "x0t": np.ascontiguousarray(x0[b].T), "adw": adw, "qkw": qkwA,
            "qpw": qpwA, "vww": vwwA, "fcw": fcwA, "c16": c16, "cstf": cstf,
            "wapP": wapP, "wawP": wawP, "wmwP": wmwP, "rwP": rwP,
            "rbias2": rbias2, "lmt": lmt,
        })
    return active_sets, in_maps


def kernel(idx, n_steps, wte, adapters, qkv_w, attn_proj, mlp_fc, mlp_proj,
           dep, router_w, router_b, lm_head_w):
    active_sets, in_maps = _host_prep(
        idx, n_steps, wte, adapters, qkv_w, attn_proj, mlp_fc, mlp_proj,
        dep, router_w, router_b, lm_head_w)

    if active_sets not in _PROGRAM_CACHE:
        _PROGRAM_CACHE[active_sets] = _build_program(active_sets)
    nc = _PROGRAM_CACHE[active_sets]

    trace = bool(int(os.environ.get("BASS_KERNEL_TRACE", "0")))
    res = run_bass_kernel_spmd(nc, in_maps, list(range(NC)), trace=trace)
    if trace and res.exec_time_ns is not None:
        print(f"HW exec time: {res.exec_time_ns} ns")

    out = np.zeros((B, T, V), np.float32)
    for c in range(NC):
        b, p = c // VSH, c % VSH
        lo = p * VW
        hi = min(lo + VW, V)
        out[b, :, lo:hi] = 15.0 * res.results[c]["out_lg"][:, 0:hi - lo].astype(np.float32)
    return out


# revision 22
# speedup vs baseline: 3.3877x; 1.0447x over previous
"""Trainium2 Bass kernel for nn_BG_ALRT_62921270886438 (moe_routing).

Sharding v2: core c -> (batch b = c // 4, pair p = c % 4).  Each core computes
only its pair's two nodes per active layer; the group-wise scatter-add target
of pair p is exactly E-rows [128p, 128p+128), so the per-step x update needs
only an AllGather (groups {0-3}, {4-7}) of each core's [128, T] acc slice.
lm_head is vocab-sharded 4 ways within each batch group (same output contract
as v1).  Matmuls run in fp16 (1 cycle/row vs 4 for fp32), x state stays fp32.

Self-contained: only numpy + the concourse toolchain on sys.path.
"""
import os

import numpy as np

import concourse.bacc as bacc
import concourse.tile as tile
from concourse import mybir
from concourse.alu_op_type import AluOpType
from concourse.bass_utils import run_bass_kernel_spmd

AF = mybir.ActivationFunctionType
F32 = mybir.dt.float32
F16 = mybir.dt.float16

B, T, E, G, GD, L, N, V = 2, 256, 512, 8, 64, 8, 64, 50257
HD = GD // 2          # 32, rope half
NC = 8                # cores
VSH = 4               # vocab shards per batch group
VW = (V + VSH - 1) // VSH          # 12565 raw shard width
VQ = ((VW + 511) // 512) * 512     # 12800 padded shard width
EPS = float(np.finfo(np.float32).eps)
KT = E // 128         # 4 contraction tiles over E
NVT = VQ // 512       # 25 vocab tiles of 512
NTT = T // 128        # 2 token tiles

_PROGRAM_CACHE = {}


def _tune_act_tables(arch):
    """Steer the act-table-load pass to one set for the whole step loop.

    All step-loop activations (square/ln/exp/relu/copy/identity) exist in
    `natural_log_exp_and_others`; tanh (lm head only) in `exp_and_others`.
    The pass picks the first listed set containing the function, which makes
    Ln and Exp resolve to different sets and thrash the single active table
    (~1.3us per reload).  get_activation_tables is functools.cache'd and
    returns the same dict object, so removing the overlapping functions from
    every other set (pure removals - set ids stay aligned with act_info.json)
    makes the combined set the unique choice.
    """
    from concourse.hw_specs import get_activation_tables
    tabs = get_activation_tables(arch)
    combined = tabs.get("natural_log_exp_and_others")
    if not combined:
        return
    for name, fns in tabs.items():
        if name != "natural_log_exp_and_others":
            fns.difference_update(combined)


def _build_program(active_sets):
    """active_sets: tuple of tuples - active layer list per step."""
    nc = bacc.Bacc("TRN2", target_bir_lowering=False, debug=False, num_devices=NC)
    _tune_act_tables(nc.m.arch)
    n_ls = max(sum(len(a) for a in active_sets), 1)
    groups = [[0, 1, 2, 3], [4, 5, 6, 7]]

    d_x0t = nc.dram_tensor("x0t", [E, T], F32, kind="ExternalInput")
    d_adw = nc.dram_tensor("adw", [L, 128, 512], F16, kind="ExternalInput")
    d_qkw = nc.dram_tensor("qkw", [L, 128, 256], F16, kind="ExternalInput")
    d_qpw = nc.dram_tensor("qpw", [L, 128, 256], F16, kind="ExternalInput")
    d_vww = nc.dram_tensor("vww", [L, 128, 128], F16, kind="ExternalInput")
    d_fcw = nc.dram_tensor("fcw", [L, 128, 512], F16, kind="ExternalInput")
    d_c16 = nc.dram_tensor("c16", [128, 705], F16, kind="ExternalInput")
    d_cf = nc.dram_tensor("cstf", [128, 1155], F32, kind="ExternalInput")
    d_wap = nc.dram_tensor("wapP", [128, L], F32, kind="ExternalInput")
    d_waw = nc.dram_tensor("wawP", [128, n_ls], F32, kind="ExternalInput")
    d_wmw = nc.dram_tensor("wmwP", [128, n_ls], F32, kind="ExternalInput")
    d_rw = nc.dram_tensor("rwP", [128, KT], F16, kind="ExternalInput")
    d_rb = nc.dram_tensor("rbias2", [1, 1], F32, kind="ExternalInput")
    d_lm = nc.dram_tensor("lmt", [E, VQ], F16, kind="ExternalInput")
    d_out = nc.dram_tensor("out_lg", [T, VQ], F16, kind="ExternalOutput")

    with tile.TileContext(nc) as tc:
        with tc.tile_pool(name="cst", bufs=1) as cst, \
             tc.tile_pool(name="st", bufs=1) as st, \
             tc.tile_pool(name="wk16", bufs=3) as wk16, \
             tc.tile_pool(name="wkf", bufs=2) as wkf, \
             tc.tile_pool(name="vsb", bufs=4) as vsb, \
             tc.tile_pool(name="ps", bufs=1, space="PSUM") as ps, \
             tc.tile_pool(name="dram", bufs=20, space="DRAM") as dram:

            # ---------------- constants ----------------
            c16 = cst.tile([128, 705], F16, tag="c16", name="c16")
            nc.sync.dma_start(c16[:], d_c16.ap())
            oblk = c16[:, 0:128]            # block-diag(64) of 1/64
            ocol = c16[:, 128:192]          # (128,64) ones
            oc1 = c16[:, 192:193]           # (128,1) ones
            sel2 = c16[0:2, 193:321]        # row0 -> parts 0:64, row1 -> 64:128
            onesrow = c16[0:1, 321:449]     # (1,128) ones
            tri2 = c16[:, 449:705]          # [tri | tri] fp16

            cf = cst.tile([128, 1155], F32, tag="cf", name="cf")
            nc.sync.dma_start(cf[:], d_cf.ap())
            CC2 = cf[:, 0:512]              # [C | C]
            SS2 = cf[:, 512:1024]           # [S | S]
            eps128 = cf[:, 1024:1025]
            eps1 = cf[0:1, 1024:1025]
            one_f = cf[0:1, 1025:1026]      # 1.0 (transpose identity)
            mln15 = cf[0:1, 1026:1027]      # -ln(15)
            orowf = cf[0:1, 1027:1155]      # (1,128) ones f32

            wap = cst.tile([128, L], F32, tag="wap", name="wap")
            nc.sync.dma_start(wap[:], d_wap.ap())
            waw = cst.tile([128, n_ls], F32, tag="waw", name="waw")
            nc.sync.dma_start(waw[:], d_waw.ap())
            wmw = cst.tile([128, n_ls], F32, tag="wmw", name="wmw")
            nc.sync.dma_start(wmw[:], d_wmw.ap())
            rw = cst.tile([128, KT], F16, tag="rw", name="rw")
            nc.sync.dma_start(rw[:], d_rw.ap())
            rbias2 = cst.tile([1, 1], F32, tag="rbias2", name="rbias2")
            nc.sync.dma_start(rbias2[:], d_rb.ap())

            adw, qkw, qpw, vww, fcw = [], [], [], [], []
            for l in range(L):
                a_t = cst.tile([128, 512], F16, tag=f"adw{l}", name=f"adw{l}")
                nc.sync.dma_start(a_t[:], d_adw.ap()[l])
                adw.append(a_t)
                q_t = cst.tile([128, 256], F16, tag=f"qkw{l}", name=f"qkw{l}")
                nc.sync.dma_start(q_t[:], d_qkw.ap()[l])
                qkw.append(q_t)
                p_t = cst.tile([128, 256], F16, tag=f"qpw{l}", name=f"qpw{l}")
                nc.sync.dma_start(p_t[:], d_qpw.ap()[l])
                qpw.append(p_t)
                v_t = cst.tile([128, 128], F16, tag=f"vww{l}", name=f"vww{l}")
                nc.sync.dma_start(v_t[:], d_vww.ap()[l])
                vww.append(v_t)
                f_t = cst.tile([128, 512], F16, tag=f"fcw{l}", name=f"fcw{l}")
                nc.sync.dma_start(f_t[:], d_fcw.ap()[l])
                fcw.append(f_t)

            # lm_head weights: full shard resident in SBUF, chunked DMA so the
            # prefetch never head-of-line blocks the per-step bounce DMAs.
            lmsb = []
            LCH = 1600
            for k in range(KT):
                t_ = cst.tile([128, VQ], F16, tag=f"lm{k}", name=f"lm{k}")
                lmsb.append(t_)
                for c0 in range(0, VQ, LCH):
                    nc.sync.dma_start(
                        t_[:, c0:c0 + LCH],
                        d_lm.ap()[k * 128:(k + 1) * 128, c0:c0 + LCH])

            # ---------------- state ----------------
            xT = [st.tile([128, T], F32, tag=f"xT{k}", name=f"xT{k}") for k in range(KT)]
            xr = [st.tile([128, T], F16, tag=f"xr{k}", name=f"xr{k}") for k in range(KT)]
            acc = st.tile([128, T], F32, tag="acc", name="acc")
            xg = st.tile([128, KT * T], F32, tag="xg", name="xg")
            pcont = st.tile([1, T], F32, tag="pcont", name="pcont")
            nc.vector.memset(pcont[:], 1.0)
            nc.gpsimd.memset(acc[:], 0.0)

            # initial x (rms applied host-side)
            for k in range(KT):
                nc.sync.dma_start(xT[k][:], d_x0t.ap()[k * 128:(k + 1) * 128, :])
                with nc.allow_low_precision(reason="fp16 compute"):
                    nc.vector.tensor_copy(xr[k][:], xT[k][:])

            # CC warm-up: dummy AllGather so the first real one is cheap
            NO_CC = bool(int(os.environ.get("BASS_V2_NO_CC", "0")))
            db_in = dram.tile([128, 8], F32, tag="dbi", name="dbi")
            db_out = dram.tile([512, 8], F32, tag="dbo", name="dbo")
            nc.sync.dma_start(db_in[:], cf[:, 0:8])
            if not NO_CC:
                nc.gpsimd.collective_compute(
                    "AllGather", mybir.AluOpType.bypass, replica_groups=groups,
                    ins=[db_in[:].opt()], outs=[db_out[:].opt()])

            ls_idx = 0
            with nc.allow_low_precision(reason="fp16 compute"):
                for t, layers in enumerate(active_sets):
                    for l in layers:
                        # ---- xi = adapters' @ x (identity folded in) ----
                        H1 = ps.tile([128, 2 * T], F32, tag="H1", bufs=1, name="ps")
                        p_xi = H1[:, 0:T]
                        p_s1 = H1[:, T:2 * T]
                        for k in range(KT):
                            nc.tensor.matmul(
                                p_xi[:], adw[l][:, k * 128:(k + 1) * 128],
                                xr[k][:], start=(k == 0), stop=(k == KT - 1))
                        xi = wk16.tile([128, T], F16, tag="xi", name="xi")
                        nc.vector.tensor_copy(xi[:], p_xi[:])

                        # ---- v per s-block (both nodes at once; vww is
                        # host-zero-padded so full-128 contraction is exact)
                        p_v = ps.tile([128, T], F32, tag="S1", bufs=1, name="ps")
                        v_sb = [None, None]
                        for s in range(2):
                            nc.tensor.matmul(
                                p_v[:, s * 128:(s + 1) * 128],
                                xi[:, s * 128:(s + 1) * 128],
                                vww[l][:], start=True, stop=True)
                            vt = vsb.tile([128, 130], F16, tag="vt", name="vt")
                            if s == 0:
                                nc.scalar.copy(vt[:, 0:64], p_v[:, 0:64])
                                nc.scalar.copy(vt[:, 65:129], p_v[:, 64:128])
                            else:
                                nc.vector.tensor_copy(vt[:, 0:64], p_v[:, 128:192])
                                nc.vector.tensor_copy(vt[:, 65:129], p_v[:, 192:256])
                            nc.gpsimd.memset(vt[:, 64:65], 1.0)
                            nc.gpsimd.memset(vt[:, 129:130], 1.0)
                            v_sb[s] = vt

                        # ---- q/k (raw + pre-permuted), both nodes packed ----
                        p_qk = ps.tile([128, 2 * T], F32, tag="A", bufs=2, name="ps")
                        p_qp = ps.tile([128, 2 * T], F32, tag="A", bufs=2, name="ps")
                        for o in range(2):
                            nc.tensor.matmul(p_qk[:, o * T:(o + 1) * T],
                                             qkw[l][:, o * 128:(o + 1) * 128],
                                             xi[:], start=True, stop=True)
                            nc.tensor.matmul(p_qp[:, o * T:(o + 1) * T],
                                             qpw[l][:, o * 128:(o + 1) * 128],
                                             xi[:], start=True, stop=True)

                        # rms scale from pre-rope q/k (rope is norm-preserving)
                        sq = wk16.tile([128, 2 * T], F16, tag="sq", name="sq")
                        nc.scalar.activation(sq[:], p_qk[:], AF.Square)
                        p_ms = ps.tile([128, 2 * T], F32, tag="A", bufs=2, name="ps")
                        nc.tensor.matmul(p_ms[:], oblk, sq[:], start=True, stop=True)
                        lnm = wkf.tile([128, 2 * T], F32, tag="srt", name="lnm")
                        nc.scalar.activation(lnm[:], p_ms[:], AF.Ln, bias=eps128)
                        rsq = wk16.tile([128, 2 * T], F16, tag="rsq", name="rsq")
                        nc.scalar.activation(rsq[:], lnm[:], AF.Exp, scale=-0.5)

                        # rope: rot = qk*C + qp*S, then normalize + split q/k
                        t1 = wk16.tile([128, 2 * T], F16, bufs=2, tag="t1", name="t1")
                        nc.vector.tensor_tensor(t1[:], p_qk[:], CC2, AluOpType.mult)
                        t2 = wk16.tile([128, 2 * T], F16, bufs=2, tag="t2", name="t2")
                        nc.vector.tensor_tensor(t2[:], p_qp[:], SS2, AluOpType.mult)
                        rop = wk16.tile([128, 2 * T], F16, bufs=2, tag="rop", name="rop")
                        nc.vector.tensor_tensor(rop[:], t1[:], t2[:], AluOpType.add)
                        qt = wk16.tile([128, T], F16, tag="qt", name="qt")
                        kt = wk16.tile([128, 2 * T], F16, tag="kt", name="kt")
                        nc.gpsimd.memset(kt[64:128, 0:T], 0.0)
                        nc.gpsimd.memset(kt[0:64, T:2 * T], 0.0)
                        for o in range(2):
                            orows = slice(64 * o, 64 * o + 64)
                            nc.vector.tensor_tensor(
                                qt[orows, :], rop[0:64, o * T:(o + 1) * T],
                                rsq[0:64, o * T:(o + 1) * T], AluOpType.mult)
                            nc.vector.tensor_tensor(
                                kt[orows, o * T:(o + 1) * T],
                                rop[64:128, o * T:(o + 1) * T],
                                rsq[64:128, o * T:(o + 1) * T], AluOpType.mult)

                        # ---- scores -> masked exp ----
                        p_s0 = ps.tile([128, 2 * T], F32, tag="A", bufs=2, name="ps")
                        for o in range(2):
                            nc.tensor.matmul(p_s0[:, o * T:(o + 1) * T],
                                             kt[:, o * T:o * T + 128], qt[:],
                                             start=True, stop=True)
                            nc.tensor.matmul(p_s1[:, o * 128:(o + 1) * 128],
                                             kt[:, o * T + 128:(o + 1) * T],
                                             qt[:, 128:256],
                                             start=True, stop=True)
                        em0 = wk16.tile([128, 2 * T], F16, tag="em0", name="em0")
                        nc.scalar.activation(em0[:], p_s0[:], AF.Exp, scale=0.125)
                        em1 = wk16.tile([128, T], F16, tag="em1", name="em1")
                        nc.scalar.activation(em1[:], p_s1[:], AF.Exp, scale=0.125)
                        # masked diagonal blocks (separate tiles, no in-place)
                        m0 = wk16.tile([128, T], F16, tag="m0", name="m0")
                        nc.gpsimd.tensor_tensor(m0[:, 0:128], em0[:, 0:128],
                                                tri2[:, 0:128], AluOpType.mult)
                        nc.gpsimd.tensor_tensor(m0[:, 128:256], em0[:, T:T + 128],
                                                tri2[:, 0:128], AluOpType.mult)
                        m1 = wk16.tile([128, T], F16, tag="m1", name="m1")
                        nc.gpsimd.tensor_tensor(m1[:], em1[:], tri2, AluOpType.mult)

                        # ---- att (+colsum via ones col) ----
                        S2 = ps.tile([128, 2 * T], F32, tag="S2", bufs=1, name="ps")
                        p_att = [S2[0:65, 0:T], S2[0:65, T:2 * T]]
                        for o in range(2):
                            pa = p_att[o]
                            nc.tensor.matmul(pa[:, 0:128],
                                             v_sb[0][:, o * 65:(o + 1) * 65],
                                             m0[:, o * 128:(o + 1) * 128],
                                             start=True, stop=True)
                            nc.tensor.matmul(pa[:, 128:256],
                                             v_sb[0][:, o * 65:(o + 1) * 65],
                                             em0[:, o * T + 128:(o + 1) * T],
                                             start=True, stop=False)
                            nc.tensor.matmul(pa[:, 128:256],
                                             v_sb[1][:, o * 65:(o + 1) * 65],
                                             m1[:, o * 128:(o + 1) * 128],
                                             start=False, stop=True)
                        rcl = wkf.tile([1, 2 * T], F32, bufs=1, tag="rcl", name="rcl")
                        nc.scalar.activation(rcl[:], S2[64:65, 0:2 * T], AF.Ln)
                        rc2 = wkf.tile([1, 2 * T], F32, bufs=1, tag="rc2", name="rc2")
                        nc.scalar.activation(rc2[:], rcl[:], AF.Exp, scale=-1.0)
                        H2 = ps.tile([128, 2 * T], F32, tag="H2", bufs=1, name="ps")
                        nc.tensor.matmul(H2[:], orowf, rc2[:], start=True, stop=True)
                        att_sb = wk16.tile([128, T], F16, tag="att", name="att")
                        nc.scalar.copy(att_sb[0:64, :], p_att[0][0:64, :])
                        nc.scalar.copy(att_sb[64:128, :], p_att[1][0:64, :])
                        tt = wk16.tile([128, T], F16, tag="tt", name="tt")
                        nc.vector.tensor_tensor(tt[0:64, :], att_sb[0:64, :],
                                                H2[0:64, 0:T], AluOpType.mult)
                        nc.vector.tensor_tensor(tt[64:128, :], att_sb[64:128, :],
                                                H2[64:128, T:2 * T], AluOpType.mult)

                        xim = wk16.tile([128, T], F16, tag="xim", name="xim")
                        nc.vector.scalar_tensor_tensor(
                            xim[:], tt[:], wap[:, l:l + 1], xi[:],
                            AluOpType.mult, AluOpType.add)
                        nc.vector.scalar_tensor_tensor(
                            acc[:], tt[:], waw[:, ls_idx:ls_idx + 1], acc[:],
                            AluOpType.mult, AluOpType.add)

                        # ---- mlp (rms folded into 1/(mean+eps) post-scale) ----
                        sqm = wk16.tile([128, T], F16, tag="sqm", name="sqm")
                        nc.gpsimd.tensor_tensor(sqm[:], xim[:], xim[:],
                                                AluOpType.mult)
                        p_mq = ps.tile([128, T], F32, tag="H3", bufs=1, name="ps")
                        nc.tensor.matmul(p_mq[:], oblk, sqm[:], start=True, stop=True)
                        lnm2 = wkf.tile([128, T], F32, bufs=1, tag="pre", name="lnm2")
                        nc.scalar.activation(lnm2[:], p_mq[:], AF.Ln, bias=eps128)
                        rec2 = wk16.tile([128, T], F16, tag="rec2", name="rec2")
                        nc.scalar.activation(rec2[:], lnm2[:], AF.Exp, scale=-1.0)

                        p_sr01 = ps.tile([64, 2 * T], F32, tag="H3", bufs=1, name="ps")
                        p_srs = [p_sr01[:, 0:T], p_sr01[:, T:2 * T]]
                        for o in range(2):
                            p_fc = ps.tile([128, 2 * T], F32, tag="B", bufs=1, name="ps")
                            for h in range(2):
                                nc.tensor.matmul(
                                    p_fc[:, h * T:(h + 1) * T],
                                    fcw[l][:, o * 256 + h * 128:o * 256 + (h + 1) * 128],
                                    xim[:], start=True, stop=True)
                            frel = wk16.tile([128, 2 * T], F16, tag="frel", name="frel")
                            nc.scalar.activation(frel[:], p_fc[:], AF.Relu)
                            rsq2 = wk16.tile([128, 2 * T], F16, tag="rsq2", name="rsq2")
                            nc.gpsimd.tensor_tensor(rsq2[:], frel[:], frel[:],
                                                    AluOpType.mult)
                            nc.tensor.matmul(p_srs[o][:], ocol, rsq2[:, 0:T],
                                             start=True, stop=False)
                            nc.tensor.matmul(p_srs[o][:], ocol, rsq2[:, T:2 * T],
                                             start=False, stop=True)
                        hm = wk16.tile([128, T], F16, tag="hm", name="hm")
                        nc.vector.tensor_tensor(hm[0:64, :], p_srs[0][:],
                                                rec2[0:64, :], AluOpType.mult)
                        nc.vector.tensor_tensor(hm[64:128, :], p_srs[1][:],
                                                rec2[64:128, :], AluOpType.mult)
                        nc.vector.scalar_tensor_tensor(
                            acc[:], hm[:], wmw[:, ls_idx:ls_idx + 1], acc[:],
                            AluOpType.mult, AluOpType.add)
                        ls_idx += 1

                    # ---- step sync: scale acc by pcont, AllGather, update x ----
                    p_pc = ps.tile([128, T], F32, tag="H1", bufs=1, name="ps")
                    nc.tensor.matmul(p_pc[:], orowf, pcont[:], start=True, stop=True)
                    acc2 = wkf.tile([128, T], F32, bufs=1, tag="acc2", name="acc2")
                    nc.vector.tensor_tensor(acc2[:], acc[:], p_pc[:], AluOpType.mult)
                    nc.gpsimd.memset(acc[:], 0.0)
                    b_in = dram.tile([128, T], F32, tag="bin", name=f"bin{t}")
                    b_out = dram.tile([KT * 128, T], F32, tag="bout", name=f"bout{t}")
                    nc.sync.dma_start(b_in[:], acc2[:])
                    if not NO_CC:
                        nc.gpsimd.collective_compute(
                            "AllGather", mybir.AluOpType.bypass, replica_groups=groups,
                            ins=[b_in[:].opt()], outs=[b_out[:].opt()])
                        for k in range(KT):
                            nc.sync.dma_start(xg[:, k * T:(k + 1) * T],
                                              b_out[k * 128:(k + 1) * 128, :])
                    else:
                        for k in range(KT):
                            nc.sync.dma_start(xg[:, k * T:(k + 1) * T], b_in[:])
                    for k in range(KT):
                        nc.vector.tensor_tensor(xT[k][:], xT[k][:],
                                                xg[:, k * T:(k + 1) * T],
                                                AluOpType.add)
                        nc.vector.tensor_copy(xr[k][:], xT[k][:])

                    # ---- router: pcont *= 1 - sigmoid(x@rw + rb) ----
                    p_ph = ps.tile([1, T], F32, tag="H3", bufs=1, name="ps")
                    for k in range(KT):
                        nc.tensor.matmul(p_ph[:], rw[:, k:k + 1], xr[k][:],
                                         start=(k == 0), stop=(k == KT - 1))
                    ez = wkf.tile([1, T], F32, bufs=1, tag="th", name="ez")
                    nc.scalar.activation(ez[:], p_ph[:], AF.Exp, bias=rbias2[:])
                    ez1 = wkf.tile([1, T], F32, bufs=1, tag="omp", name="ez1")
                    nc.vector.tensor_scalar(ez1[:], ez[:], 1.0, 1.0,
                                            AluOpType.mult, AluOpType.add)
                    lz = wkf.tile([1, T], F32, bufs=1, tag="lz", name="lz")
                    nc.scalar.activation(lz[:], ez1[:], AF.Ln)
                    omp = wkf.tile([1, T], F32, bufs=1, tag="omp2", name="omp")
                    nc.scalar.activation(omp[:], lz[:], AF.Exp, scale=-1.0)
                    nc.vector.tensor_tensor(pcont[:], pcont[:], omp[:],
                                            AluOpType.mult)

                # ---------------- final rms + lm_head ----------------
                p_mr = ps.tile([1, T], F32, tag="H3", bufs=1, name="ps")
                for k in range(KT):
                    sqf = wk16.tile([128, T], F16, tag="sqf", name="sqf")
                    nc.scalar.activation(sqf[:], xr[k][:], AF.Square)
                    nc.tensor.matmul(p_mr[:], oc1, sqf[:],
                                     start=(k == 0), stop=(k == KT - 1))
                lnf = wkf.tile([1, T], F32, bufs=1, tag="rr", name="lnf")
                nc.scalar.activation(lnf[:], p_mr[:], AF.Ln, bias=eps1,
                                     scale=1.0 / E)
                rr15 = wkf.tile([1, T], F32, bufs=1, tag="rr15", name="rr15")
                nc.scalar.activation(rr15[:], lnf[:], AF.Exp, scale=-0.5,
                                     bias=mln15)
                rcol = []
                for i in range(NTT):
                    p_tr = ps.tile([128, 1], F32, tag="S1", bufs=1, name="ptr")
                    nc.tensor.transpose(p_tr[:], rr15[:, i * 128:(i + 1) * 128],
                                        one_f)
                    rc = st.tile([128, 1], F32, tag=f"rcol{i}", name=f"rcol{i}")
                    nc.scalar.copy(rc[:], p_tr[:])
                    rcol.append(rc)

                for i in range(NTT):
                    for v in range(NVT):
                        p_lg = ps.tile([128, 512], F32, tag="A", bufs=2, name="ps")
                        for k in range(KT):
                            nc.tensor.matmul(
                                p_lg[:], xr[k][:, i * 128:(i + 1) * 128],
                                lmsb[k][:, v * 512:(v + 1) * 512],
                                start=(k == 0), stop=(k == KT - 1))
                        lth = wk16.tile([128, 512], F16, tag="lth", name="lth")
                        nc.scalar.activation(lth[:], p_lg[:], AF.Tanh,
                                             scale=rcol[i][:])
                        nc.sync.dma_start(
                            d_out.ap()[i * 128:(i + 1) * 128,
                                       v * 512:(v + 1) * 512],
                            lth[:])

    nc.compile()
    return nc


def _rms_np(x):
    return x * (1.0 / np.sqrt(np.mean(x * x, axis=-1, keepdims=True) + EPS))


def _host_prep(idx, n_steps, wte, adapters, qkv_w, attn_proj, mlp_fc, mlp_proj,
               dep, router_w, router_b, lm_head_w):
    idx = np.asarray(idx)
    wte = np.asarray(wte, np.float32)
    adapters = np.asarray(adapters, np.float32)
    qkv_w = np.asarray(qkv_w, np.float32)
    attn_proj = np.asarray(attn_proj, np.float32)
    mlp_fc = np.asarray(mlp_fc, np.float32)
    mlp_proj = np.asarray(mlp_proj, np.float32)
    dep = np.asarray(dep, np.float32)
    router_w = np.asarray(router_w, np.float32).reshape(E, 1)
    router_b = np.asarray(router_b, np.float32).reshape(-1)
    lm_head_w = np.asarray(lm_head_w, np.float32)
    ns = int(n_steps)

    dp = np.maximum(dep, 0.0)
    depths = np.zeros((N,), np.float32)
    for _ in range(L):
        depths = (dp @ (depths + 1.0)).astype(np.float32)

    w_eff = np.zeros((ns, N), np.float32)
    active_sets = []
    for t in range(ns):
        td = t * (L / ns)
        w_all = np.exp(-np.abs(depths - np.float32(td))).astype(np.float32)
        w = np.where(w_all > 0.15, w_all, 0.0).astype(np.float32)
        w_eff[t] = w
        active_sets.append(tuple(sorted({n // G for n in range(N) if w[n] > 0})))
    active_sets = tuple(active_sets)
    n_ls = max(sum(len(a) for a in active_sets), 1)

    # fold the group-slice identity into the adapters
    adapters_f = adapters.copy()
    for n in range(N):
        g = n % G
        adapters_f[n, :, g * GD:(g + 1) * GD] += np.eye(GD, dtype=np.float32)

    # rope permutation of the q/k OUTPUT index: out j <- out (j+32)%64 within
    # each 64-block (q block and k block separately)
    perm64 = (np.arange(GD) + HD) % GD
    perm128 = np.concatenate([perm64, GD + perm64])

    w_ap = attn_proj.sum(axis=2)
    w_mp = mlp_proj.sum(axis=2)

    # per-pair weight payloads
    payload = []
    for p in range(VSH):
        adw = np.zeros((L, 128, 512), np.float16)
        qkwA = np.zeros((L, 128, 256), np.float16)
        qpwA = np.zeros((L, 128, 256), np.float16)
        vwwA = np.zeros((L, 128, 128), np.float16)
        fcwA = np.zeros((L, 128, 512), np.float16)
        wapP = np.zeros((128, L), np.float32)
        wawP = np.zeros((128, n_ls), np.float32)
        wmwP = np.zeros((128, n_ls), np.float32)
        for l in range(L):
            for o in range(2):
                n = l * G + 2 * p + o
                rows = slice(o * 64, (o + 1) * 64)
                for k in range(KT):
                    adw[l, :, k * 128 + o * 64: k * 128 + (o + 1) * 64] = \
                        adapters_f[n, :, k * 128:(k + 1) * 128].T
                # zero-padded full-128-contraction stationaries (node o's
                # weights live on its own 64 rows; the rest stay zero)
                qkwA[l, rows, o * 128:(o + 1) * 128] = qkv_w[n, 0:128, :].T
                qpwA[l, rows, o * 128:(o + 1) * 128] = qkv_w[n, 0:128, :].T[:, perm128]
                vwwA[l, rows, o * 64:(o + 1) * 64] = qkv_w[n, 128:192, :].T
                fcwA[l, rows, o * 256:(o + 1) * 256] = mlp_fc[n].T
                wapP[o * 64:(o + 1) * 64, l] = w_ap[n]
        ls = 0
        for tt, layers in enumerate(active_sets):
            for l in layers:
                for o in range(2):
                    n = l * G + 2 * p + o
                    wawP[o * 64:(o + 1) * 64, ls] = w_ap[n] * w_eff[tt, n]
                    wmwP[o * 64:(o + 1) * 64, ls] = w_mp[n] * w_eff[tt, n]
                ls += 1
        payload.append((adw, qkwA, qpwA, vwwA, fcwA, wapP, wawP, wmwP))

    # constants
    c16 = np.zeros((128, 705), np.float16)
    ob = np.zeros((128, 128), np.float32)
    ob[0:64, 0:64] = 1.0 / GD
    ob[64:128, 64:128] = 1.0 / GD
    c16[:, 0:128] = ob.astype(np.float16)
    c16[:, 128:192] = 1.0
    c16[:, 192:193] = 1.0
    c16[0, 193:257] = 1.0
    c16[1, 257:321] = 1.0
    c16[0, 321:449] = 1.0
    s_i = np.arange(128)[:, None]
    t_i = np.arange(128)[None, :]
    tri = (s_i <= t_i).astype(np.float16)
    c16[:, 449:577] = tri
    c16[:, 577:705] = tri

    inv_freq = 1.0 / (10000.0 ** (np.arange(0, GD, 2, dtype=np.float64) / GD))
    freqs = np.outer(np.arange(T), inv_freq)
    cosT = np.cos(freqs).astype(np.float32).T
    sinT = np.sin(freqs).astype(np.float32).T
    cstf = np.zeros((128, 1155), np.float32)
    for blk in range(4):
        cstf[blk * 32:(blk + 1) * 32, 0:256] = cosT
        cstf[blk * 32:(blk + 1) * 32, 256:512] = cosT
        cstf[blk * 32:(blk + 1) * 32, 512:768] = sinT * (1.0 if blk % 2 == 0 else -1.0)
        cstf[blk * 32:(blk + 1) * 32, 768:1024] = sinT * (1.0 if blk % 2 == 0 else -1.0)
    cstf[:, 1024] = EPS
    cstf[0, 1025] = 1.0
    cstf[0, 1026] = -np.log(15.0)
    cstf[0, 1027:1155] = 1.0

    rwP = np.zeros((128, KT), np.float16)
    for k in range(KT):
        rwP[:, k] = router_w[k * 128:(k + 1) * 128, 0].astype(np.float16)
    rbias2 = np.full((1, 1), np.float32(router_b[0]), np.float32)

    x0 = _rms_np(wte[idx])  # (B, T, E) f32

    in_maps = []
    for c in range(NC):
        b, p = c // VSH, c % VSH
        lo = p * VW
        hi = min(lo + VW, V)
        lmt = np.zeros((E, VQ), np.float16)
        lmt[:, 0:hi - lo] = lm_head_w[lo:hi, :].T.astype(np.float16)
        adw, qkwA, qpwA, vwwA, fcwA, wapP, wawP, wmwP = payload[p]
        in_maps.append({
            "x0t": np.ascontiguousarray(x0[b].T), "adw": adw, "qkw": qkwA,
            "qpw": qpwA, "vww": vwwA, "fcw": fcwA, "c16": c16, "cstf": cstf,
            "wapP": wapP, "wawP": wawP, "wmwP": wmwP, "rwP": rwP,
            "rbias2": rbias2, "lmt": lmt,
        })
    return active_sets, in_maps


def kernel(idx, n_steps, wte, adapters, qkv_w, attn_proj, mlp_fc, mlp_proj,
           dep, router_w, router_b, lm_head_w):
    active_sets, in_maps = _host_prep(
        idx, n_steps, wte, adapters, qkv_w, attn_proj, mlp_fc, mlp_proj,
        dep, router_w, router_b, lm_head_w)

    if active_sets not in _PROGRAM_CACHE:
        _PROGRAM_CACHE[active_sets] = _build_program(active_sets)
    nc = _PROGRAM_CACHE[active_sets]

    trace = bool(int(os.environ.get("BASS_KERNEL_TRACE", "0")))
    res = run_bass_kernel_spmd(nc, in_maps, list(range(NC)), trace=trace)
    if trace and res.exec_time_ns is not None:
        print(f"HW exec time: {res.exec_time_ns} ns")

    out = np.zeros((B, T, V), np.float32)
    for c in range(NC):
        b, p = c // VSH, c % VSH
        lo = p * VW
        hi = min(lo + VW, V)
        out[b, :, lo:hi] = 15.0 * res.results[c]["out_lg"][:, 0:hi - lo].astype(np.float32)
    return out


# revision 23
# speedup vs baseline: 3.8537x; 1.1376x over previous
"""Trainium2 Bass kernel for nn_BG_ALRT_62921270886438 (moe_routing).

Sharding v2: core c -> (batch b = c // 4, pair p = c % 4).  Each core computes
only its pair's two nodes per active layer; the group-wise scatter-add target
of pair p is exactly E-rows [128p, 128p+128), so the per-step x update needs
only an AllGather (groups {0-3}, {4-7}) of each core's [128, T] acc slice.
lm_head is vocab-sharded 4 ways within each batch group (same output contract
as v1).  Matmuls run in fp16 (1 cycle/row vs 4 for fp32), x state stays fp32.

Self-contained: only numpy + the concourse toolchain on sys.path.
"""
import os

import numpy as np

import concourse.bacc as bacc
import concourse.tile as tile
from concourse import mybir
from concourse.alu_op_type import AluOpType
from concourse.bass_utils import run_bass_kernel_spmd

AF = mybir.ActivationFunctionType
F32 = mybir.dt.float32
F16 = mybir.dt.float16

B, T, E, G, GD, L, N, V = 2, 256, 512, 8, 64, 8, 64, 50257
HD = GD // 2          # 32, rope half
NC = 8                # cores
VSH = 4               # vocab shards per batch group
VW = (V + VSH - 1) // VSH          # 12565 raw shard width
VQ = ((VW + 511) // 512) * 512     # 12800 padded shard width
EPS = float(np.finfo(np.float32).eps)
KT = E // 128         # 4 contraction tiles over E
NVT = VQ // 512       # 25 vocab tiles of 512
NTT = T // 128        # 2 token tiles

_PROGRAM_CACHE = {}


def _tune_act_tables(arch):
    """Steer the act-table-load pass to one set for the whole step loop.

    All step-loop activations (square/ln/exp/relu/copy/identity) exist in
    `natural_log_exp_and_others`; tanh (lm head only) in `exp_and_others`.
    The pass picks the first listed set containing the function, which makes
    Ln and Exp resolve to different sets and thrash the single active table
    (~1.3us per reload).  get_activation_tables is functools.cache'd and
    returns the same dict object, so removing the overlapping functions from
    every other set (pure removals - set ids stay aligned with act_info.json)
    makes the combined set the unique choice.
    """
    from concourse.hw_specs import get_activation_tables
    tabs = get_activation_tables(arch)
    combined = tabs.get("natural_log_exp_and_others")
    if not combined:
        return
    for name, fns in tabs.items():
        if name != "natural_log_exp_and_others":
            fns.difference_update(combined)


def _build_program(active_sets):
    """active_sets: tuple of tuples - active layer list per step."""
    nc = bacc.Bacc("TRN2", target_bir_lowering=False, debug=False, num_devices=NC)
    _tune_act_tables(nc.m.arch)
    n_ls = max(sum(len(a) for a in active_sets), 1)
    groups = [[0, 1, 2, 3], [4, 5, 6, 7]]

    d_x0t = nc.dram_tensor("x0t", [E, T], F32, kind="ExternalInput")
    d_adw = nc.dram_tensor("adw", [L, 128, 512], F16, kind="ExternalInput")
    d_qkw = nc.dram_tensor("qkw", [L, 128, 256], F16, kind="ExternalInput")
    d_qpw = nc.dram_tensor("qpw", [L, 128, 256], F16, kind="ExternalInput")
    d_vww = nc.dram_tensor("vww", [L, 128, 128], F16, kind="ExternalInput")
    d_fcw = nc.dram_tensor("fcw", [L, 128, 512], F16, kind="ExternalInput")
    d_c16 = nc.dram_tensor("c16", [128, 705], F16, kind="ExternalInput")
    d_cf = nc.dram_tensor("cstf", [128, 1155], F32, kind="ExternalInput")
    d_wap = nc.dram_tensor("wapP", [128, L], F32, kind="ExternalInput")
    d_waw = nc.dram_tensor("wawP", [128, n_ls], F32, kind="ExternalInput")
    d_wmw = nc.dram_tensor("wmwP", [128, n_ls], F32, kind="ExternalInput")
    d_rw = nc.dram_tensor("rwP", [128, KT], F16, kind="ExternalInput")
    d_rb = nc.dram_tensor("rbias2", [1, 1], F32, kind="ExternalInput")
    d_lm = nc.dram_tensor("lmt", [E, VQ], F16, kind="ExternalInput")
    d_out = nc.dram_tensor("out_lg", [T, VQ], F16, kind="ExternalOutput")

    with tile.TileContext(nc) as tc:
        with tc.tile_pool(name="cst", bufs=1) as cst, \
             tc.tile_pool(name="st", bufs=1) as st, \
             tc.tile_pool(name="wk16", bufs=3) as wk16, \
             tc.tile_pool(name="wkf", bufs=2) as wkf, \
             tc.tile_pool(name="vsb", bufs=4) as vsb, \
             tc.tile_pool(name="ps", bufs=1, space="PSUM") as ps, \
             tc.tile_pool(name="dram", bufs=20, space="DRAM") as dram:

            # ---------------- constants ----------------
            c16 = cst.tile([128, 705], F16, tag="c16", name="c16")
            nc.sync.dma_start(c16[:], d_c16.ap())
            oblk = c16[:, 0:128]            # block-diag(64) of 1/64
            ocol = c16[:, 128:192]          # (128,64) ones
            oc1 = c16[:, 192:193]           # (128,1) ones
            sel2 = c16[0:2, 193:321]        # row0 -> parts 0:64, row1 -> 64:128
            onesrow = c16[0:1, 321:449]     # (1,128) ones
            tri2 = c16[:, 449:705]          # [tri | tri] fp16

            cf = cst.tile([128, 1155], F32, tag="cf", name="cf")
            nc.sync.dma_start(cf[:], d_cf.ap())
            CC2 = cf[:, 0:512]              # [C | C]
            SS2 = cf[:, 512:1024]           # [S | S]
            eps128 = cf[:, 1024:1025]
            eps1 = cf[0:1, 1024:1025]
            one_f = cf[0:1, 1025:1026]      # 1.0 (transpose identity)
            mln15 = cf[0:1, 1026:1027]      # -ln(15)
            orowf = cf[0:1, 1027:1155]      # (1,128) ones f32

            wap = cst.tile([128, L], F32, tag="wap", name="wap")
            nc.sync.dma_start(wap[:], d_wap.ap())
            waw = cst.tile([128, n_ls], F32, tag="waw", name="waw")
            nc.sync.dma_start(waw[:], d_waw.ap())
            wmw = cst.tile([128, n_ls], F32, tag="wmw", name="wmw")
            nc.sync.dma_start(wmw[:], d_wmw.ap())
            rw = cst.tile([128, KT], F16, tag="rw", name="rw")
            nc.sync.dma_start(rw[:], d_rw.ap())
            rbias2 = cst.tile([1, 1], F32, tag="rbias2", name="rbias2")
            nc.sync.dma_start(rbias2[:], d_rb.ap())

            adw, qkw, qpw, vww, fcw = [], [], [], [], []
            for l in range(L):
                a_t = cst.tile([128, 512], F16, tag=f"adw{l}", name=f"adw{l}")
                nc.sync.dma_start(a_t[:], d_adw.ap()[l])
                adw.append(a_t)
                q_t = cst.tile([128, 256], F16, tag=f"qkw{l}", name=f"qkw{l}")
                nc.sync.dma_start(q_t[:], d_qkw.ap()[l])
                qkw.append(q_t)
                p_t = cst.tile([128, 256], F16, tag=f"qpw{l}", name=f"qpw{l}")
                nc.sync.dma_start(p_t[:], d_qpw.ap()[l])
                qpw.append(p_t)
                v_t = cst.tile([128, 128], F16, tag=f"vww{l}", name=f"vww{l}")
                nc.sync.dma_start(v_t[:], d_vww.ap()[l])
                vww.append(v_t)
                f_t = cst.tile([128, 512], F16, tag=f"fcw{l}", name=f"fcw{l}")
                nc.sync.dma_start(f_t[:], d_fcw.ap()[l])
                fcw.append(f_t)

            # lm_head weights: full shard resident in SBUF, chunked DMA so the
            # prefetch never head-of-line blocks the per-step bounce DMAs.
            lmsb = []
            LCH = 1600
            for k in range(KT):
                t_ = cst.tile([128, VQ], F16, tag=f"lm{k}", name=f"lm{k}")
                lmsb.append(t_)
                for c0 in range(0, VQ, LCH):
                    nc.sync.dma_start(
                        t_[:, c0:c0 + LCH],
                        d_lm.ap()[k * 128:(k + 1) * 128, c0:c0 + LCH])

            # ---------------- state ----------------
            xT = [st.tile([128, T], F32, tag=f"xT{k}", name=f"xT{k}") for k in range(KT)]
            xr = [st.tile([128, T], F16, tag=f"xr{k}", name=f"xr{k}") for k in range(KT)]
            acc = st.tile([128, T], F32, tag="acc", name="acc")
            xg = st.tile([128, KT * T], F32, tag="xg", name="xg")
            pcont = st.tile([1, T], F32, tag="pcont", name="pcont")
            nc.vector.memset(pcont[:], 1.0)
            nc.gpsimd.memset(acc[:], 0.0)

            # initial x (rms applied host-side)
            for k in range(KT):
                nc.sync.dma_start(xT[k][:], d_x0t.ap()[k * 128:(k + 1) * 128, :])
                with nc.allow_low_precision(reason="fp16 compute"):
                    nc.vector.tensor_copy(xr[k][:], xT[k][:])

            # CC warm-up: dummy AllGather so the first real one is cheap
            NO_CC = bool(int(os.environ.get("BASS_V2_NO_CC", "0")))
            db_in = dram.tile([128, 8], F32, tag="dbi", name="dbi")
            db_out = dram.tile([512, 8], F32, tag="dbo", name="dbo")
            nc.sync.dma_start(db_in[:], cf[:, 0:8])
            if not NO_CC:
                nc.gpsimd.collective_compute(
                    "AllGather", mybir.AluOpType.bypass, replica_groups=groups,
                    ins=[db_in[:].opt()], outs=[db_out[:].opt()])

            ls_idx = 0
            with nc.allow_low_precision(reason="fp16 compute"):
                def make_unit(l, ls_i):
                    """Four emission phases for one (layer, pair) unit; the
                    step loop staggers phases across the step's units so each
                    engine queue interleaves independent work."""
                    S = {}

                    def p1():
                        H1 = ps.tile([128, 2 * T], F32, tag="H1", bufs=1, name="ps")
                        S["p_s1"] = H1[:, T:2 * T]
                        p_xi = H1[:, 0:T]
                        for k in range(KT):
                            nc.tensor.matmul(
                                p_xi[:], adw[l][:, k * 128:(k + 1) * 128],
                                xr[k][:], start=(k == 0), stop=(k == KT - 1))
                        xi = wk16.tile([128, T], F16, tag="xi", name="xi")
                        nc.vector.tensor_copy(xi[:], p_xi[:])
                        S["xi"] = xi
                        p_v = ps.tile([128, T], F32, tag="S1", bufs=1, name="ps")
                        v_sb = [None, None]
                        for s in range(2):
                            nc.tensor.matmul(
                                p_v[:, s * 128:(s + 1) * 128],
                                xi[:, s * 128:(s + 1) * 128],
                                vww[l][:], start=True, stop=True)
                            vt = vsb.tile([128, 130], F16, tag="vt", name="vt")
                            if s == 0:
                                nc.scalar.copy(vt[:, 0:64], p_v[:, 0:64])
                                nc.scalar.copy(vt[:, 65:129], p_v[:, 64:128])
                            else:
                                nc.vector.tensor_copy(vt[:, 0:64], p_v[:, 128:192])
                                nc.vector.tensor_copy(vt[:, 65:129], p_v[:, 192:256])
                            nc.gpsimd.memset(vt[:, 64:65], 1.0)
                            nc.gpsimd.memset(vt[:, 129:130], 1.0)
                            v_sb[s] = vt
                        S["v_sb"] = v_sb
                        p_qk = ps.tile([128, 2 * T], F32, tag="A", bufs=2, name="ps")
                        p_qp = ps.tile([128, 2 * T], F32, tag="A", bufs=2, name="ps")
                        for o in range(2):
                            nc.tensor.matmul(p_qk[:, o * T:(o + 1) * T],
                                             qkw[l][:, o * 128:(o + 1) * 128],
                                             xi[:], start=True, stop=True)
                            nc.tensor.matmul(p_qp[:, o * T:(o + 1) * T],
                                             qpw[l][:, o * 128:(o + 1) * 128],
                                             xi[:], start=True, stop=True)
                        S["p_qk"], S["p_qp"] = p_qk, p_qp

                    def p2():
                        p_qk, p_qp = S["p_qk"], S["p_qp"]
                        sq = wk16.tile([128, 2 * T], F16, tag="sq", name="sq")
                        nc.scalar.activation(sq[:], p_qk[:], AF.Square)
                        p_ms = ps.tile([128, 2 * T], F32, tag="A", bufs=2, name="ps")
                        nc.tensor.matmul(p_ms[:], oblk, sq[:], start=True, stop=True)
                        lnm = wkf.tile([128, 2 * T], F32, tag="srt", name="lnm")
                        nc.scalar.activation(lnm[:], p_ms[:], AF.Ln, bias=eps128)
                        rsq = wk16.tile([128, 2 * T], F16, tag="rsq", name="rsq")
                        nc.scalar.activation(rsq[:], lnm[:], AF.Exp, scale=-0.5)
                        t1 = wk16.tile([128, 2 * T], F16, bufs=2, tag="t1", name="t1")
                        nc.vector.tensor_tensor(t1[:], p_qk[:], CC2, AluOpType.mult)
                        t2 = wk16.tile([128, 2 * T], F16, bufs=2, tag="t2", name="t2")
                        nc.vector.tensor_tensor(t2[:], p_qp[:], SS2, AluOpType.mult)
                        rop = wk16.tile([128, 2 * T], F16, bufs=2, tag="rop", name="rop")
                        nc.vector.tensor_tensor(rop[:], t1[:], t2[:], AluOpType.add)
                        qt = wk16.tile([128, T], F16, tag="qt", name="qt")
                        kt = wk16.tile([128, 2 * T], F16, tag="kt", name="kt")
                        nc.gpsimd.memset(kt[64:128, 0:T], 0.0)
                        nc.gpsimd.memset(kt[0:64, T:2 * T], 0.0)
                        for o in range(2):
                            orows = slice(64 * o, 64 * o + 64)
                            nc.vector.tensor_tensor(
                                qt[orows, :], rop[0:64, o * T:(o + 1) * T],
                                rsq[0:64, o * T:(o + 1) * T], AluOpType.mult)
                            nc.vector.tensor_tensor(
                                kt[orows, o * T:(o + 1) * T],
                                rop[64:128, o * T:(o + 1) * T],
                                rsq[64:128, o * T:(o + 1) * T], AluOpType.mult)
                        p_s0 = ps.tile([128, 2 * T], F32, tag="A", bufs=2, name="ps")
                        p_s1 = S["p_s1"]
                        for o in range(2):
                            nc.tensor.matmul(p_s0[:, o * T:(o + 1) * T],
                                             kt[:, o * T:o * T + 128], qt[:],
                                             start=True, stop=True)
                            nc.tensor.matmul(p_s1[:, o * 128:(o + 1) * 128],
                                             kt[:, o * T + 128:(o + 1) * T],
                                             qt[:, 128:256],
                                             start=True, stop=True)
                        em0 = wk16.tile([128, 2 * T], F16, tag="em0", name="em0")
                        nc.scalar.activation(em0[:], p_s0[:], AF.Exp, scale=0.125)
                        em1 = wk16.tile([128, T], F16, tag="em1", name="em1")
                        nc.scalar.activation(em1[:], p_s1[:], AF.Exp, scale=0.125)
                        m0 = wk16.tile([128, T], F16, tag="m0", name="m0")
                        nc.gpsimd.tensor_tensor(m0[:, 0:128], em0[:, 0:128],
                                                tri2[:, 0:128], AluOpType.mult)
                        nc.gpsimd.tensor_tensor(m0[:, 128:256], em0[:, T:T + 128],
                                                tri2[:, 0:128], AluOpType.mult)
                        m1 = wk16.tile([128, T], F16, tag="m1", name="m1")
                        nc.gpsimd.tensor_tensor(m1[:], em1[:], tri2, AluOpType.mult)
                        S["em0"], S["m0"], S["m1"] = em0, m0, m1

                    def p3():
                        em0, m0, m1 = S["em0"], S["m0"], S["m1"]
                        v_sb, xi = S["v_sb"], S["xi"]
                        S2 = ps.tile([128, 2 * T], F32, tag="S2", bufs=1, name="ps")
                        p_att = [S2[0:65, 0:T], S2[0:65, T:2 * T]]
                        for o in range(2):
                            pa = p_att[o]
                            nc.tensor.matmul(pa[:, 0:128],
                                             v_sb[0][:, o * 65:(o + 1) * 65],
                                             m0[:, o * 128:(o + 1) * 128],
                                             start=True, stop=True)
                            nc.tensor.matmul(pa[:, 128:256],
                                             v_sb[0][:, o * 65:(o + 1) * 65],
                                             em0[:, o * T + 128:(o + 1) * T],
                                             start=True, stop=False)
                            nc.tensor.matmul(pa[:, 128:256],
                                             v_sb[1][:, o * 65:(o + 1) * 65],
                                             m1[:, o * 128:(o + 1) * 128],
                                             start=False, stop=True)
                        rcl = wkf.tile([1, 2 * T], F32, bufs=1, tag="rcl", name="rcl")
                        nc.scalar.activation(rcl[:], S2[64:65, 0:2 * T], AF.Ln)
                        rc2 = wkf.tile([1, 2 * T], F32, bufs=1, tag="rc2", name="rc2")
                        nc.scalar.activation(rc2[:], rcl[:], AF.Exp, scale=-1.0)
                        H2 = ps.tile([128, 2 * T], F32, tag="H2", bufs=1, name="ps")
                        nc.tensor.matmul(H2[:], orowf, rc2[:], start=True, stop=True)
                        att_sb = wk16.tile([128, T], F16, tag="att", name="att")
                        nc.scalar.copy(att_sb[0:64, :], p_att[0][0:64, :])
                        nc.scalar.copy(att_sb[64:128, :], p_att[1][0:64, :])
                        tt = wk16.tile([128, T], F16, tag="tt", name="tt")
                        nc.vector.tensor_tensor(tt[0:64, :], att_sb[0:64, :],
                                                H2[0:64, 0:T], AluOpType.mult)
                        nc.vector.tensor_tensor(tt[64:128, :], att_sb[64:128, :],
                                                H2[64:128, T:2 * T], AluOpType.mult)
                        xim = wk16.tile([128, T], F16, tag="xim", name="xim")
                        nc.vector.scalar_tensor_tensor(
                            xim[:], tt[:], wap[:, l:l + 1], xi[:],
                            AluOpType.mult, AluOpType.add)
                        nc.vector.scalar_tensor_tensor(
                            acc[:], tt[:], waw[:, ls_i:ls_i + 1], acc[:],
                            AluOpType.mult, AluOpType.add)
                        S["xim"] = xim

                    def p4():
                        xim = S["xim"]
                        sqm = wk16.tile([128, T], F16, tag="sqm", name="sqm")
                        nc.gpsimd.tensor_tensor(sqm[:], xim[:], xim[:],
                                                AluOpType.mult)
                        p_mq = ps.tile([128, T], F32, tag="H3", bufs=1, name="ps")
                        nc.tensor.matmul(p_mq[:], oblk, sqm[:], start=True, stop=True)
                        lnm2 = wkf.tile([128, T], F32, bufs=1, tag="pre", name="lnm2")
                        nc.scalar.activation(lnm2[:], p_mq[:], AF.Ln, bias=eps128)
                        rec2 = wk16.tile([128, T], F16, tag="rec2", name="rec2")
                        nc.scalar.activation(rec2[:], lnm2[:], AF.Exp, scale=-1.0)
                        p_sr01 = ps.tile([64, 2 * T], F32, tag="H3", bufs=1, name="ps")
                        p_srs = [p_sr01[:, 0:T], p_sr01[:, T:2 * T]]
                        for o in range(2):
                            p_fc = ps.tile([128, 2 * T], F32, tag="B", bufs=1, name="ps")
                            for h in range(2):
                                nc.tensor.matmul(
                                    p_fc[:, h * T:(h + 1) * T],
                                    fcw[l][:, o * 256 + h * 128:o * 256 + (h + 1) * 128],
                                    xim[:], start=True, stop=True)
                            frel = wk16.tile([128, 2 * T], F16, tag="frel", name="frel")
                            nc.scalar.activation(frel[:], p_fc[:], AF.Relu)
                            rsq2 = wk16.tile([128, 2 * T], F16, tag="rsq2", name="rsq2")
                            nc.gpsimd.tensor_tensor(rsq2[:], frel[:], frel[:],
                                                    AluOpType.mult)
                            nc.tensor.matmul(p_srs[o][:], ocol, rsq2[:, 0:T],
                                             start=True, stop=False)
                            nc.tensor.matmul(p_srs[o][:], ocol, rsq2[:, T:2 * T],
                                             start=False, stop=True)
                        hm = wk16.tile([128, T], F16, tag="hm", name="hm")
                        nc.vector.tensor_tensor(hm[0:64, :], p_srs[0][:],
                                                rec2[0:64, :], AluOpType.mult)
                        nc.vector.tensor_tensor(hm[64:128, :], p_srs[1][:],
                                                rec2[64:128, :], AluOpType.mult)
                        nc.vector.scalar_tensor_tensor(
                            acc[:], hm[:], wmw[:, ls_i:ls_i + 1], acc[:],
                            AluOpType.mult, AluOpType.add)

                    return [p1, p2, p3, p4]

                for t, layers in enumerate(active_sets):
                    units = [make_unit(l, ls_idx + j) for j, l in enumerate(layers)]
                    ls_idx += len(layers)
                    NPH = 4
                    for k in range(len(units) + NPH - 1):
                        for j in range(len(units)):
                            phn = k - j
                            if 0 <= phn < NPH:
                                units[j][phn]()

                    # ---- step sync: scale acc by pcont, AllGather, update x ----
                    p_pc = ps.tile([128, T], F32, tag="H1", bufs=1, name="ps")
                    nc.tensor.matmul(p_pc[:], orowf, pcont[:], start=True, stop=True)
                    acc2 = wkf.tile([128, T], F32, bufs=1, tag="acc2", name="acc2")
                    nc.vector.tensor_tensor(acc2[:], acc[:], p_pc[:], AluOpType.mult)
                    nc.gpsimd.memset(acc[:], 0.0)
                    b_in = dram.tile([128, T], F32, tag="bin", name=f"bin{t}")
                    b_out = dram.tile([KT * 128, T], F32, tag="bout", name=f"bout{t}")
                    nc.sync.dma_start(b_in[:], acc2[:])
                    if not NO_CC:
                        nc.gpsimd.collective_compute(
                            "AllGather", mybir.AluOpType.bypass, replica_groups=groups,
                            ins=[b_in[:].opt()], outs=[b_out[:].opt()])
                        for k in range(KT):
                            nc.sync.dma_start(xg[:, k * T:(k + 1) * T],
                                              b_out[k * 128:(k + 1) * 128, :])
                    else:
                        for k in range(KT):
                            nc.sync.dma_start(xg[:, k * T:(k + 1) * T], b_in[:])
                    for k in range(KT):
                        nc.vector.tensor_tensor(xT[k][:], xT[k][:],
                                                xg[:, k * T:(k + 1) * T],
                                                AluOpType.add)
                        nc.vector.tensor_copy(xr[k][:], xT[k][:])

                    # ---- router: pcont *= 1 - sigmoid(x@rw + rb) ----
                    p_ph = ps.tile([1, T], F32, tag="H3", bufs=1, name="ps")
                    for k in range(KT):
                        nc.tensor.matmul(p_ph[:], rw[:, k:k + 1], xr[k][:],
                                         start=(k == 0), stop=(k == KT - 1))
                    ez = wkf.tile([1, T], F32, bufs=1, tag="th", name="ez")
                    nc.scalar.activation(ez[:], p_ph[:], AF.Exp, bias=rbias2[:])
                    ez1 = wkf.tile([1, T], F32, bufs=1, tag="omp", name="ez1")
                    nc.vector.tensor_scalar(ez1[:], ez[:], 1.0, 1.0,
                                            AluOpType.mult, AluOpType.add)
                    lz = wkf.tile([1, T], F32, bufs=1, tag="lz", name="lz")
                    nc.scalar.activation(lz[:], ez1[:], AF.Ln)
                    omp = wkf.tile([1, T], F32, bufs=1, tag="omp2", name="omp")
                    nc.scalar.activation(omp[:], lz[:], AF.Exp, scale=-1.0)
                    nc.vector.tensor_tensor(pcont[:], pcont[:], omp[:],
                                            AluOpType.mult)

                # ---------------- final rms + lm_head ----------------
                p_mr = ps.tile([1, T], F32, tag="H3", bufs=1, name="ps")
                for k in range(KT):
                    sqf = wk16.tile([128, T], F16, tag="sqf", name="sqf")
                    nc.scalar.activation(sqf[:], xr[k][:], AF.Square)
                    nc.tensor.matmul(p_mr[:], oc1, sqf[:],
                                     start=(k == 0), stop=(k == KT - 1))
                lnf = wkf.tile([1, T], F32, bufs=1, tag="rr", name="lnf")
                nc.scalar.activation(lnf[:], p_mr[:], AF.Ln, bias=eps1,
                                     scale=1.0 / E)
                rr15 = wkf.tile([1, T], F32, bufs=1, tag="rr15", name="rr15")
                nc.scalar.activation(rr15[:], lnf[:], AF.Exp, scale=-0.5,
                                     bias=mln15)
                rcol = []
                for i in range(NTT):
                    p_tr = ps.tile([128, 1], F32, tag="S1", bufs=1, name="ptr")
                    nc.tensor.transpose(p_tr[:], rr15[:, i * 128:(i + 1) * 128],
                                        one_f)
                    rc = st.tile([128, 1], F32, tag=f"rcol{i}", name=f"rcol{i}")
                    nc.scalar.copy(rc[:], p_tr[:])
                    rcol.append(rc)

                for i in range(NTT):
                    for v in range(NVT):
                        p_lg = ps.tile([128, 512], F32, tag="A", bufs=2, name="ps")
                        for k in range(KT):
                            nc.tensor.matmul(
                                p_lg[:], xr[k][:, i * 128:(i + 1) * 128],
                                lmsb[k][:, v * 512:(v + 1) * 512],
                                start=(k == 0), stop=(k == KT - 1))
                        lth = wk16.tile([128, 512], F16, tag="lth", name="lth")
                        nc.scalar.activation(lth[:], p_lg[:], AF.Tanh,
                                             scale=rcol[i][:])
                        nc.sync.dma_start(
                            d_out.ap()[i * 128:(i + 1) * 128,
                                       v * 512:(v + 1) * 512],
                            lth[:])

    nc.compile()
    return nc


def _rms_np(x):
    return x * (1.0 / np.sqrt(np.mean(x * x, axis=-1, keepdims=True) + EPS))


def _host_prep(idx, n_steps, wte, adapters, qkv_w, attn_proj, mlp_fc, mlp_proj,
               dep, router_w, router_b, lm_head_w):
    idx = np.asarray(idx)
    wte = np.asarray(wte, np.float32)
    adapters = np.asarray(adapters, np.float32)
    qkv_w = np.asarray(qkv_w, np.float32)
    attn_proj = np.asarray(attn_proj, np.float32)
    mlp_fc = np.asarray(mlp_fc, np.float32)
    mlp_proj = np.asarray(mlp_proj, np.float32)
    dep = np.asarray(dep, np.float32)
    router_w = np.asarray(router_w, np.float32).reshape(E, 1)
    router_b = np.asarray(router_b, np.float32).reshape(-1)
    lm_head_w = np.asarray(lm_head_w, np.float32)
    ns = int(n_steps)

    dp = np.maximum(dep, 0.0)
    depths = np.zeros((N,), np.float32)
    for _ in range(L):
        depths = (dp @ (depths + 1.0)).astype(np.float32)

    w_eff = np.zeros((ns, N), np.float32)
    active_sets = []
    for t in range(ns):
        td = t * (L / ns)
        w_all = np.exp(-np.abs(depths - np.float32(td))).astype(np.float32)
        w = np.where(w_all > 0.15, w_all, 0.0).astype(np.float32)
        w_eff[t] = w
        active_sets.append(tuple(sorted({n // G for n in range(N) if w[n] > 0})))
    active_sets = tuple(active_sets)
    n_ls = max(sum(len(a) for a in active_sets), 1)

    # fold the group-slice identity into the adapters
    adapters_f = adapters.copy()
    for n in range(N):
        g = n % G
        adapters_f[n, :, g * GD:(g + 1) * GD] += np.eye(GD, dtype=np.float32)

    # rope permutation of the q/k OUTPUT index: out j <- out (j+32)%64 within
    # each 64-block (q block and k block separately)
    perm64 = (np.arange(GD) + HD) % GD
    perm128 = np.concatenate([perm64, GD + perm64])

    w_ap = attn_proj.sum(axis=2)
    w_mp = mlp_proj.sum(axis=2)

    # per-pair weight payloads
    payload = []
    for p in range(VSH):
        adw = np.zeros((L, 128, 512), np.float16)
        qkwA = np.zeros((L, 128, 256), np.float16)
        qpwA = np.zeros((L, 128, 256), np.float16)
        vwwA = np.zeros((L, 128, 128), np.float16)
        fcwA = np.zeros((L, 128, 512), np.float16)
        wapP = np.zeros((128, L), np.float32)
        wawP = np.zeros((128, n_ls), np.float32)
        wmwP = np.zeros((128, n_ls), np.float32)
        for l in range(L):
            for o in range(2):
                n = l * G + 2 * p + o
                rows = slice(o * 64, (o + 1) * 64)
                for k in range(KT):
                    adw[l, :, k * 128 + o * 64: k * 128 + (o + 1) * 64] = \
                        adapters_f[n, :, k * 128:(k + 1) * 128].T
                # zero-padded full-128-contraction stationaries (node o's
                # weights live on its own 64 rows; the rest stay zero)
                qkwA[l, rows, o * 128:(o + 1) * 128] = qkv_w[n, 0:128, :].T
                qpwA[l, rows, o * 128:(o + 1) * 128] = qkv_w[n, 0:128, :].T[:, perm128]
                vwwA[l, rows, o * 64:(o + 1) * 64] = qkv_w[n, 128:192, :].T
                fcwA[l, rows, o * 256:(o + 1) * 256] = mlp_fc[n].T
                wapP[o * 64:(o + 1) * 64, l] = w_ap[n]
        ls = 0
        for tt, layers in enumerate(active_sets):
            for l in layers:
                for o in range(2):
                    n = l * G + 2 * p + o
                    wawP[o * 64:(o + 1) * 64, ls] = w_ap[n] * w_eff[tt, n]
                    wmwP[o * 64:(o + 1) * 64, ls] = w_mp[n] * w_eff[tt, n]
                ls += 1
        payload.append((adw, qkwA, qpwA, vwwA, fcwA, wapP, wawP, wmwP))

    # constants
    c16 = np.zeros((128, 705), np.float16)
    ob = np.zeros((128, 128), np.float32)
    ob[0:64, 0:64] = 1.0 / GD
    ob[64:128, 64:128] = 1.0 / GD
    c16[:, 0:128] = ob.astype(np.float16)
    c16[:, 128:192] = 1.0
    c16[:, 192:193] = 1.0
    c16[0, 193:257] = 1.0
    c16[1, 257:321] = 1.0
    c16[0, 321:449] = 1.0
    s_i = np.arange(128)[:, None]
    t_i = np.arange(128)[None, :]
    tri = (s_i <= t_i).astype(np.float16)
    c16[:, 449:577] = tri
    c16[:, 577:705] = tri

    inv_freq = 1.0 / (10000.0 ** (np.arange(0, GD, 2, dtype=np.float64) / GD))
    freqs = np.outer(np.arange(T), inv_freq)
    cosT = np.cos(freqs).astype(np.float32).T
    sinT = np.sin(freqs).astype(np.float32).T
    cstf = np.zeros((128, 1155), np.float32)
    for blk in range(4):
        cstf[blk * 32:(blk + 1) * 32, 0:256] = cosT
        cstf[blk * 32:(blk + 1) * 32, 256:512] = cosT
        cstf[blk * 32:(blk + 1) * 32, 512:768] = sinT * (1.0 if blk % 2 == 0 else -1.0)
        cstf[blk * 32:(blk + 1) * 32, 768:1024] = sinT * (1.0 if blk % 2 == 0 else -1.0)
    cstf[:, 1024] = EPS
    cstf[0, 1025] = 1.0
    cstf[0, 1026] = -np.log(15.0)
    cstf[0, 1027:1155] = 1.0

    rwP = np.zeros((128, KT), np.float16)
    for k in range(KT):
        rwP[:, k] = router_w[k * 128:(k + 1) * 128, 0].astype(np.float16)
    rbias2 = np.full((1, 1), np.float32(router_b[0]), np.float32)

    x0 = _rms_np(wte[idx])  # (B, T, E) f32

    in_maps = []
    for c in range(NC):
        b, p = c // VSH, c % VSH
        lo = p * VW
        hi = min(lo + VW, V)
        lmt = np.zeros((E, VQ), np.float16)
        lmt[:, 0:hi - lo] = lm_head_w[lo:hi, :].T.astype(np.float16)
        adw, qkwA, qpwA, vwwA, fcwA, wapP, wawP, wmwP = payload[p]
        in_maps.append({
            "x0t": np.ascontiguousarray(x0[b].T), "adw": adw, "qkw": qkwA,
            "qpw": qpwA, "vww": vwwA, "fcw": fcwA, "c16": c16, "cstf": cstf,
            "wapP": wapP, "wawP": wawP, "wmwP": wmwP, "rwP": rwP,
            "rbias2": rbias2, "lmt": lmt,
        })
    return active_sets, in_maps


def kernel(idx, n_steps, wte, adapters, qkv_w, attn_proj, mlp_fc, mlp_proj,
           dep, router_w, router_b, lm_head_w):
    active_sets, in_maps = _host_prep(
        idx, n_steps, wte, adapters, qkv_w, attn_proj, mlp_fc, mlp_proj,
        dep, router_w, router_b, lm_head_w)

    if active_sets not in _PROGRAM_CACHE:
        _PROGRAM_CACHE[active_sets] = _build_program(active_sets)
    nc = _PROGRAM_CACHE[active_sets]

    trace = bool(int(os.environ.get("BASS_KERNEL_TRACE", "0")))
    res = run_bass_kernel_spmd(nc, in_maps, list(range(NC)), trace=trace)
    if trace and res.exec_time_ns is not None:
        print(f"HW exec time: {res.exec_time_ns} ns")

    out = np.zeros((B, T, V), np.float32)
    for c in range(NC):
        b, p = c // VSH, c % VSH
        lo = p * VW
        hi = min(lo + VW, V)
        out[b, :, lo:hi] = 15.0 * res.results[c]["out_lg"][:, 0:hi - lo].astype(np.float32)
    return out


# revision 24
# speedup vs baseline: 3.9901x; 1.0354x over previous
"""Trainium2 Bass kernel for nn_BG_ALRT_62921270886438 (moe_routing).

Sharding v2: core c -> (batch b = c // 4, pair p = c % 4).  Each core computes
only its pair's two nodes per active layer; the group-wise scatter-add target
of pair p is exactly E-rows [128p, 128p+128), so the per-step x update needs
only an AllGather (groups {0-3}, {4-7}) of each core's [128, T] acc slice.
lm_head is vocab-sharded 4 ways within each batch group (same output contract
as v1).  Matmuls run in fp16 (1 cycle/row vs 4 for fp32), x state stays fp32.

Self-contained: only numpy + the concourse toolchain on sys.path.
"""
import os

import numpy as np

import concourse.bacc as bacc
import concourse.tile as tile
from concourse import mybir
from concourse.alu_op_type import AluOpType
from concourse.bass_utils import run_bass_kernel_spmd

AF = mybir.ActivationFunctionType
F32 = mybir.dt.float32
F16 = mybir.dt.float16

B, T, E, G, GD, L, N, V = 2, 256, 512, 8, 64, 8, 64, 50257
HD = GD // 2          # 32, rope half
NC = 8                # cores
VSH = 4               # vocab shards per batch group
VW = (V + VSH - 1) // VSH          # 12565 raw shard width
VQ = ((VW + 511) // 512) * 512     # 12800 padded shard width
EPS = float(np.finfo(np.float32).eps)
KT = E // 128         # 4 contraction tiles over E
NVT = VQ // 512       # 25 vocab tiles of 512
NTT = T // 128        # 2 token tiles

_PROGRAM_CACHE = {}


def _tune_act_tables(arch):
    """Steer the act-table-load pass to one set for the whole step loop.

    All step-loop activations (square/ln/exp/relu/copy/identity) exist in
    `natural_log_exp_and_others`; tanh (lm head only) in `exp_and_others`.
    The pass picks the first listed set containing the function, which makes
    Ln and Exp resolve to different sets and thrash the single active table
    (~1.3us per reload).  get_activation_tables is functools.cache'd and
    returns the same dict object, so removing the overlapping functions from
    every other set (pure removals - set ids stay aligned with act_info.json)
    makes the combined set the unique choice.
    """
    from concourse.hw_specs import get_activation_tables
    tabs = get_activation_tables(arch)
    combined = tabs.get("natural_log_exp_and_others")
    if not combined:
        return
    for name, fns in tabs.items():
        if name != "natural_log_exp_and_others":
            fns.difference_update(combined)


def _build_program(active_sets):
    """active_sets: tuple of tuples - active layer list per step."""
    nc = bacc.Bacc("TRN2", target_bir_lowering=False, debug=False, num_devices=NC)
    _tune_act_tables(nc.m.arch)
    n_ls = max(sum(len(a) for a in active_sets), 1)
    groups = [[0, 1, 2, 3], [4, 5, 6, 7]]

    d_x0t = nc.dram_tensor("x0t", [E, T], F32, kind="ExternalInput")
    d_adw = nc.dram_tensor("adw", [L, 128, 512], F16, kind="ExternalInput")
    d_qkw = nc.dram_tensor("qkw", [L, 128, 256], F16, kind="ExternalInput")
    d_qpw = nc.dram_tensor("qpw", [L, 128, 256], F16, kind="ExternalInput")
    d_vww = nc.dram_tensor("vww", [L, 128, 128], F16, kind="ExternalInput")
    d_fcw = nc.dram_tensor("fcw", [L, 128, 512], F16, kind="ExternalInput")
    d_c16 = nc.dram_tensor("c16", [128, 705], F16, kind="ExternalInput")
    d_cf = nc.dram_tensor("cstf", [128, 1155], F32, kind="ExternalInput")
    d_wap = nc.dram_tensor("wapP", [128, L], F32, kind="ExternalInput")
    d_waw = nc.dram_tensor("wawP", [128, n_ls], F32, kind="ExternalInput")
    d_wmw = nc.dram_tensor("wmwP", [128, n_ls], F32, kind="ExternalInput")
    d_rw = nc.dram_tensor("rwP", [128, KT], F16, kind="ExternalInput")
    d_rb = nc.dram_tensor("rbias2", [1, 1], F32, kind="ExternalInput")
    d_lm = nc.dram_tensor("lmt", [E, VQ], F16, kind="ExternalInput")
    d_out = nc.dram_tensor("out_lg", [T, VQ], F16, kind="ExternalOutput")

    with tile.TileContext(nc) as tc:
        with tc.tile_pool(name="cst", bufs=1) as cst, \
             tc.tile_pool(name="st", bufs=1) as st, \
             tc.tile_pool(name="wk16", bufs=3) as wk16, \
             tc.tile_pool(name="wkf", bufs=2) as wkf, \
             tc.tile_pool(name="vsb", bufs=4) as vsb, \
             tc.tile_pool(name="ps", bufs=1, space="PSUM") as ps, \
             tc.tile_pool(name="dram", bufs=20, space="DRAM") as dram:

            # ---------------- constants ----------------
            c16 = cst.tile([128, 705], F16, tag="c16", name="c16")
            nc.sync.dma_start(c16[:], d_c16.ap())
            oblk = c16[:, 0:128]            # block-diag(64) of 1/64
            ocol = c16[:, 128:192]          # (128,64) ones
            oc1 = c16[:, 192:193]           # (128,1) ones
            sel2 = c16[0:2, 193:321]        # row0 -> parts 0:64, row1 -> 64:128
            onesrow = c16[0:1, 321:449]     # (1,128) ones
            tri2 = c16[:, 449:705]          # [tri | tri] fp16

            cf = cst.tile([128, 1155], F32, tag="cf", name="cf")
            nc.sync.dma_start(cf[:], d_cf.ap())
            CC2 = cf[:, 0:512]              # [C | C]
            SS2 = cf[:, 512:1024]           # [S | S]
            eps128 = cf[:, 1024:1025]
            eps1 = cf[0:1, 1024:1025]
            one_f = cf[0:1, 1025:1026]      # 1.0 (transpose identity)
            mln15 = cf[0:1, 1026:1027]      # -ln(15)
            orowf = cf[0:1, 1027:1155]      # (1,128) ones f32

            wap = cst.tile([128, L], F32, tag="wap", name="wap")
            nc.sync.dma_start(wap[:], d_wap.ap())
            waw = cst.tile([128, n_ls], F32, tag="waw", name="waw")
            nc.sync.dma_start(waw[:], d_waw.ap())
            wmw = cst.tile([128, n_ls], F32, tag="wmw", name="wmw")
            nc.sync.dma_start(wmw[:], d_wmw.ap())
            rw = cst.tile([128, KT], F16, tag="rw", name="rw")
            nc.sync.dma_start(rw[:], d_rw.ap())
            rbias2 = cst.tile([1, 1], F32, tag="rbias2", name="rbias2")
            nc.sync.dma_start(rbias2[:], d_rb.ap())

            adw, qkw, qpw, vww, fcw = [], [], [], [], []
            for l in range(L):
                a_t = cst.tile([128, 512], F16, tag=f"adw{l}", name=f"adw{l}")
                nc.sync.dma_start(a_t[:], d_adw.ap()[l])
                adw.append(a_t)
                q_t = cst.tile([128, 256], F16, tag=f"qkw{l}", name=f"qkw{l}")
                nc.sync.dma_start(q_t[:], d_qkw.ap()[l])
                qkw.append(q_t)
                p_t = cst.tile([128, 256], F16, tag=f"qpw{l}", name=f"qpw{l}")
                nc.sync.dma_start(p_t[:], d_qpw.ap()[l])
                qpw.append(p_t)
                v_t = cst.tile([128, 128], F16, tag=f"vww{l}", name=f"vww{l}")
                nc.sync.dma_start(v_t[:], d_vww.ap()[l])
                vww.append(v_t)
                f_t = cst.tile([128, 512], F16, tag=f"fcw{l}", name=f"fcw{l}")
                nc.sync.dma_start(f_t[:], d_fcw.ap()[l])
                fcw.append(f_t)

            # lm_head weights: full shard resident in SBUF, chunked DMA so the
            # prefetch never head-of-line blocks the per-step bounce DMAs.
            lmsb = []
            LCH = 1600
            for k in range(KT):
                t_ = cst.tile([128, VQ], F16, tag=f"lm{k}", name=f"lm{k}")
                lmsb.append(t_)
                for c0 in range(0, VQ, LCH):
                    nc.sync.dma_start(
                        t_[:, c0:c0 + LCH],
                        d_lm.ap()[k * 128:(k + 1) * 128, c0:c0 + LCH])

            # ---------------- state ----------------
            xT = [st.tile([128, T], F32, tag=f"xT{k}", name=f"xT{k}") for k in range(KT)]
            xr = [st.tile([128, T], F16, tag=f"xr{k}", name=f"xr{k}") for k in range(KT)]
            acc = st.tile([128, T], F32, tag="acc", name="acc")
            xg = st.tile([128, KT * T], F16, tag="xg", name="xg")
            pcont = st.tile([1, T], F32, tag="pcont", name="pcont")
            nc.vector.memset(pcont[:], 1.0)
            nc.gpsimd.memset(acc[:], 0.0)

            # initial x (rms applied host-side)
            for k in range(KT):
                nc.sync.dma_start(xT[k][:], d_x0t.ap()[k * 128:(k + 1) * 128, :])
                with nc.allow_low_precision(reason="fp16 compute"):
                    nc.vector.tensor_copy(xr[k][:], xT[k][:])

            # CC warm-up: dummy AllGather so the first real one is cheap
            NO_CC = bool(int(os.environ.get("BASS_V2_NO_CC", "0")))
            db_in = dram.tile([128, 8], F32, tag="dbi", name="dbi")
            db_out = dram.tile([512, 8], F32, tag="dbo", name="dbo")
            nc.sync.dma_start(db_in[:], cf[:, 0:8])
            if not NO_CC:
                nc.gpsimd.collective_compute(
                    "AllGather", mybir.AluOpType.bypass, replica_groups=groups,
                    ins=[db_in[:].opt()], outs=[db_out[:].opt()])

            ls_idx = 0
            with nc.allow_low_precision(reason="fp16 compute"):
                def make_unit(l, ls_i):
                    """Four emission phases for one (layer, pair) unit; the
                    step loop staggers phases across the step's units so each
                    engine queue interleaves independent work."""
                    S = {}

                    def p1():
                        H1 = ps.tile([128, 2 * T], F32, tag="H1", bufs=1, name="ps")
                        S["p_s1"] = H1[:, T:2 * T]
                        p_xi = H1[:, 0:T]
                        for k in range(KT):
                            nc.tensor.matmul(
                                p_xi[:], adw[l][:, k * 128:(k + 1) * 128],
                                xr[k][:], start=(k == 0), stop=(k == KT - 1))
                        xi = wk16.tile([128, T], F16, tag="xi", name="xi")
                        nc.vector.tensor_copy(xi[:], p_xi[:])
                        S["xi"] = xi
                        p_v = ps.tile([128, T], F32, tag="S1", bufs=1, name="ps")
                        v_sb = [None, None]
                        for s in range(2):
                            nc.tensor.matmul(
                                p_v[:, s * 128:(s + 1) * 128],
                                xi[:, s * 128:(s + 1) * 128],
                                vww[l][:], start=True, stop=True)
                            vt = vsb.tile([128, 130], F16, tag="vt", name="vt")
                            if s == 0:
                                nc.scalar.copy(vt[:, 0:64], p_v[:, 0:64])
                                nc.scalar.copy(vt[:, 65:129], p_v[:, 64:128])
                            else:
                                nc.vector.tensor_copy(vt[:, 0:64], p_v[:, 128:192])
                                nc.vector.tensor_copy(vt[:, 65:129], p_v[:, 192:256])
                            nc.gpsimd.memset(vt[:, 64:65], 1.0)
                            nc.gpsimd.memset(vt[:, 129:130], 1.0)
                            v_sb[s] = vt
                        S["v_sb"] = v_sb
                        p_qk = ps.tile([128, 2 * T], F32, tag="A", bufs=2, name="ps")
                        p_qp = ps.tile([128, 2 * T], F32, tag="A", bufs=2, name="ps")
                        for o in range(2):
                            nc.tensor.matmul(p_qk[:, o * T:(o + 1) * T],
                                             qkw[l][:, o * 128:(o + 1) * 128],
                                             xi[:], start=True, stop=True)
                            nc.tensor.matmul(p_qp[:, o * T:(o + 1) * T],
                                             qpw[l][:, o * 128:(o + 1) * 128],
                                             xi[:], start=True, stop=True)
                        S["p_qk"], S["p_qp"] = p_qk, p_qp

                    def p2():
                        p_qk, p_qp = S["p_qk"], S["p_qp"]
                        sq = wk16.tile([128, 2 * T], F16, tag="sq", name="sq")
                        nc.scalar.activation(sq[:], p_qk[:], AF.Square)
                        p_ms = ps.tile([128, 2 * T], F32, tag="A", bufs=2, name="ps")
                        nc.tensor.matmul(p_ms[:], oblk, sq[:], start=True, stop=True)
                        lnm = wkf.tile([128, 2 * T], F32, tag="srt", name="lnm")
                        nc.scalar.activation(lnm[:], p_ms[:], AF.Ln, bias=eps128)
                        rsq = wk16.tile([128, 2 * T], F16, tag="rsq", name="rsq")
                        nc.scalar.activation(rsq[:], lnm[:], AF.Exp, scale=-0.5)
                        t1 = wk16.tile([128, 2 * T], F16, bufs=2, tag="t1", name="t1")
                        nc.vector.tensor_tensor(t1[:], p_qk[:], CC2, AluOpType.mult)
                        t2 = wk16.tile([128, 2 * T], F16, bufs=2, tag="t2", name="t2")
                        nc.vector.tensor_tensor(t2[:], p_qp[:], SS2, AluOpType.mult)
                        rop = wk16.tile([128, 2 * T], F16, bufs=2, tag="rop", name="rop")
                        nc.vector.tensor_tensor(rop[:], t1[:], t2[:], AluOpType.add)
                        qt = wk16.tile([128, T], F16, tag="qt", name="qt")
                        kt = wk16.tile([128, 2 * T], F16, tag="kt", name="kt")
                        nc.gpsimd.memset(kt[64:128, 0:T], 0.0)
                        nc.gpsimd.memset(kt[0:64, T:2 * T], 0.0)
                        for o in range(2):
                            orows = slice(64 * o, 64 * o + 64)
                            nc.vector.tensor_tensor(
                                qt[orows, :], rop[0:64, o * T:(o + 1) * T],
                                rsq[0:64, o * T:(o + 1) * T], AluOpType.mult)
                            nc.vector.tensor_tensor(
                                kt[orows, o * T:(o + 1) * T],
                                rop[64:128, o * T:(o + 1) * T],
                                rsq[64:128, o * T:(o + 1) * T], AluOpType.mult)
                        p_s0 = ps.tile([128, 2 * T], F32, tag="A", bufs=2, name="ps")
                        p_s1 = S["p_s1"]
                        for o in range(2):
                            nc.tensor.matmul(p_s0[:, o * T:(o + 1) * T],
                                             kt[:, o * T:o * T + 128], qt[:],
                                             start=True, stop=True)
                            nc.tensor.matmul(p_s1[:, o * 128:(o + 1) * 128],
                                             kt[:, o * T + 128:(o + 1) * T],
                                             qt[:, 128:256],
                                             start=True, stop=True)
                        em0 = wk16.tile([128, 2 * T], F16, tag="em0", name="em0")
                        nc.scalar.activation(em0[:], p_s0[:], AF.Exp, scale=0.125)
                        em1 = wk16.tile([128, T], F16, tag="em1", name="em1")
                        nc.scalar.activation(em1[:], p_s1[:], AF.Exp, scale=0.125)
                        m0 = wk16.tile([128, T], F16, tag="m0", name="m0")
                        nc.gpsimd.tensor_tensor(m0[:, 0:128], em0[:, 0:128],
                                                tri2[:, 0:128], AluOpType.mult)
                        nc.gpsimd.tensor_tensor(m0[:, 128:256], em0[:, T:T + 128],
                                                tri2[:, 0:128], AluOpType.mult)
                        m1 = wk16.tile([128, T], F16, tag="m1", name="m1")
                        nc.gpsimd.tensor_tensor(m1[:], em1[:], tri2, AluOpType.mult)
                        S["em0"], S["m0"], S["m1"] = em0, m0, m1

                    def p3():
                        em0, m0, m1 = S["em0"], S["m0"], S["m1"]
                        v_sb, xi = S["v_sb"], S["xi"]
                        S2 = ps.tile([128, 2 * T], F32, tag="S2", bufs=1, name="ps")
                        p_att = [S2[0:65, 0:T], S2[0:65, T:2 * T]]
                        for o in range(2):
                            pa = p_att[o]
                            nc.tensor.matmul(pa[:, 0:128],
                                             v_sb[0][:, o * 65:(o + 1) * 65],
                                             m0[:, o * 128:(o + 1) * 128],
                                             start=True, stop=True)
                            nc.tensor.matmul(pa[:, 128:256],
                                             v_sb[0][:, o * 65:(o + 1) * 65],
                                             em0[:, o * T + 128:(o + 1) * T],
                                             start=True, stop=False)
                            nc.tensor.matmul(pa[:, 128:256],
                                             v_sb[1][:, o * 65:(o + 1) * 65],
                                             m1[:, o * 128:(o + 1) * 128],
                                             start=False, stop=True)
                        rcl = wkf.tile([1, 2 * T], F32, bufs=1, tag="rcl", name="rcl")
                        nc.scalar.activation(rcl[:], S2[64:65, 0:2 * T], AF.Ln)
                        rc2 = wkf.tile([1, 2 * T], F32, bufs=1, tag="rc2", name="rc2")
                        nc.scalar.activation(rc2[:], rcl[:], AF.Exp, scale=-1.0)
                        H2 = ps.tile([128, 2 * T], F32, tag="H2", bufs=1, name="ps")
                        nc.tensor.matmul(H2[:], orowf, rc2[:], start=True, stop=True)
                        att_sb = wk16.tile([128, T], F16, tag="att", name="att")
                        nc.scalar.copy(att_sb[0:64, :], p_att[0][0:64, :])
                        nc.scalar.copy(att_sb[64:128, :], p_att[1][0:64, :])
                        tt = wk16.tile([128, T], F16, tag="tt", name="tt")
                        nc.vector.tensor_tensor(tt[0:64, :], att_sb[0:64, :],
                                                H2[0:64, 0:T], AluOpType.mult)
                        nc.vector.tensor_tensor(tt[64:128, :], att_sb[64:128, :],
                                                H2[64:128, T:2 * T], AluOpType.mult)
                        xim = wk16.tile([128, T], F16, tag="xim", name="xim")
                        nc.vector.scalar_tensor_tensor(
                            xim[:], tt[:], wap[:, l:l + 1], xi[:],
                            AluOpType.mult, AluOpType.add)
                        nc.vector.scalar_tensor_tensor(
                            acc[:], tt[:], waw[:, ls_i:ls_i + 1], acc[:],
                            AluOpType.mult, AluOpType.add)
                        S["xim"] = xim

                    def p4():
                        xim = S["xim"]
                        sqm = wk16.tile([128, T], F16, tag="sqm", name="sqm")
                        nc.gpsimd.tensor_tensor(sqm[:], xim[:], xim[:],
                                                AluOpType.mult)
                        p_mq = ps.tile([128, T], F32, tag="H3", bufs=1, name="ps")
                        nc.tensor.matmul(p_mq[:], oblk, sqm[:], start=True, stop=True)
                        lnm2 = wkf.tile([128, T], F32, bufs=1, tag="pre", name="lnm2")
                        nc.scalar.activation(lnm2[:], p_mq[:], AF.Ln, bias=eps128)
                        rec2 = wk16.tile([128, T], F16, tag="rec2", name="rec2")
                        nc.scalar.activation(rec2[:], lnm2[:], AF.Exp, scale=-1.0)
                        p_sr01 = ps.tile([64, 2 * T], F32, tag="H3", bufs=1, name="ps")
                        p_srs = [p_sr01[:, 0:T], p_sr01[:, T:2 * T]]
                        for o in range(2):
                            p_fc = ps.tile([128, 2 * T], F32, tag="B", bufs=1, name="ps")
                            for h in range(2):
                                nc.tensor.matmul(
                                    p_fc[:, h * T:(h + 1) * T],
                                    fcw[l][:, o * 256 + h * 128:o * 256 + (h + 1) * 128],
                                    xim[:], start=True, stop=True)
                            frel = wk16.tile([128, 2 * T], F16, tag="frel", name="frel")
                            nc.scalar.activation(frel[:], p_fc[:], AF.Relu)
                            rsq2 = wk16.tile([128, 2 * T], F16, tag="rsq2", name="rsq2")
                            nc.gpsimd.tensor_tensor(rsq2[:], frel[:], frel[:],
                                                    AluOpType.mult)
                            nc.tensor.matmul(p_srs[o][:], ocol, rsq2[:, 0:T],
                                             start=True, stop=False)
                            nc.tensor.matmul(p_srs[o][:], ocol, rsq2[:, T:2 * T],
                                             start=False, stop=True)
                        hm = wk16.tile([128, T], F16, tag="hm", name="hm")
                        nc.vector.tensor_tensor(hm[0:64, :], p_srs[0][:],
                                                rec2[0:64, :], AluOpType.mult)
                        nc.vector.tensor_tensor(hm[64:128, :], p_srs[1][:],
                                                rec2[64:128, :], AluOpType.mult)
                        nc.vector.scalar_tensor_tensor(
                            acc[:], hm[:], wmw[:, ls_i:ls_i + 1], acc[:],
                            AluOpType.mult, AluOpType.add)

                    return [p1, p2, p3, p4]

                for t, layers in enumerate(active_sets):
                    units = [make_unit(l, ls_idx + j) for j, l in enumerate(layers)]
                    ls_idx += len(layers)
                    NPH = 4
                    for k in range(len(units) + NPH - 1):
                        for j in range(len(units)):
                            phn = k - j
                            if 0 <= phn < NPH:
                                units[j][phn]()

                    # ---- step sync: scale acc by pcont, AllGather, update x ----
                    p_pc = ps.tile([128, T], F32, tag="H1", bufs=1, name="ps")
                    nc.tensor.matmul(p_pc[:], orowf, pcont[:], start=True, stop=True)
                    acc2 = wk16.tile([128, T], F16, bufs=1, tag="acc2", name="acc2")
                    nc.vector.tensor_tensor(acc2[:], acc[:], p_pc[:], AluOpType.mult)
                    nc.gpsimd.memset(acc[:], 0.0)
                    b_in = dram.tile([128, T], F16, tag="bin", name=f"bin{t}")
                    b_out = dram.tile([KT * 128, T], F16, tag="bout", name=f"bout{t}")
                    nc.sync.dma_start(b_in[:], acc2[:])
                    if not NO_CC:
                        nc.gpsimd.collective_compute(
                            "AllGather", mybir.AluOpType.bypass, replica_groups=groups,
                            ins=[b_in[:].opt()], outs=[b_out[:].opt()])
                        for k in range(KT):
                            nc.sync.dma_start(xg[:, k * T:(k + 1) * T],
                                              b_out[k * 128:(k + 1) * 128, :])
                    else:
                        for k in range(KT):
                            nc.sync.dma_start(xg[:, k * T:(k + 1) * T], b_in[:])
                    for k in range(KT):
                        nc.vector.tensor_tensor(xT[k][:], xT[k][:],
                                                xg[:, k * T:(k + 1) * T],
                                                AluOpType.add)
                        nc.vector.tensor_copy(xr[k][:], xT[k][:])

                    # ---- router: pcont *= 1 - sigmoid(x@rw + rb) ----
                    p_ph = ps.tile([1, T], F32, tag="H3", bufs=1, name="ps")
                    for k in range(KT):
                        nc.tensor.matmul(p_ph[:], rw[:, k:k + 1], xr[k][:],
                                         start=(k == 0), stop=(k == KT - 1))
                    ez = wkf.tile([1, T], F32, bufs=1, tag="th", name="ez")
                    nc.scalar.activation(ez[:], p_ph[:], AF.Exp, bias=rbias2[:])
                    ez1 = wkf.tile([1, T], F32, bufs=1, tag="omp", name="ez1")
                    nc.vector.tensor_scalar(ez1[:], ez[:], 1.0, 1.0,
                                            AluOpType.mult, AluOpType.add)
                    lz = wkf.tile([1, T], F32, bufs=1, tag="lz", name="lz")
                    nc.scalar.activation(lz[:], ez1[:], AF.Ln)
                    omp = wkf.tile([1, T], F32, bufs=1, tag="omp2", name="omp")
                    nc.scalar.activation(omp[:], lz[:], AF.Exp, scale=-1.0)
                    nc.vector.tensor_tensor(pcont[:], pcont[:], omp[:],
                                            AluOpType.mult)

                # ---------------- final rms + lm_head ----------------
                p_mr = ps.tile([1, T], F32, tag="H3", bufs=1, name="ps")
                for k in range(KT):
                    sqf = wk16.tile([128, T], F16, tag="sqf", name="sqf")
                    nc.scalar.activation(sqf[:], xr[k][:], AF.Square)
                    nc.tensor.matmul(p_mr[:], oc1, sqf[:],
                                     start=(k == 0), stop=(k == KT - 1))
                lnf = wkf.tile([1, T], F32, bufs=1, tag="rr", name="lnf")
                nc.scalar.activation(lnf[:], p_mr[:], AF.Ln, bias=eps1,
                                     scale=1.0 / E)
                rr15 = wkf.tile([1, T], F32, bufs=1, tag="rr15", name="rr15")
                nc.scalar.activation(rr15[:], lnf[:], AF.Exp, scale=-0.5,
                                     bias=mln15)
                rcol = []
                for i in range(NTT):
                    p_tr = ps.tile([128, 1], F32, tag="S1", bufs=1, name="ptr")
                    nc.tensor.transpose(p_tr[:], rr15[:, i * 128:(i + 1) * 128],
                                        one_f)
                    rc = st.tile([128, 1], F32, tag=f"rcol{i}", name=f"rcol{i}")
                    nc.scalar.copy(rc[:], p_tr[:])
                    rcol.append(rc)

                for i in range(NTT):
                    for v in range(NVT):
                        p_lg = ps.tile([128, 512], F32, tag="A", bufs=2, name="ps")
                        for k in range(KT):
                            nc.tensor.matmul(
                                p_lg[:], xr[k][:, i * 128:(i + 1) * 128],
                                lmsb[k][:, v * 512:(v + 1) * 512],
                                start=(k == 0), stop=(k == KT - 1))
                        lth = wk16.tile([128, 512], F16, tag="lth", name="lth")
                        nc.scalar.activation(lth[:], p_lg[:], AF.Tanh,
                                             scale=rcol[i][:])
                        nc.sync.dma_start(
                            d_out.ap()[i * 128:(i + 1) * 128,
                                       v * 512:(v + 1) * 512],
                            lth[:])

    nc.compile()
    return nc


def _rms_np(x):
    return x * (1.0 / np.sqrt(np.mean(x * x, axis=-1, keepdims=True) + EPS))


def _host_prep(idx, n_steps, wte, adapters, qkv_w, attn_proj, mlp_fc, mlp_proj,
               dep, router_w, router_b, lm_head_w):
    idx = np.asarray(idx)
    wte = np.asarray(wte, np.float32)
    adapters = np.asarray(adapters, np.float32)
    qkv_w = np.asarray(qkv_w, np.float32)
    attn_proj = np.asarray(attn_proj, np.float32)
    mlp_fc = np.asarray(mlp_fc, np.float32)
    mlp_proj = np.asarray(mlp_proj, np.float32)
    dep = np.asarray(dep, np.float32)
    router_w = np.asarray(router_w, np.float32).reshape(E, 1)
    router_b = np.asarray(router_b, np.float32).reshape(-1)
    lm_head_w = np.asarray(lm_head_w, np.float32)
    ns = int(n_steps)

    dp = np.maximum(dep, 0.0)
    depths = np.zeros((N,), np.float32)
    for _ in range(L):
        depths = (dp @ (depths + 1.0)).astype(np.float32)

    w_eff = np.zeros((ns, N), np.float32)
    active_sets = []
    for t in range(ns):
        td = t * (L / ns)
        w_all = np.exp(-np.abs(depths - np.float32(td))).astype(np.float32)
        w = np.where(w_all > 0.15, w_all, 0.0).astype(np.float32)
        w_eff[t] = w
        active_sets.append(tuple(sorted({n // G for n in range(N) if w[n] > 0})))
    active_sets = tuple(active_sets)
    n_ls = max(sum(len(a) for a in active_sets), 1)

    # fold the group-slice identity into the adapters
    adapters_f = adapters.copy()
    for n in range(N):
        g = n % G
        adapters_f[n, :, g * GD:(g + 1) * GD] += np.eye(GD, dtype=np.float32)

    # rope permutation of the q/k OUTPUT index: out j <- out (j+32)%64 within
    # each 64-block (q block and k block separately)
    perm64 = (np.arange(GD) + HD) % GD
    perm128 = np.concatenate([perm64, GD + perm64])

    w_ap = attn_proj.sum(axis=2)
    w_mp = mlp_proj.sum(axis=2)

    # per-pair weight payloads
    payload = []
    for p in range(VSH):
        adw = np.zeros((L, 128, 512), np.float16)
        qkwA = np.zeros((L, 128, 256), np.float16)
        qpwA = np.zeros((L, 128, 256), np.float16)
        vwwA = np.zeros((L, 128, 128), np.float16)
        fcwA = np.zeros((L, 128, 512), np.float16)
        wapP = np.zeros((128, L), np.float32)
        wawP = np.zeros((128, n_ls), np.float32)
        wmwP = np.zeros((128, n_ls), np.float32)
        for l in range(L):
            for o in range(2):
                n = l * G + 2 * p + o
                rows = slice(o * 64, (o + 1) * 64)
                for k in range(KT):
                    adw[l, :, k * 128 + o * 64: k * 128 + (o + 1) * 64] = \
                        adapters_f[n, :, k * 128:(k + 1) * 128].T
                # zero-padded full-128-contraction stationaries (node o's
                # weights live on its own 64 rows; the rest stay zero)
                qkwA[l, rows, o * 128:(o + 1) * 128] = qkv_w[n, 0:128, :].T
                qpwA[l, rows, o * 128:(o + 1) * 128] = qkv_w[n, 0:128, :].T[:, perm128]
                vwwA[l, rows, o * 64:(o + 1) * 64] = qkv_w[n, 128:192, :].T
                fcwA[l, rows, o * 256:(o + 1) * 256] = mlp_fc[n].T
                wapP[o * 64:(o + 1) * 64, l] = w_ap[n]
        ls = 0
        for tt, layers in enumerate(active_sets):
            for l in layers:
                for o in range(2):
                    n = l * G + 2 * p + o
                    wawP[o * 64:(o + 1) * 64, ls] = w_ap[n] * w_eff[tt, n]
                    wmwP[o * 64:(o + 1) * 64, ls] = w_mp[n] * w_eff[tt, n]
                ls += 1
        payload.append((adw, qkwA, qpwA, vwwA, fcwA, wapP, wawP, wmwP))

    # constants
    c16 = np.zeros((128, 705), np.float16)
    ob = np.zeros((128, 128), np.float32)
    ob[0:64, 0:64] = 1.0 / GD
    ob[64:128, 64:128] = 1.0 / GD
    c16[:, 0:128] = ob.astype(np.float16)
    c16[:, 128:192] = 1.0
    c16[:, 192:193] = 1.0
    c16[0, 193:257] = 1.0
    c16[1, 257:321] = 1.0
    c16[0, 321:449] = 1.0
    s_i = np.arange(128)[:, None]
    t_i = np.arange(128)[None, :]
    tri = (s_i <= t_i).astype(np.float16)
    c16[:, 449:577] = tri
    c16[:, 577:705] = tri

    inv_freq = 1.0 / (10000.0 ** (np.arange(0, GD, 2, dtype=np.float64) / GD))
    freqs = np.outer(np.arange(T), inv_freq)
    cosT = np.cos(freqs).astype(np.float32).T
    sinT = np.sin(freqs).astype(np.float32).T
    cstf = np.zeros((128, 1155), np.float32)
    for blk in range(4):
        cstf[blk * 32:(blk + 1) * 32, 0:256] = cosT
        cstf[blk * 32:(blk + 1) * 32, 256:512] = cosT
        cstf[blk * 32:(blk + 1) * 32, 512:768] = sinT * (1.0 if blk % 2 == 0 else -1.0)
        cstf[blk * 32:(blk + 1) * 32, 768:1024] = sinT * (1.0 if blk % 2 == 0 else -1.0)
    cstf[:, 1024] = EPS
    cstf[0, 1025] = 1.0
    cstf[0, 1026] = -np.log(15.0)
    cstf[0, 1027:1155] = 1.0

    rwP = np.zeros((128, KT), np.float16)
    for k in range(KT):
        rwP[:, k] = router_w[k * 128:(k + 1) * 128, 0].astype(np.float16)
    rbias2 = np.full((1, 1), np.float32(router_b[0]), np.float32)

    x0 = _rms_np(wte[idx])  # (B, T, E) f32

    in_maps = []
    for c in range(NC):
        b, p = c // VSH, c % VSH
        lo = p * VW
        hi = min(lo + VW, V)
        lmt = np.zeros((E, VQ), np.float16)
        lmt[:, 0:hi - lo] = lm_head_w[lo:hi, :].T.astype(np.float16)
        adw, qkwA, qpwA, vwwA, fcwA, wapP, wawP, wmwP = payload[p]
        in_maps.append({
            "x0t": np.ascontiguousarray(x0[b].T), "adw": adw, "qkw": qkwA,
            "qpw": qpwA, "vww": vwwA, "fcw": fcwA, "c16": c16, "cstf": cstf,
            "wapP": wapP, "wawP": wawP, "wmwP": wmwP, "rwP": rwP,
            "rbias2": rbias2, "lmt": lmt,
        })
    return active_sets, in_maps


def kernel(idx, n_steps, wte, adapters, qkv_w, attn_proj, mlp_fc, mlp_proj,
           dep, router_w, router_b, lm_head_w):
    active_sets, in_maps = _host_prep(
        idx, n_steps, wte, adapters, qkv_w, attn_proj, mlp_fc, mlp_proj,
        dep, router_w, router_b, lm_head_w)

    if active_sets not in _PROGRAM_CACHE:
        _PROGRAM_CACHE[active_sets] = _build_program(active_sets)
    nc = _PROGRAM_CACHE[active_sets]

    trace = bool(int(os.environ.get("BASS_KERNEL_TRACE", "0")))
    res = run_bass_kernel_spmd(nc, in_maps, list(range(NC)), trace=trace)
    if trace and res.exec_time_ns is not None:
        print(f"HW exec time: {res.exec_time_ns} ns")

    out = np.zeros((B, T, V), np.float32)
    for c in range(NC):
        b, p = c // VSH, c % VSH
        lo = p * VW
        hi = min(lo + VW, V)
        out[b, :, lo:hi] = 15.0 * res.results[c]["out_lg"][:, 0:hi - lo].astype(np.float32)
    return out
